# revision 74
# baseline (speedup 1.0000x reference)
"""Trainium2 Bass kernel for DeformableMultiHeadedAttention.

Data-parallel over batch B=8 across 8 NeuronCores (one batch element per
core, identical programs, no collectives). Heavy matmuls run fp8-e4m3
DoubleRow (0.5 PE cycles/row) wherever a numpy precision study showed the
final rel-err stays ~0.006 (tolerance 2e-2); V projection and the final
Z@Wo stay bf16 (fp8 there blows the budget).

Per-core pipeline (f32 psum accumulate everywhere):
  1. q,k f32 -> SWDGE cast-DMA -> DRAM fp8; v -> bf16. q/k DMA-transpose as
     uint16 feature-PAIRS, which lands directly in DoubleRow's [K,2,M]
     operand layout; v transposes bf16 feature-major.
  2. Projections on PE: Q'/K' via fp8 DoubleRow (weights host-scaled x64,
     paired rows [2f, 2f+1]); 1/64 descales in the psum-evacuation copies
     (k via per-j ACT Identity+bias) or folds into the DSA exp scale (qraw
     keeps 64x, one paired DVE add with a [128,2,1]-broadcast bias). V'
     token-major bf16 with bias via a K=1 rank-1 matmul.
  3. Q pooling (AvgPool k=5, zero pad) as 3 shifted DVE adds; 1/5 folded
     into the exp scale.
  4. DSA (windows of 8): per 128-token tile: bank-segregated 64-row score
     matmuls, exp on ACT, block-diag mask mul alternating DVE/GPSIMD,
     attn@V with a ones-column denominator, 1/den scale on DVE. Output
     token-major -> DRAM (axd, bf16).
  5. Re-layouts from axd: wt_view (window-summary transposes), axt
     (feature-major attn_x), pv (window-major payload, SWDGE cast to fp8).
  6. win_tok LayerNorm + exact GELU -> wtn fp8; pq/pk projections fp8
     DoubleRow with host column-permuted weights so each head sits on 32
     partitions x 2 dc-slots; PSA scores fp8 DoubleRow per head.
  7. PSA softmax normalization deferred: exp stays UNNORMALIZED in fp8
     (values ~1.0, ideal e4m3 range); pout contracts raw exp against fp8 pv
     (DoubleRow half-0 head / plain fp8 half-1 head - DoubleRow outputs must
     sit at psum column position 0); per-pair replicated den (DoubleRow
     ones matmuls) -> recip -> one DVE mul on the pout psum; Z = pn + attn_x
     via GPSIMD adds (SBUF-only bf16).
  8. Final out = Z @ Wo bf16 with bo via rank-1 matmul, ACT psum copies,
     bf16 DRAM stores (host upcasts to f32).
"""

import sys
from contextlib import ExitStack

for _p in ("/opt/trn_rl_repo/concourse", "/opt/trn_rl_repo"):
    if _p not in sys.path:
        sys.path.insert(0, _p)

import numpy as np
import ml_dtypes

import concourse.bass as bass
import concourse.mybir as mybir
import concourse.tile as tile
from concourse import bacc
from concourse.tile import add_dep_helper
from concourse.bass_utils import run_bass_kernel_spmd

BF16 = mybir.dt.bfloat16
F32 = mybir.dt.float32
FP8 = mybir.dt.float8e4
DR = mybir.MatmulPerfMode.DoubleRow
AF = mybir.ActivationFunctionType
ALU = mybir.AluOpType

B, M, D = 8, 4096, 512
H, HD = 8, 64
WIN = 7
PW = WIN + 1
QNB = 5
QLEN = 3584
WN = M // PW
SCALE = D ** -0.5
EPS = 1e-5
NCHUNK = 8
CH = 512
SCH = 1024           # super-chunk (transpose batch) size
NSC = M // SCH
CAST_RANGES = [(0, 1024), (1024, 2048), (2048, 4096)]
SC2CAST = {0: (0, 0), 1: (1, 0), 2: (2, 0), 3: (2, 1024)}  # sc -> (group, row0)
PERM = [(h % 2) * 4 + h // 2 for h in range(H)]  # head -> DSA psum slot
W8SCALE = 64.0  # host pre-scale on fp8 weights; 1/64 folded into psum copies

# PSA pq/pk column permutation: dc-group j, partition p -> original dout.
# Head h occupies 32 partitions at 32*(h%4) across the dc pair 2*(h//4),
# with features 0:32 in the even dc and 32:64 in the odd dc, so psa score
# matmuls can run fp8 DoubleRow over [32, 2] feature tiles.
PSA_PERM = [((j // 2) * 4 + p // 32) * 64 + (j % 2) * 32 + (p % 32)
            for j in range(4) for p in range(128)]


def build_program():
    nc = bacc.Bacc("TRN2", target_bir_lowering=False, debug=False, num_devices=8)

    t = {}
    t["q_in"] = nc.dram_tensor("q", [M, D], F32, kind="ExternalInput")
    t["k_in"] = nc.dram_tensor("k", [M, D], F32, kind="ExternalInput")
    t["v_in"] = nc.dram_tensor("v", [M, D], F32, kind="ExternalInput")
    for nm in ("wv", "wo"):
        t[nm] = nc.dram_tensor(nm, [D, D], BF16, kind="ExternalInput")
    for nm in ("wq8", "wk8", "wpq8", "wpk8"):
        t[nm] = nc.dram_tensor(nm, [256, 2, D], FP8, kind="ExternalInput")
    for nm in ("bq64_c", "bk_c", "bpq_c", "bpk_c", "ln_g_c", "ln_b_c"):
        t[nm] = nc.dram_tensor(nm, [128, 4], F32, kind="ExternalInput")
    t["bv_r"] = nc.dram_tensor("bv_r", [1, D], BF16, kind="ExternalInput")
    t["bo_r"] = nc.dram_tensor("bo_r", [1, D], BF16, kind="ExternalInput")
    t["bv_f"] = nc.dram_tensor("bv_f", [128, D], BF16, kind="ExternalInput")
    t["bo_f"] = nc.dram_tensor("bo_f", [128, D], BF16, kind="ExternalInput")
    t["bmask"] = nc.dram_tensor("bmask", [128, 128], BF16, kind="ExternalInput")
    t["out"] = nc.dram_tensor("out", [QLEN, D], BF16, kind="ExternalOutput")
    t["axd"] = nc.dram_tensor("axd_s", [M, D], BF16, kind="Internal")
    t["zd"] = nc.dram_tensor("zd_s", [QLEN, D], BF16, kind="Internal")
    # per-group cast targets: separate tensors so the tile framework's
    # tensor-granular dependency tracking doesn't serialize casts behind
    # earlier chunks' transpose reads (false WAR). First two groups are
    # small so compute can start early.
    # q/k cast straight to fp8 (transposed later as uint16 feature-pairs,
    # which lands in exactly the DoubleRow [K,2,M] operand layout); v stays
    # bf16 for precision.
    for nm in ("q", "k", "v"):
        dt_ = BF16 if nm == "v" else FP8
        for g, (lo, hi) in enumerate(CAST_RANGES):
            t[f"{nm}b{g}"] = nc.dram_tensor(f"{nm}b{g}_s", [hi - lo, D], dt_,
                                            kind="Internal")

    with tile.TileContext(nc) as tc:
        _build(nc, tc, t)
    nc.compile()
    return nc


def _build(nc, tc, t):
    axd, out = t["axd"], t["out"]

    with ExitStack() as octx:
        singles = octx.enter_context(tc.tile_pool(name="singles", bufs=1))

        # f32->bf16 cast DMAs first; few large batches keep the SWDGE
        # completion-semaphore lanes from being recycled between casts.
        cast_insts = {}
        srcs_d = {"q": t["q_in"], "k": t["k_in"], "v": t["v_in"]}
        for g, (lo, hi) in enumerate(CAST_RANGES):
            for nm in ("q", "v", "k"):
                ci = nc.gpsimd.dma_start(
                    out=t[f"{nm}b{g}"][:, :],
                    in_=srcs_d[nm][lo:hi, :])
                cast_insts[(nm, g)] = ci

        W = {}
        # wq loads immediately (first projection needs it); the other loads
        # are gated behind the first v cast so the q/v transposes win the
        # startup DMA race.
        gate0 = cast_insts[("v", 0)]
        for nm in ("wq8", "wk8"):
            W[nm] = singles.tile([128, 2, 2, D], FP8, tag=nm, name=f"w_{nm}")
            wi = nc.scalar.dma_start(out=W[nm][:],
                                     in_=t[nm].ap().rearrange(
                                         "(g p) s d -> p g s d", p=128))
            if nm != "wq8":
                add_dep_helper(wi.ins, gate0.ins, reason="dma order")
        W["wv"] = singles.tile([128, 4, D], BF16, tag="wv", name="w_wv")
        wi = nc.scalar.dma_start(out=W["wv"][:],
                                 in_=t["wv"].ap().rearrange("(c p) d -> p c d",
                                                            p=128))
        add_dep_helper(wi.ins, gate0.ins, reason="dma order")
        bias_cols = {}
        for nm in ("bq64_c", "bk_c"):
            bias_cols[nm] = singles.tile([128, 4], F32, tag=nm, name=f"bc_{nm}")
            nc.scalar.dma_start(out=bias_cols[nm][:], in_=t[nm][:, :])
        bv_sb = singles.tile([1, D], BF16)
        nc.scalar.dma_start(out=bv_sb[:], in_=t["bv_r"][:, :])
        mask_sb = singles.tile([128, 128], BF16)
        wi = nc.scalar.dma_start(out=mask_sb[:], in_=t["bmask"][:, :])
        add_dep_helper(wi.ins, cast_insts[("k", 0)].ins, reason="dma order")
        ones_row = singles.tile([1, 128], BF16)
        nc.vector.memset(ones_row[:], 1.0)
        ones_col = singles.tile([128, 1], BF16)
        nc.vector.memset(ones_col[:], 1.0)
        ones_full = singles.tile([128, 128], BF16)
        nc.vector.memset(ones_full[:], 1.0)
        eps_sb = singles.tile([128, 1], F32)
        nc.vector.memset(eps_sb[:], EPS)

        axd_writers = []
        axt_trans = []
        p2a = octx.enter_context(tc.tile_pool(name="p2a", bufs=1))
        axt = p2a.tile([128, 4, M], BF16, tag="axt")

        def load_phase2_weights():
            # ordering shim: keep these dep-free loads from being hoisted by
            # the scheduler ahead of the startup-critical input transposes.
            # Gated on the first attn_x pair store (~1/3 through phase 1),
            # which fires well before Act SEQ reaches these instructions, so
            # no head-of-line blocking on the Act sequencer.
            W["wo"] = singles.tile([128, 4, D], BF16, tag="wo", name="w_wo")
            nc.scalar.dma_start(out=W["wo"][:],
                                in_=t["wo"].ap().rearrange(
                                    "(c p) d -> p c d", p=128))
            for nm in ("wpq8", "wpk8"):
                W[nm] = singles.tile([128, 2, 2, D], FP8, tag=nm, name=f"w_{nm}")
                nc.scalar.dma_start(out=W[nm][:],
                                    in_=t[nm].ap().rearrange(
                                        "(g p) s d -> p g s d", p=128))
            for nm in ("bpq_c", "bpk_c", "ln_g_c", "ln_b_c"):
                bias_cols[nm] = singles.tile([128, 4], F32, tag=nm, name=f"bc_{nm}")
                nc.scalar.dma_start(out=bias_cols[nm][:], in_=t[nm][:, :])

        # ---- win_tok LN + GELU + pq/pk projections, by window quarters.
        # Quarters 0-2 run inside phase 1 as their attn_x pairs land; only
        # quarter 3 remains for the phase transition.
        lnp = octx.enter_context(tc.tile_pool(name="lnp", bufs=1))
        wtn = lnp.tile([128, 4, WN], FP8, tag="wtn")
        pqT = lnp.tile([128, 4, WN], FP8, tag="pqT")
        pkT = lnp.tile([128, 4, WN], FP8, tag="pkT")
        # dedicated feature-major copy of the window-summary tokens: cheap
        # strided-row transposes that unblock LN without the full axt pair
        wt_view = lnp.tile([128, 4, WN], BF16, tag="wtT")
        RN = WN // 2

        def ln_half_groups(r, psum_pool, ps_tag):
            st = {}

            def u_sq():
                wsq = lnp.tile([128, 4, RN], BF16, tag="wsq", name=f"wsq_{r}")
                src = wt_view[:, :, r * RN:(r + 1) * RN]
                if r == 1:
                    # transition half: DVE is idle here and this avoids an
                    # ACT Square-table reload on the critical chain
                    nc.vector.tensor_mul(wsq[:], src, src)
                else:
                    nc.scalar.activation(wsq[:], src, AF.Square)
                st["wsq"] = wsq

            def u_moments():
                ps = psum_pool.tile([128, 2, RN], F32, tag=ps_tag,
                                    name=f"ln_ps_{r}")
                for j in range(4):
                    nc.tensor.matmul(ps[:, 0, :], ones_full[:],
                                     wt_view[:, j, r * RN:(r + 1) * RN],
                                     start=(j == 0), stop=(j == 3),
                                     skip_group_check=True)
                    nc.tensor.matmul(ps[:, 1, :], ones_full[:], st["wsq"][:, j, :],
                                     start=(j == 0), stop=(j == 3),
                                     skip_group_check=True)
                mu = lnp.tile([128, RN], F32, tag="mu_sb", name=f"mu_{r}")
                nc.scalar.mul(mu[:], ps[:, 0, :], 1.0 / D)
                ex2 = lnp.tile([128, RN], F32, tag="ex2_sb", name=f"ex2_{r}")
                nc.scalar.mul(ex2[:], ps[:, 1, :], 1.0 / D)
                st["mu"], st["ex2"] = mu, ex2

            def u_stats():
                mu, ex2 = st["mu"], st["ex2"]
                var = lnp.tile([128, RN], F32, tag="var_sb", name=f"var_{r}")
                nc.vector.tensor_mul(var[:], mu[:], mu[:])
                nc.vector.tensor_sub(var[:], ex2[:], var[:])
                sd = lnp.tile([128, RN], F32, tag="sd", name=f"sd_{r}")
                nc.scalar.activation(sd[:], var[:], AF.Sqrt, bias=eps_sb[:])
                rstd = lnp.tile([128, RN], F32, tag="rstd", name=f"rstd_{r}")
                nc.vector.reciprocal(rstd[:], sd[:])
                st["rstd"] = rstd

            def u_ln(j):
                tmp = lnp.tile([128, RN], F32, tag="lntmp", bufs=2,
                               name=f"lnt_{r}_{j}")
                nc.vector.tensor_sub(tmp[:], wt_view[:, j, r * RN:(r + 1) * RN],
                                     st["mu"][:])
                nc.vector.tensor_mul(tmp[:], tmp[:], st["rstd"][:])
                nc.scalar.activation(wtn[:, j, r * RN:(r + 1) * RN], tmp[:],
                                     AF.Gelu,
                                     bias=bias_cols["ln_b_c"][:, j:j + 1],
                                     scale=bias_cols["ln_g_c"][:, j:j + 1])

            def u_pp(j):
                ps = psum_pool.tile([128, 2, RN], F32, tag=ps_tag,
                                    name=f"pp_{r}_{j}")
                for g in range(2):
                    nc.tensor.matmul(ps[:, 0, :],
                                     W["wpq8"][:, g, :, j * 128:(j + 1) * 128],
                                     wtn[:, 2 * g:2 * g + 2, r * RN:(r + 1) * RN],
                                     start=(g == 0), stop=(g == 1),
                                     perf_mode=DR, skip_group_check=True)
                    nc.tensor.matmul(ps[:, 1, :],
                                     W["wpk8"][:, g, :, j * 128:(j + 1) * 128],
                                     wtn[:, 2 * g:2 * g + 2, r * RN:(r + 1) * RN],
                                     start=(g == 0), stop=(g == 1),
                                     perf_mode=DR, skip_group_check=True)
                nc.scalar.activation(pqT[:, j, r * RN:(r + 1) * RN],
                                     ps[:, 0, :], AF.Identity,
                                     bias=bias_cols["bpq_c"][:, j:j + 1],
                                     scale=1.0 / W8SCALE)
                nc.scalar.activation(pkT[:, j, r * RN:(r + 1) * RN],
                                     ps[:, 1, :], AF.Identity,
                                     bias=bias_cols["bpk_c"][:, j:j + 1],
                                     scale=1.0 / W8SCALE)

            g = [u_sq, u_moments, u_stats]
            g += [lambda j=j: u_ln(j) for j in range(4)]
            g += [lambda j=j: u_pp(j) for j in range(4)]
            return g

        # ================= phase 1 =================
        with ExitStack() as ctx:
            p1 = ctx.enter_context(tc.tile_pool(name="p1", bufs=1))
            kT = p1.tile([128, 4, 3, CH], BF16, tag="kT")        # ring of 3 chunks
            qpT = p1.tile([128, 4, 3, CH], BF16, tag="qpT")      # ring of 3 chunks
            vtm = p1.tile([128, 12, 8, 65], BF16, tag="vtm")     # ring of 12 tiles, 65-col/head
            nc.vector.memset(vtm[:, :, :, 64:65], 1.0)           # ones col for denominators
            # projected-q ring of 3 chunk slots with 2-col halos on each side:
            # slot layout [0:2]=left halo, [2:CH+2]=chunk body, [CH+2:CH+4]=right halo
            qraw = p1.tile([128, 4, 3, CH + 4], BF16, tag="qraw")
            nc.vector.memset(qraw[:, :, 0, 0:2], 0.0)            # chunk 0 left edge

            xtp = ctx.enter_context(tc.tile_pool(name="xtp", bufs=3))
            ps_proj = ctx.enter_context(tc.tile_pool(name="ps_proj", bufs=2, space="PSUM"))
            ps_st = ctx.enter_context(tc.tile_pool(name="ps_st", bufs=1, space="PSUM"))
            ps_out = ctx.enter_context(tc.tile_pool(name="ps_out", bufs=1, space="PSUM"))
            dsa_sb = ctx.enter_context(tc.tile_pool(name="dsa_sb", bufs=3))
            pool_tmp = ctx.enter_context(tc.tile_pool(name="pool_tmp", bufs=2))
            ax_pool = ctx.enter_context(tc.tile_pool(name="ax_sb", bufs=2))

            def load_xt_super(sc):
                g, row0 = SC2CAST[sc]
                tiles = {}
                # v: bf16, 4 feature groups of 128. q/k: fp8 transposed as
                # uint16 feature-PAIRS (2 groups of 128 pairs) -> partition p
                # of group gg holds features 2*(gg*128+p), 2*(gg*128+p)+1
                # interleaved, exactly the DoubleRow [K, 2, M] layout.
                xt = xtp.tile([128, 4, SCH], BF16, tag="xt_v",
                              name=f"xt_v_{sc}")
                for dc in range(4):
                    ti = nc.sync.dma_start(
                        out=xt[:, dc, :],
                        in_=t[f"vb{g}"][row0:row0 + SCH,
                                        dc * 128:(dc + 1) * 128],
                        transpose=True)
                    add_dep_helper(ti.ins, cast_insts[("v", g)].ins,
                                   reason="transpose reads cast output")
                tiles["v"] = xt
                for nm in ("q", "k"):
                    xt = xtp.tile([128, 2, SCH], mybir.dt.uint16,
                                  tag=f"xt_{nm}", name=f"xt_{nm}_{sc}")
                    src16 = t[f"{nm}b{g}"].ap().bitcast(mybir.dt.uint16)
                    for gg in range(2):
                        ti = nc.sync.dma_start(
                            out=xt[:, gg, :],
                            in_=src16[row0:row0 + SCH,
                                      gg * 128:(gg + 1) * 128],
                            transpose=True)
                        add_dep_helper(ti.ins, cast_insts[(nm, g)].ins,
                                       reason="transpose reads cast output")
                    tiles[nm] = xt
                return tiles

            def proj_fm_pair(xt, off, wname, c, jp):
                """Projections for j-group pair (2jp, 2jp+1) into one 2-bank
                psum. q: one paired DVE add (bias [128,2,1] broadcast) writes
                qraw at 64x scale (the 1/64 is folded into the DSA exp scale,
                host pre-scales bq by 64). k: two per-j biased ACT copies
                (ACT bias APs are per-partition scalars only)."""
                ps = ps_proj.tile([128, 2, CH], F32, tag="proj",
                                  name=f"ps_{wname}_{jp}")
                for jj in range(2):
                    j = 2 * jp + jj
                    for g in range(2):
                        rhs = (xt[:, g, off:off + CH].bitcast(FP8)
                               .rearrange("p (n s) -> p s n", s=2))
                        nc.tensor.matmul(ps[:, jj, :],
                                         W[wname][:, g, :, j * 128:(j + 1) * 128],
                                         rhs, start=(g == 0), stop=(g == 1),
                                         perf_mode=DR, skip_group_check=True)
                if wname == "wq8":
                    nc.vector.tensor_add(
                        qraw[:, 2 * jp:2 * jp + 2, c % 3, 2:2 + CH], ps[:],
                        bias_cols["bq64_c"][:, 2 * jp:2 * jp + 2]
                        .unsqueeze(2).to_broadcast((128, 2, CH)))
                else:
                    for jj in range(2):
                        j = 2 * jp + jj
                        nc.scalar.activation(kT[:, j, c % 3, :], ps[:, jj, :],
                                             AF.Identity,
                                             bias=bias_cols["bk_c"][:, j:j + 1],
                                             scale=1.0 / W8SCALE)

            def proj_v_pair(xt, off, c, tp):
                ps = ps_proj.tile([128, 2, D], F32, tag="proj", name=f"ps_v_{tp}")
                for tt_ in range(2):
                    tt = 2 * tp + tt_
                    for dk in range(4):
                        nc.tensor.matmul(ps[:, tt_, :],
                                         xt[:, dk, off + tt * 128:off + (tt + 1) * 128],
                                         W["wv"][:, dk, :], start=(dk == 0), stop=False,
                                         skip_group_check=True)
                    nc.tensor.matmul(ps[:, tt_, :], ones_row[:], bv_sb[:], start=False,
                                     stop=True, skip_group_check=True)
                s = (c * 4 + 2 * tp) % 12
                nc.scalar.copy(vtm[:, s:s + 2, :, 0:64],
                               ps[:].rearrange("p t (h d) -> p t h d", h=H))

            def halo_copies(c):
                """After chunk c's q-projections land in slot c%3, export its
                edges into the neighbouring slots' halo columns."""
                if c > 0:
                    nc.scalar.copy(qraw[:, :, (c - 1) % 3, CH + 2:CH + 4],
                                   qraw[:, :, c % 3, 2:4])
                if c + 1 < NCHUNK:
                    nc.scalar.copy(qraw[:, :, (c + 1) % 3, 0:2],
                                   qraw[:, :, c % 3, CH:CH + 2])
                else:
                    nc.vector.memset(qraw[:, :, c % 3, CH + 2:CH + 4], 0.0)

            def pool_chunk(c):
                s = c % 3
                ta = pool_tmp.tile([128, 4, CH + 2], BF16, tag="ta")
                nc.vector.tensor_add(ta[:], qraw[:, :, s, 0:CH + 2],
                                     qraw[:, :, s, 1:CH + 3])
                tb = pool_tmp.tile([128, 4, CH], BF16, tag="tb")
                nc.vector.tensor_add(tb[:], ta[:, :, 0:CH], ta[:, :, 2:CH + 2])
                nc.vector.tensor_add(qpT[:, :, c % 3, :], tb[:],
                                     qraw[:, :, s, 4:CH + 4])

            def dsa_scores(c, lt):
                """MM1 (+ rank-17 additive mask) + exp for tile lt of chunk c."""
                st = ps_st.tile([128, 8, 128], F32, tag="st", name=f"st_{c}_{lt}")
                for h in range(H):
                    hp = PERM[h]
                    base = (h % 2) * 64
                    lhsT = kT[base:base + 64, h // 2, c % 3, lt * 128:(lt + 1) * 128]
                    rhs = qpT[base:base + 64, h // 2, c % 3, lt * 128:(lt + 1) * 128]
                    nc.tensor.matmul(st[:, hp, :], lhsT, rhs, start=True, stop=True,
                                     skip_group_check=True)
                expS = dsa_sb.tile([128, 8, 128], BF16, tag="expS",
                                   name=f"expS_{c}_{lt}")
                # qpT carries a 64x scale (folded out here); alternate the
                # mask mul between DVE and GPSIMD to balance engine load
                nc.scalar.activation(expS[:], st[:], AF.Exp,
                                     scale=SCALE / QNB / W8SCALE)
                eng = nc.vector if lt % 2 == 0 else nc.gpsimd
                eng.tensor_mul(expS[:], expS[:],
                               mask_sb[:].unsqueeze(1).to_broadcast((128, 8, 128)))
                return expS

            def dsa_out(c, lt, masked, ax_out):
                """attn@V with ones-col denominators, then normalize."""
                outp = ps_out.tile([128, 2, 512], F32, tag="outp",
                                   name=f"outp_{c}_{lt}")
                for h in range(H):
                    hp = PERM[h]
                    nc.tensor.matmul(outp[:, h // 4, (h % 4) * 65:(h % 4) * 65 + 65],
                                     masked[:, hp, :],
                                     vtm[:, (c * 4 + lt) % 12, h, :],
                                     start=True, stop=True, skip_group_check=True)
                recip = dsa_sb.tile([128, 2, 4], F32, tag="recip",
                                    name=f"recip_{c}_{lt}")
                den_view = bass.AP(outp.tensor, outp[:].offset + 64,
                                   [outp[:].ap[0], [512, 2], [65, 4]])
                nc.vector.reciprocal(recip[:], den_view)
                # V' already contains +bv (rank-1 matmul in proj_v); attention
                # weights sum to 1 after the 1/den scale, so bias is exact.
                av_view = bass.AP(outp.tensor, outp[:].offset,
                                  [outp[:].ap[0], [512, 2], [65, 4], [1, 64]])
                nc.vector.tensor_mul(
                    ax_out.rearrange("p (a b d) -> p a b d", a=2, b=4),
                    av_view,
                    recip[:].unsqueeze(3).to_broadcast((128, 2, 4, 64)))

            def dsa_group_list(c, ax):
                masked = {}
                g = []
                g.append(lambda: masked.__setitem__(0, dsa_scores(c, 0)))
                g.append(lambda: masked.__setitem__(1, dsa_scores(c, 1)))
                g.append(lambda: dsa_out(c, 0, masked.pop(0), ax[:, 0, :]))
                g.append(lambda: masked.__setitem__(2, dsa_scores(c, 2)))
                g.append(lambda: dsa_out(c, 1, masked.pop(1), ax[:, 1, :]))
                g.append(lambda: masked.__setitem__(3, dsa_scores(c, 3)))
                g.append(lambda: dsa_out(c, 2, masked.pop(2), ax[:, 2, :]))
                g.append(lambda: dsa_out(c, 3, masked.pop(3), ax[:, 3, :]))
                return g

            def store_ax_pair(cp, ax2):
                """Store DSA output for chunks (cp, cp+1), then transpose the
                pair back feature-major (overlaps phase 1). The last pair is
                stored per chunk so the final win_tok transposes - which gate
                the LN chain at the phase transition - wait on a half-size
                store."""
                wsrc = axd.ap().rearrange("(w s) d -> w s d", s=PW)
                dst = axd.ap().rearrange("(g lt p) d -> g p lt d", lt=8, p=128)
                wi = nc.gpsimd.dma_start(out=dst[cp // 2], in_=ax2[:])
                wis = [wi]
                for dc in range(4):
                    wt = nc.sync.dma_start(
                        out=wt_view[:, dc, cp * 64:(cp + 2) * 64],
                        in_=wsrc[cp * 64:(cp + 2) * 64, 0,
                                 dc * 128:(dc + 1) * 128],
                        transpose=True)
                    add_dep_helper(wt.ins, wi.ins,
                                   reason="win_tok transpose reads axd pair")
                axd_writers.append(wis)
                tis = []
                for dc in range(4):
                    ti = nc.sync.dma_start(
                        out=axt[:, dc, cp * CH:(cp + 2) * CH],
                        in_=axd[cp * CH:(cp + 2) * CH, dc * 128:(dc + 1) * 128],
                        transpose=True)
                    for wi in wis:
                        add_dep_helper(ti.ins, wi.ins,
                                       reason="axt transpose reads axd pair")
                    tis.append(ti)
                axt_trans.append(tis)

            xt_tiles = {0: load_xt_super(0)}
            extra = []      # deferred phase-2 prologue work units
            ax2 = None

            for c in range(NCHUNK + 2):
                if c == 5:
                    load_phase2_weights()
                if c == 7:
                    extra.extend(ln_half_groups(0, ps_proj, "proj"))
                pgroups = []
                if c < NCHUNK:
                    sc = c // 2
                    off = (c % 2) * CH
                    if c % 2 == 0 and sc + 1 < NSC:
                        xt_tiles[sc + 1] = load_xt_super(sc + 1)
                    qxt = xt_tiles[sc]["q"]
                    kxt = xt_tiles[sc]["k"]
                    vxt = xt_tiles[sc]["v"]
                    for jp in range(2):
                        pgroups.append(lambda jp=jp, x=qxt, o=off, c=c:
                                       proj_fm_pair(x, o, "wq8", c, jp))
                    for tp in range(2):
                        pgroups.append(lambda tp=tp, x=vxt, o=off, c=c:
                                       proj_v_pair(x, o, c, tp))
                    for jp in range(2):
                        pgroups.append(lambda jp=jp, x=kxt, o=off, c=c:
                                       proj_fm_pair(x, o, "wk8", c, jp))
                dgroups = []
                dc_ = c - 2
                if dc_ >= 0:
                    if dc_ % 2 == 0:
                        ax2 = ax_pool.tile([128, 8, D], BF16, tag="ax",
                                           name=f"ax_{dc_}")
                    axv = ax2[:, (dc_ % 2) * 4:(dc_ % 2) * 4 + 4, :]
                    dgroups = dsa_group_list(dc_, axv)
                # weave: spread D groups evenly through the P stream;
                # pool(c-1) after the 2 Q-projection pairs; extra units fill
                # remaining slots late in phase 1
                npg, ndg = len(pgroups), len(dgroups)
                if npg:
                    # per-pgroup D-group quota (6 pgroups hosting up to 8)
                    quota = (1, 2, 1, 1, 2, 1)
                    di = 0
                    for i in range(npg):
                        pgroups[i]()
                        if i == 1:
                            halo_copies(c)
                            if c >= 1:
                                pool_chunk(c - 1)
                        for _ in range(quota[i]):
                            if di < ndg:
                                dgroups[di]()
                                di += 1
                                if extra:
                                    extra.pop(0)()
                    while di < ndg:
                        dgroups[di]()
                        di += 1
                else:
                    if 1 <= c <= NCHUNK:
                        pool_chunk(c - 1)
                    for g in dgroups:
                        g()
                        if extra:
                            extra.pop(0)()
                if dc_ >= 0 and dc_ % 2 == 1:
                    store_ax_pair(dc_ - 1, ax2)
            while extra:
                extra.pop(0)()

        # ================= phase 2 =================
        with ExitStack() as ctx:
            p2 = ctx.enter_context(tc.tile_pool(name="p2", bufs=1))
            ps2 = ctx.enter_context(tc.tile_pool(name="ps2", bufs=2, space="PSUM"))
            ps2b = ctx.enter_context(tc.tile_pool(name="ps2b", bufs=2, space="PSUM"))
            sb2 = ctx.enter_context(tc.tile_pool(name="sb2", bufs=2))

            bo_sb = singles.tile([1, D], BF16)
            nc.scalar.dma_start(out=bo_sb[:], in_=t["bo_r"][:, :])

            # pv gathered window-major with a SWDGE cast to fp8 for the
            # DoubleRow pout matmuls.
            pv = p2.tile([128, 4, WIN, D], FP8, tag="pv")
            srcv = axd.ap().rearrange("(cc p w) d -> cc p w d", p=128, w=PW)
            for cc in range(4):
                gi = nc.gpsimd.dma_start(out=pv[:, cc, :, :], in_=srcv[cc, :, 1:PW, :])
                for wi in axd_writers[cc]:
                    add_dep_helper(gi.ins, wi.ins, reason="pv gather")
                # ordering shim: run the gathers after the last attn_x pair's
                # feature-major transposes so the transition chain
                # (store -> axt -> LN -> pq/pk -> PSA) isn't queued behind
                # them; pout doesn't need pv until well after.
                for ti in axt_trans[-1]:
                    add_dep_helper(gi.ins, ti.ins, reason="dma order")

            # ---- LN + GELU + pq/pk for the second window half ----
            for u in ln_half_groups(1, ps2, "ps2"):
                u()

            # ---- PSA softmax per head; pout per head-pair right after ----
            # Scores run fp8 DoubleRow over [32, 2] feature tiles; exp output
            # stays UNNORMALIZED in fp8 (values ~1.0, ideal e4m3 range). pout
            # contracts raw exp against fp8 pv, and the 1/den normalization is
            # applied afterwards on the psum via a per-pair recip tile whose
            # partition rows are already head-matched (h0 on 0:64, h1 on
            # 64:128) thanks to the DoubleRow den matmul's 64-row output.
            zt = p2.tile([128, 4, QLEN], BF16, tag="zt")
            ones8_2 = p2.tile([128, 2, 128], FP8, tag="ones8")
            nc.vector.memset(ones8_2[:], 1.0)

            def psa_scores(h):
                b32 = 32 * (h % 4)
                a = 2 * (h // 4)
                es = sb2.tile([128, 4, WN], FP8, tag="psa_exp", bufs=8,
                              name=f"es_{h}")
                for cp in range(2):
                    ps = ps2b.tile([128, 2, WN], F32, tag="pair",
                                   name=f"st_{h}_{cp}")
                    for ch in range(2):
                        cc = cp * 2 + ch
                        nc.tensor.matmul(
                            ps[:, ch, :],
                            pkT[b32:b32 + 32, a:a + 2, cc * 128:(cc + 1) * 128],
                            pqT[b32:b32 + 32, a:a + 2, :], start=True, stop=True,
                            perf_mode=DR, skip_group_check=True,
                            tile_position=(b32, 0))
                    nc.scalar.activation(es[:, 2 * cp:2 * cp + 2, :], ps[:],
                                         AF.Exp, scale=SCALE)
                return es

            def psa_norm(j, es0, es1):
                """den + recip for head pair j (heads 2j, 2j+1). DoubleRow
                outputs must sit at column position 0, so each head gets a
                full 128-partition replicated den psum; the recips then read
                partition-aligned halves into one pair tile (h0 rows on 0:64,
                h1 on 64:128) for the single pout normalization mul."""
                recipd = sb2.tile([128, WN], F32, tag="psa_recip", bufs=2,
                                  name=f"r_{j}")
                for half, es in ((0, es0), (1, es1)):
                    ps_den = ps2b.tile([128, WN], F32, tag="psa_den", bufs=2,
                                       name=f"d_{j}_{half}")
                    for cp in range(2):
                        nc.tensor.matmul(
                            ps_den[:], ones8_2[:], es[:, 2 * cp:2 * cp + 2, :],
                            start=(cp == 0), stop=(cp == 1),
                            perf_mode=DR, skip_group_check=True)
                    nc.vector.reciprocal(recipd[half * 64:(half + 1) * 64, :],
                                         ps_den[half * 64:(half + 1) * 64, :])
                return recipd

            def pout_pair(j, wh, es0, es1, recipd):
                """pout for head-pair j over query-window half wh."""
                w0 = wh * (WN // 2)
                for i in range(WIN):
                    po = ps2.tile([128, WN // 2], F32, tag="ps2",
                                  name=f"po_{j}_{i}_{wh}")
                    # DoubleRow requires output column position 0, so only
                    # the half-0 head runs DR; half-1 (psum base 64) uses
                    # plain fp8 matmuls.
                    h0 = 2 * j
                    for cp in range(2):
                        nc.tensor.matmul(
                            po[0:64, :],
                            pv[:, 2 * cp:2 * cp + 2, i, h0 * 64:(h0 + 1) * 64],
                            es0[:, 2 * cp:2 * cp + 2, w0:w0 + WN // 2],
                            start=(cp == 0), stop=(cp == 1),
                            perf_mode=DR, skip_group_check=True)
                    h1 = 2 * j + 1
                    for cc in range(4):
                        nc.tensor.matmul(
                            po[64:128, :],
                            pv[:, cc, i, h1 * 64:(h1 + 1) * 64],
                            es1[:, cc, w0:w0 + WN // 2],
                            start=(cc == 0), stop=(cc == 3),
                            skip_group_check=True)
                    pn = sb2.tile([128, WN // 2], BF16, tag="pn", bufs=4,
                                  name=f"pn_{j}_{i}_{wh}")
                    nc.vector.tensor_mul(pn[:], po[:], recipd[:, w0:w0 + WN // 2])
                    # SBUF-only bf16 add: run it on GPSIMD to keep DVE free
                    # for the psum-reading normalization muls
                    nc.gpsimd.tensor_add(
                        zt[:, j, :].rearrange("p (w i) -> p w i", i=WIN)
                        [:, w0:w0 + WN // 2, i],
                        pn[:],
                        axt[:, j, :].rearrange("p (w s) -> p w s", s=PW)
                        [:, w0:w0 + WN // 2, 1 + i])

            outv = out.ap().rearrange("(g tt p) d -> g p tt d", tt=2, p=128)

            def final_group(g, split_store=False):
                o_sb = sb2.tile([128, 2, D], BF16, tag="osb", bufs=4,
                                name=f"osb_{g}")
                for q in range(2):
                    tt = g * 2 + q
                    ps = ps2.tile([128, D], F32, tag="ps2", name=f"fin_{tt}")
                    for dk in range(4):
                        nc.tensor.matmul(ps[:], zt[:, dk, tt * 128:(tt + 1) * 128],
                                         W["wo"][:, dk, :], start=(dk == 0),
                                         stop=False, skip_group_check=True)
                    # bo via rank-1 matmul; psum evacuation on ACT (idle in
                    # the fin tail) instead of a DVE add
                    nc.tensor.matmul(ps[:], ones_row[:], bo_sb[:], start=False,
                                     stop=True, skip_group_check=True)
                    nc.scalar.copy(o_sb[:, q, :], ps[:])
                    if split_store:
                        nc.sync.dma_start(out=outv[g][:, q, :],
                                          in_=o_sb[:, q, :])
                if not split_store:
                    nc.sync.dma_start(out=outv[g], in_=o_sb[:])

            # pipeline: scores(h+1) | norm(j) once its pair of heads is
            # scored | pout(j) right after; the last pair is split by
            # query-window half so the first finals overlap its second half
            es_store = {0: psa_scores(0)}
            recs = {}
            done_pairs = 0
            for h in range(1, H):
                es_store[h] = psa_scores(h)
                if h % 2 == 1:
                    j = h // 2
                    recs[j] = psa_norm(j, es_store[2 * j], es_store[2 * j + 1])
                if h % 2 == 0 and done_pairs in recs:
                    j = done_pairs
                    pout_pair(j, 0, es_store[2 * j], es_store[2 * j + 1], recs[j])
                    pout_pair(j, 1, es_store[2 * j], es_store[2 * j + 1], recs[j])
                    es_store.pop(2 * j), es_store.pop(2 * j + 1), recs.pop(j)
                    done_pairs += 1
            while done_pairs < 4:
                j = done_pairs
                pout_pair(j, 0, es_store[2 * j], es_store[2 * j + 1], recs[j])
                if j == 3:
                    for g in range(7):
                        final_group(g)
                pout_pair(j, 1, es_store[2 * j], es_store[2 * j + 1], recs[j])
                done_pairs += 1
            for g in range(7, QLEN // 256):
                final_group(g, split_store=(g == QLEN // 256 - 1))


_NC_CACHE = None


def _get_program():
    global _NC_CACHE
    if _NC_CACHE is None:
        _NC_CACHE = build_program()
    return _NC_CACHE


def _fp8_paired(Wm, perm=None, pair="dc"):
    """Host prep for fp8 DoubleRow lhsT: optional column permutation, x64
    scale, then row pairing. pair="dc": rows (2g+s)*128+p -> [g*128+p, s]
    (matches the wtn dc-group layout); pair="consec": rows 2f+s -> [f, s]
    (matches the uint16-pair input transposes)."""
    w = np.asarray(Wm, np.float32)
    if perm is not None:
        w = w[:, perm]
    w = (w * W8SCALE).astype(ml_dtypes.float8_e4m3fn)
    if pair == "consec":
        return np.ascontiguousarray(w.reshape(256, 2, D))
    # rows: r = g*256 + s*128 + p  ->  out[g*128+p, s, :]
    return np.ascontiguousarray(
        w.reshape(2, 2, 128, D).transpose(0, 2, 1, 3).reshape(256, 2, D))


def _host_consts(Wk, bk, Wv, bv, Wq, bq, ln_g, ln_b, Wpq, bpq, Wpk, bpk, Wo, bo):
    bf = ml_dtypes.bfloat16
    col = lambda b: np.asarray(b, np.float32).reshape(4, 128).T.copy()
    perm = np.asarray(PSA_PERM)
    consts = {
        "wq8": _fp8_paired(Wq, pair="consec"),
        "wk8": _fp8_paired(Wk, pair="consec"),
        "wv": np.asarray(Wv, np.float32).astype(bf),
        "wpq8": _fp8_paired(Wpq, perm),
        "wpk8": _fp8_paired(Wpk, perm),
        "wo": np.asarray(Wo, np.float32).astype(bf),
        "bq64_c": col(np.asarray(bq, np.float32) * W8SCALE), "bk_c": col(bk),
        "bpq_c": col(np.asarray(bpq, np.float32)[perm]),
        "bpk_c": col(np.asarray(bpk, np.float32)[perm]),
        "ln_g_c": col(ln_g), "ln_b_c": col(ln_b),
        "bv_r": np.asarray(bv, np.float32).reshape(1, D).astype(bf),
        "bo_r": np.asarray(bo, np.float32).reshape(1, D).astype(bf),
        "bv_f": np.tile(np.asarray(bv, np.float32).reshape(1, D), (128, 1)).astype(bf),
        "bo_f": np.tile(np.asarray(bo, np.float32).reshape(1, D), (128, 1)).astype(bf),
    }
    m = np.zeros((128, 128), np.float32)
    for g in range(16):
        m[g * PW:(g + 1) * PW, g * PW:(g + 1) * PW] = 1.0
    consts["bmask"] = m.astype(bf)
    return consts


def kernel(k, v, q, query_len, Wk, bk, Wv, bv, Wq, bq, ln_g, ln_b,
           Wpq, bpq, Wpk, bpk, Wo, bo):
    nc = _get_program()
    consts = _host_consts(Wk, bk, Wv, bv, Wq, bq, ln_g, ln_b,
                          Wpq, bpq, Wpk, bpk, Wo, bo)
    k = np.asarray(k, np.float32)
    v = np.asarray(v, np.float32)
    q = np.asarray(q, np.float32)
    in_maps = []
    for b in range(B):
        m = {"q": np.ascontiguousarray(q[b]), "k": np.ascontiguousarray(k[b]),
             "v": np.ascontiguousarray(v[b])}
        m.update(consts)
        in_maps.append(m)
    res = run_bass_kernel_spmd(nc, in_maps, core_ids=list(range(B)))
    return np.stack([np.asarray(res.results[b]["out"], np.float32)
                     for b in range(B)], axis=0)


if __name__ == "__main__":
    nc = build_program()
    print("program built ok")



# revision 82
# speedup vs baseline: 1.0052x; 1.0052x over previous
"""Trainium2 Bass kernel for DeformableMultiHeadedAttention.

Data-parallel over batch B=8 across 8 NeuronCores (one batch element per
core, identical programs, no collectives). Heavy matmuls run fp8-e4m3
DoubleRow (0.5 PE cycles/row) wherever a numpy precision study showed the
final rel-err stays ~0.006 (tolerance 2e-2); V projection and the final
Z@Wo stay bf16 (fp8 there blows the budget).

Per-core pipeline (f32 psum accumulate everywhere):
  1. q,k f32 -> SWDGE cast-DMA -> DRAM fp8; v -> bf16. q/k DMA-transpose as
     uint16 feature-PAIRS, which lands directly in DoubleRow's [K,2,M]
     operand layout; v transposes bf16 feature-major.
  2. Projections on PE: Q'/K' via fp8 DoubleRow (weights host-scaled x64,
     paired rows [2f, 2f+1]); 1/64 descales in the psum-evacuation copies
     (k via per-j ACT Identity+bias) or folds into the DSA exp scale (qraw
     keeps 64x, one paired DVE add with a [128,2,1]-broadcast bias). V'
     token-major bf16 with bias via a K=1 rank-1 matmul.
  3. Q pooling (AvgPool k=5, zero pad) as 3 shifted DVE adds; 1/5 folded
     into the exp scale.
  4. DSA (windows of 8): per 128-token tile: bank-segregated 64-row score
     matmuls, exp on ACT, block-diag mask mul alternating DVE/GPSIMD,
     attn@V with a ones-column denominator, 1/den scale on DVE. Output
     token-major -> DRAM (axd, bf16).
  5. Re-layouts from axd: wt_view (window-summary transposes), axt
     (feature-major attn_x), pv (window-major payload, SWDGE cast to fp8).
  6. win_tok LayerNorm + exact GELU -> wtn fp8; pq/pk projections fp8
     DoubleRow with host column-permuted weights so each head sits on 32
     partitions x 2 dc-slots; PSA scores fp8 DoubleRow per head.
  7. PSA softmax normalization deferred: exp stays UNNORMALIZED in fp8
     (values ~1.0, ideal e4m3 range); pout contracts raw exp against fp8 pv
     (DoubleRow half-0 head / plain fp8 half-1 head - DoubleRow outputs must
     sit at psum column position 0); per-pair replicated den (DoubleRow
     ones matmuls) -> recip -> one DVE mul on the pout psum; Z = pn + attn_x
     via GPSIMD adds (SBUF-only bf16).
  8. Final out = Z @ Wo bf16 with bo via rank-1 matmul, ACT psum copies,
     bf16 DRAM stores (host upcasts to f32).
"""

import sys
from contextlib import ExitStack

for _p in ("/opt/trn_rl_repo/concourse", "/opt/trn_rl_repo"):
    if _p not in sys.path:
        sys.path.insert(0, _p)

import numpy as np
import ml_dtypes

import concourse.bass as bass
import concourse.mybir as mybir
import concourse.tile as tile
from concourse import bacc
from concourse.tile import add_dep_helper
from concourse.bass_utils import run_bass_kernel_spmd

BF16 = mybir.dt.bfloat16
F32 = mybir.dt.float32
FP8 = mybir.dt.float8e4
DR = mybir.MatmulPerfMode.DoubleRow
AF = mybir.ActivationFunctionType
ALU = mybir.AluOpType

B, M, D = 8, 4096, 512
H, HD = 8, 64
WIN = 7
PW = WIN + 1
QNB = 5
QLEN = 3584
WN = M // PW
SCALE = D ** -0.5
EPS = 1e-5
NCHUNK = 8
CH = 512
SCH = 1024           # super-chunk (transpose batch) size
NSC = M // SCH
CAST_RANGES = [(0, 1024), (1024, 2048), (2048, 4096)]
SC2CAST = {0: (0, 0), 1: (1, 0), 2: (2, 0), 3: (2, 1024)}  # sc -> (group, row0)
PERM = [(h % 2) * 4 + h // 2 for h in range(H)]  # head -> DSA psum slot
W8SCALE = 64.0  # host pre-scale on fp8 weights; 1/64 folded into psum copies

# PSA pq/pk column permutation: dc-group j, partition p -> original dout.
# Head h occupies 32 partitions at 32*(h%4) across the dc pair 2*(h//4),
# with features 0:32 in the even dc and 32:64 in the odd dc, so psa score
# matmuls can run fp8 DoubleRow over [32, 2] feature tiles.
PSA_PERM = [((j // 2) * 4 + p // 32) * 64 + (j % 2) * 32 + (p % 32)
            for j in range(4) for p in range(128)]


def build_program():
    nc = bacc.Bacc("TRN2", target_bir_lowering=False, debug=False, num_devices=8)

    t = {}
    t["q_in"] = nc.dram_tensor("q", [M, D], F32, kind="ExternalInput")
    t["k_in"] = nc.dram_tensor("k", [M, D], F32, kind="ExternalInput")
    t["v_in"] = nc.dram_tensor("v", [M, D], F32, kind="ExternalInput")
    for nm in ("wv", "wo"):
        t[nm] = nc.dram_tensor(nm, [D, D], BF16, kind="ExternalInput")
    for nm in ("wq8", "wk8", "wpq8", "wpk8"):
        t[nm] = nc.dram_tensor(nm, [256, 2, D], FP8, kind="ExternalInput")
    for nm in ("bq64_c", "bk_c", "bpq_c", "bpk_c", "ln_g_c", "ln_b_c"):
        t[nm] = nc.dram_tensor(nm, [128, 4], F32, kind="ExternalInput")
    t["bv_r"] = nc.dram_tensor("bv_r", [1, D], BF16, kind="ExternalInput")
    t["bo_r"] = nc.dram_tensor("bo_r", [1, D], BF16, kind="ExternalInput")
    t["bv_f"] = nc.dram_tensor("bv_f", [128, D], BF16, kind="ExternalInput")
    t["bo_f"] = nc.dram_tensor("bo_f", [128, D], BF16, kind="ExternalInput")
    t["bmask"] = nc.dram_tensor("bmask", [128, 128], BF16, kind="ExternalInput")
    t["out"] = nc.dram_tensor("out", [QLEN, D], BF16, kind="ExternalOutput")
    t["axd"] = nc.dram_tensor("axd_s", [M, D], BF16, kind="Internal")
    t["zd"] = nc.dram_tensor("zd_s", [QLEN, D], BF16, kind="Internal")
    # per-group cast targets: separate tensors so the tile framework's
    # tensor-granular dependency tracking doesn't serialize casts behind
    # earlier chunks' transpose reads (false WAR). First two groups are
    # small so compute can start early.
    # q/k cast straight to fp8 (transposed later as uint16 feature-pairs,
    # which lands in exactly the DoubleRow [K,2,M] operand layout); v stays
    # bf16 for precision.
    for nm in ("q", "k", "v"):
        dt_ = BF16 if nm == "v" else FP8
        for g, (lo, hi) in enumerate(CAST_RANGES):
            t[f"{nm}b{g}"] = nc.dram_tensor(f"{nm}b{g}_s", [hi - lo, D], dt_,
                                            kind="Internal")

    with tile.TileContext(nc) as tc:
        _build(nc, tc, t)
    nc.compile()
    return nc


def _build(nc, tc, t):
    axd, out = t["axd"], t["out"]

    with ExitStack() as octx:
        singles = octx.enter_context(tc.tile_pool(name="singles", bufs=1))

        # f32->bf16 cast DMAs first; few large batches keep the SWDGE
        # completion-semaphore lanes from being recycled between casts.
        cast_insts = {}
        srcs_d = {"q": t["q_in"], "k": t["k_in"], "v": t["v_in"]}
        for g, (lo, hi) in enumerate(CAST_RANGES):
            for nm in ("q", "v", "k"):
                ci = nc.gpsimd.dma_start(
                    out=t[f"{nm}b{g}"][:, :],
                    in_=srcs_d[nm][lo:hi, :])
                cast_insts[(nm, g)] = ci

        W = {}
        # wq loads immediately (first projection needs it); the other loads
        # are gated behind the first v cast so the q/v transposes win the
        # startup DMA race.
        gate0 = cast_insts[("v", 0)]
        for nm in ("wq8", "wk8"):
            W[nm] = singles.tile([128, 2, 2, D], FP8, tag=nm, name=f"w_{nm}")
            wi = nc.scalar.dma_start(out=W[nm][:],
                                     in_=t[nm].ap().rearrange(
                                         "(g p) s d -> p g s d", p=128))
            if nm != "wq8":
                add_dep_helper(wi.ins, gate0.ins, reason="dma order")
        W["wv"] = singles.tile([128, 4, D], BF16, tag="wv", name="w_wv")
        wi = nc.scalar.dma_start(out=W["wv"][:],
                                 in_=t["wv"].ap().rearrange("(c p) d -> p c d",
                                                            p=128))
        add_dep_helper(wi.ins, gate0.ins, reason="dma order")
        bias_cols = {}
        for nm in ("bq64_c", "bk_c"):
            bias_cols[nm] = singles.tile([128, 4], F32, tag=nm, name=f"bc_{nm}")
            nc.scalar.dma_start(out=bias_cols[nm][:], in_=t[nm][:, :])
        bv_sb = singles.tile([1, D], BF16)
        nc.scalar.dma_start(out=bv_sb[:], in_=t["bv_r"][:, :])
        mask_sb = singles.tile([128, 128], BF16)
        wi = nc.scalar.dma_start(out=mask_sb[:], in_=t["bmask"][:, :])
        add_dep_helper(wi.ins, cast_insts[("k", 0)].ins, reason="dma order")
        ones_row = singles.tile([1, 128], BF16)
        nc.vector.memset(ones_row[:], 1.0)
        ones_col = singles.tile([128, 1], BF16)
        nc.vector.memset(ones_col[:], 1.0)
        ones_full = singles.tile([128, 128], BF16)
        nc.vector.memset(ones_full[:], 1.0)
        eps_sb = singles.tile([128, 1], F32)
        nc.vector.memset(eps_sb[:], EPS)

        axd_writers = []
        axt_trans = []
        p2a = octx.enter_context(tc.tile_pool(name="p2a", bufs=1))
        axt = p2a.tile([128, 4, M], BF16, tag="axt")

        def load_phase2_weights():
            # ordering shim: keep these dep-free loads from being hoisted by
            # the scheduler ahead of the startup-critical input transposes.
            # Gated on the first attn_x pair store (~1/3 through phase 1),
            # which fires well before Act SEQ reaches these instructions, so
            # no head-of-line blocking on the Act sequencer.
            W["wo"] = singles.tile([128, 4, D], BF16, tag="wo", name="w_wo")
            nc.scalar.dma_start(out=W["wo"][:],
                                in_=t["wo"].ap().rearrange(
                                    "(c p) d -> p c d", p=128))
            for nm in ("wpq8", "wpk8"):
                W[nm] = singles.tile([128, 2, 2, D], FP8, tag=nm, name=f"w_{nm}")
                nc.scalar.dma_start(out=W[nm][:],
                                    in_=t[nm].ap().rearrange(
                                        "(g p) s d -> p g s d", p=128))
            for nm in ("bpq_c", "bpk_c", "ln_g_c", "ln_b_c"):
                bias_cols[nm] = singles.tile([128, 4], F32, tag=nm, name=f"bc_{nm}")
                nc.scalar.dma_start(out=bias_cols[nm][:], in_=t[nm][:, :])

        # ---- win_tok LN + GELU + pq/pk projections, by window quarters.
        # Quarters 0-2 run inside phase 1 as their attn_x pairs land; only
        # quarter 3 remains for the phase transition.
        lnp = octx.enter_context(tc.tile_pool(name="lnp", bufs=1))
        wtn = lnp.tile([128, 4, WN], FP8, tag="wtn")
        pqT = lnp.tile([128, 4, WN], FP8, tag="pqT")
        pkT = lnp.tile([128, 4, WN], FP8, tag="pkT")
        # dedicated feature-major copy of the window-summary tokens: cheap
        # strided-row transposes that unblock LN without the full axt pair
        wt_view = lnp.tile([128, 4, WN], BF16, tag="wtT")
        RN = WN // 2

        def ln_half_groups(r, psum_pool, ps_tag):
            st = {}

            def u_sq():
                wsq = lnp.tile([128, 4, RN], BF16, tag="wsq", name=f"wsq_{r}")
                src = wt_view[:, :, r * RN:(r + 1) * RN]
                if r == 1:
                    # transition half: DVE is idle here and this avoids an
                    # ACT Square-table reload on the critical chain
                    nc.vector.tensor_mul(wsq[:], src, src)
                else:
                    nc.scalar.activation(wsq[:], src, AF.Square)
                st["wsq"] = wsq

            def u_moments():
                ps = psum_pool.tile([128, 2, RN], F32, tag=ps_tag,
                                    name=f"ln_ps_{r}")
                for j in range(4):
                    nc.tensor.matmul(ps[:, 0, :], ones_full[:],
                                     wt_view[:, j, r * RN:(r + 1) * RN],
                                     start=(j == 0), stop=(j == 3),
                                     skip_group_check=True)
                    nc.tensor.matmul(ps[:, 1, :], ones_full[:], st["wsq"][:, j, :],
                                     start=(j == 0), stop=(j == 3),
                                     skip_group_check=True)
                mu = lnp.tile([128, RN], F32, tag="mu_sb", name=f"mu_{r}")
                nc.scalar.mul(mu[:], ps[:, 0, :], 1.0 / D)
                ex2 = lnp.tile([128, RN], F32, tag="ex2_sb", name=f"ex2_{r}")
                nc.scalar.mul(ex2[:], ps[:, 1, :], 1.0 / D)
                st["mu"], st["ex2"] = mu, ex2

            def u_stats():
                mu, ex2 = st["mu"], st["ex2"]
                var = lnp.tile([128, RN], F32, tag="var_sb", name=f"var_{r}")
                nc.vector.tensor_mul(var[:], mu[:], mu[:])
                nc.vector.tensor_sub(var[:], ex2[:], var[:])
                sd = lnp.tile([128, RN], F32, tag="sd", name=f"sd_{r}")
                nc.scalar.activation(sd[:], var[:], AF.Sqrt, bias=eps_sb[:])
                rstd = lnp.tile([128, RN], F32, tag="rstd", name=f"rstd_{r}")
                nc.vector.reciprocal(rstd[:], sd[:])
                st["rstd"] = rstd

            def u_ln(j):
                tmp = lnp.tile([128, RN], F32, tag="lntmp", bufs=2,
                               name=f"lnt_{r}_{j}")
                nc.vector.tensor_sub(tmp[:], wt_view[:, j, r * RN:(r + 1) * RN],
                                     st["mu"][:])
                nc.vector.tensor_mul(tmp[:], tmp[:], st["rstd"][:])
                nc.scalar.activation(wtn[:, j, r * RN:(r + 1) * RN], tmp[:],
                                     AF.Gelu,
                                     bias=bias_cols["ln_b_c"][:, j:j + 1],
                                     scale=bias_cols["ln_g_c"][:, j:j + 1])

            def u_pp(j):
                ps = psum_pool.tile([128, 2, RN], F32, tag=ps_tag,
                                    name=f"pp_{r}_{j}")
                for g in range(2):
                    nc.tensor.matmul(ps[:, 0, :],
                                     W["wpq8"][:, g, :, j * 128:(j + 1) * 128],
                                     wtn[:, 2 * g:2 * g + 2, r * RN:(r + 1) * RN],
                                     start=(g == 0), stop=(g == 1),
                                     perf_mode=DR, skip_group_check=True)
                    nc.tensor.matmul(ps[:, 1, :],
                                     W["wpk8"][:, g, :, j * 128:(j + 1) * 128],
                                     wtn[:, 2 * g:2 * g + 2, r * RN:(r + 1) * RN],
                                     start=(g == 0), stop=(g == 1),
                                     perf_mode=DR, skip_group_check=True)
                nc.scalar.activation(pqT[:, j, r * RN:(r + 1) * RN],
                                     ps[:, 0, :], AF.Identity,
                                     bias=bias_cols["bpq_c"][:, j:j + 1],
                                     scale=1.0 / W8SCALE)
                nc.scalar.activation(pkT[:, j, r * RN:(r + 1) * RN],
                                     ps[:, 1, :], AF.Identity,
                                     bias=bias_cols["bpk_c"][:, j:j + 1],
                                     scale=1.0 / W8SCALE)

            g = [u_sq, u_moments, u_stats]
            g += [lambda j=j: u_ln(j) for j in range(4)]
            g += [lambda j=j: u_pp(j) for j in range(4)]
            return g

        # ================= phase 1 =================
        with ExitStack() as ctx:
            p1 = ctx.enter_context(tc.tile_pool(name="p1", bufs=1))
            kT = p1.tile([128, 4, 3, CH], BF16, tag="kT")        # ring of 3 chunks
            qpT = p1.tile([128, 4, 3, CH], BF16, tag="qpT")      # ring of 3 chunks
            vtm = p1.tile([128, 12, 8, 65], BF16, tag="vtm")     # ring of 12 tiles, 65-col/head
            nc.vector.memset(vtm[:, :, :, 64:65], 1.0)           # ones col for denominators
            # projected-q ring of 3 chunk slots with 2-col halos on each side:
            # slot layout [0:2]=left halo, [2:CH+2]=chunk body, [CH+2:CH+4]=right halo
            qraw = p1.tile([128, 4, 3, CH + 4], BF16, tag="qraw")
            nc.vector.memset(qraw[:, :, 0, 0:2], 0.0)            # chunk 0 left edge

            xtp = ctx.enter_context(tc.tile_pool(name="xtp", bufs=3))
            ps_proj = ctx.enter_context(tc.tile_pool(name="ps_proj", bufs=2, space="PSUM"))
            ps_st = ctx.enter_context(tc.tile_pool(name="ps_st", bufs=1, space="PSUM"))
            ps_out = ctx.enter_context(tc.tile_pool(name="ps_out", bufs=1, space="PSUM"))
            dsa_sb = ctx.enter_context(tc.tile_pool(name="dsa_sb", bufs=3))
            pool_tmp = ctx.enter_context(tc.tile_pool(name="pool_tmp", bufs=2))
            ax_pool = ctx.enter_context(tc.tile_pool(name="ax_sb", bufs=2))

            def load_xt_super(sc):
                g, row0 = SC2CAST[sc]
                tiles = {}
                # v: bf16, 4 feature groups of 128. q/k: fp8 transposed as
                # uint16 feature-PAIRS (2 groups of 128 pairs) -> partition p
                # of group gg holds features 2*(gg*128+p), 2*(gg*128+p)+1
                # interleaved, exactly the DoubleRow [K, 2, M] layout.
                xt = xtp.tile([128, 4, SCH], BF16, tag="xt_v",
                              name=f"xt_v_{sc}")
                for dc in range(4):
                    ti = nc.sync.dma_start(
                        out=xt[:, dc, :],
                        in_=t[f"vb{g}"][row0:row0 + SCH,
                                        dc * 128:(dc + 1) * 128],
                        transpose=True)
                    add_dep_helper(ti.ins, cast_insts[("v", g)].ins,
                                   reason="transpose reads cast output")
                tiles["v"] = xt
                for nm in ("q", "k"):
                    xt = xtp.tile([128, 2, SCH], mybir.dt.uint16,
                                  tag=f"xt_{nm}", name=f"xt_{nm}_{sc}")
                    src16 = t[f"{nm}b{g}"].ap().bitcast(mybir.dt.uint16)
                    for gg in range(2):
                        ti = nc.sync.dma_start(
                            out=xt[:, gg, :],
                            in_=src16[row0:row0 + SCH,
                                      gg * 128:(gg + 1) * 128],
                            transpose=True)
                        add_dep_helper(ti.ins, cast_insts[(nm, g)].ins,
                                       reason="transpose reads cast output")
                    tiles[nm] = xt
                return tiles

            def proj_fm_pair(xt, off, wname, c, jp):
                """Projections for j-group pair (2jp, 2jp+1) into one 2-bank
                psum. q: one paired DVE add (bias [128,2,1] broadcast) writes
                qraw at 64x scale (the 1/64 is folded into the DSA exp scale,
                host pre-scales bq by 64). k: two per-j biased ACT copies
                (ACT bias APs are per-partition scalars only)."""
                ps = ps_proj.tile([128, 2, CH], F32, tag="proj",
                                  name=f"ps_{wname}_{jp}")
                for jj in range(2):
                    j = 2 * jp + jj
                    for g in range(2):
                        rhs = (xt[:, g, off:off + CH].bitcast(FP8)
                               .rearrange("p (n s) -> p s n", s=2))
                        nc.tensor.matmul(ps[:, jj, :],
                                         W[wname][:, g, :, j * 128:(j + 1) * 128],
                                         rhs, start=(g == 0), stop=(g == 1),
                                         perf_mode=DR, skip_group_check=True)
                if wname == "wq8":
                    nc.vector.tensor_add(
                        qraw[:, 2 * jp:2 * jp + 2, c % 3, 2:2 + CH], ps[:],
                        bias_cols["bq64_c"][:, 2 * jp:2 * jp + 2]
                        .unsqueeze(2).to_broadcast((128, 2, CH)))
                else:
                    for jj in range(2):
                        j = 2 * jp + jj
                        nc.scalar.activation(kT[:, j, c % 3, :], ps[:, jj, :],
                                             AF.Identity,
                                             bias=bias_cols["bk_c"][:, j:j + 1],
                                             scale=1.0 / W8SCALE)

            def proj_v_pair(xt, off, c, tp):
                ps = ps_proj.tile([128, 2, D], F32, tag="proj", name=f"ps_v_{tp}")
                for tt_ in range(2):
                    tt = 2 * tp + tt_
                    for dk in range(4):
                        nc.tensor.matmul(ps[:, tt_, :],
                                         xt[:, dk, off + tt * 128:off + (tt + 1) * 128],
                                         W["wv"][:, dk, :], start=(dk == 0), stop=False,
                                         skip_group_check=True)
                    nc.tensor.matmul(ps[:, tt_, :], ones_row[:], bv_sb[:], start=False,
                                     stop=True, skip_group_check=True)
                s = (c * 4 + 2 * tp) % 12
                nc.scalar.copy(vtm[:, s:s + 2, :, 0:64],
                               ps[:].rearrange("p t (h d) -> p t h d", h=H))

            def halo_copies(c):
                """After chunk c's q-projections land in slot c%3, export its
                edges into the neighbouring slots' halo columns."""
                if c > 0:
                    nc.scalar.copy(qraw[:, :, (c - 1) % 3, CH + 2:CH + 4],
                                   qraw[:, :, c % 3, 2:4])
                if c + 1 < NCHUNK:
                    nc.scalar.copy(qraw[:, :, (c + 1) % 3, 0:2],
                                   qraw[:, :, c % 3, CH:CH + 2])
                else:
                    nc.vector.memset(qraw[:, :, c % 3, CH + 2:CH + 4], 0.0)

            def pool_chunk(c):
                s = c % 3
                ta = pool_tmp.tile([128, 4, CH + 2], BF16, tag="ta")
                nc.vector.tensor_add(ta[:], qraw[:, :, s, 0:CH + 2],
                                     qraw[:, :, s, 1:CH + 3])
                tb = pool_tmp.tile([128, 4, CH], BF16, tag="tb")
                nc.vector.tensor_add(tb[:], ta[:, :, 0:CH], ta[:, :, 2:CH + 2])
                nc.vector.tensor_add(qpT[:, :, c % 3, :], tb[:],
                                     qraw[:, :, s, 4:CH + 4])

            def dsa_scores(c, lt):
                """MM1 (+ rank-17 additive mask) + exp for tile lt of chunk c."""
                st = ps_st.tile([128, 8, 128], F32, tag="st", name=f"st_{c}_{lt}")
                for h in range(H):
                    hp = PERM[h]
                    base = (h % 2) * 64
                    lhsT = kT[base:base + 64, h // 2, c % 3, lt * 128:(lt + 1) * 128]
                    rhs = qpT[base:base + 64, h // 2, c % 3, lt * 128:(lt + 1) * 128]
                    nc.tensor.matmul(st[:, hp, :], lhsT, rhs, start=True, stop=True,
                                     skip_group_check=True)
                expS = dsa_sb.tile([128, 8, 128], BF16, tag="expS",
                                   name=f"expS_{c}_{lt}")
                # qpT carries a 64x scale (folded out here); alternate the
                # mask mul between DVE and GPSIMD to balance engine load
                nc.scalar.activation(expS[:], st[:], AF.Exp,
                                     scale=SCALE / QNB / W8SCALE)
                eng = nc.vector if lt % 2 == 0 else nc.gpsimd
                eng.tensor_mul(expS[:], expS[:],
                               mask_sb[:].unsqueeze(1).to_broadcast((128, 8, 128)))
                return expS

            def dsa_out(c, lt, masked, ax_out):
                """attn@V with ones-col denominators, then normalize."""
                outp = ps_out.tile([128, 2, 512], F32, tag="outp",
                                   name=f"outp_{c}_{lt}")
                for h in range(H):
                    hp = PERM[h]
                    nc.tensor.matmul(outp[:, h // 4, (h % 4) * 65:(h % 4) * 65 + 65],
                                     masked[:, hp, :],
                                     vtm[:, (c * 4 + lt) % 12, h, :],
                                     start=True, stop=True, skip_group_check=True)
                recip = dsa_sb.tile([128, 2, 4], F32, tag="recip",
                                    name=f"recip_{c}_{lt}")
                den_view = bass.AP(outp.tensor, outp[:].offset + 64,
                                   [outp[:].ap[0], [512, 2], [65, 4]])
                nc.vector.reciprocal(recip[:], den_view)
                # V' already contains +bv (rank-1 matmul in proj_v); attention
                # weights sum to 1 after the 1/den scale, so bias is exact.
                av_view = bass.AP(outp.tensor, outp[:].offset,
                                  [outp[:].ap[0], [512, 2], [65, 4], [1, 64]])
                nc.vector.tensor_mul(
                    ax_out.rearrange("p (a b d) -> p a b d", a=2, b=4),
                    av_view,
                    recip[:].unsqueeze(3).to_broadcast((128, 2, 4, 64)))

            def dsa_group_list(c, ax):
                masked = {}
                g = []
                g.append(lambda: masked.__setitem__(0, dsa_scores(c, 0)))
                g.append(lambda: masked.__setitem__(1, dsa_scores(c, 1)))
                g.append(lambda: dsa_out(c, 0, masked.pop(0), ax[:, 0, :]))
                g.append(lambda: masked.__setitem__(2, dsa_scores(c, 2)))
                g.append(lambda: dsa_out(c, 1, masked.pop(1), ax[:, 1, :]))
                g.append(lambda: masked.__setitem__(3, dsa_scores(c, 3)))
                g.append(lambda: dsa_out(c, 2, masked.pop(2), ax[:, 2, :]))
                g.append(lambda: dsa_out(c, 3, masked.pop(3), ax[:, 3, :]))
                return g

            def store_ax_pair(cp, ax2):
                """Store DSA output for chunks (cp, cp+1), then transpose the
                pair back feature-major (overlaps phase 1). The last pair is
                stored per chunk so the final win_tok transposes - which gate
                the LN chain at the phase transition - wait on a half-size
                store."""
                wsrc = axd.ap().rearrange("(w s) d -> w s d", s=PW)
                dst = axd.ap().rearrange("(g lt p) d -> g p lt d", lt=8, p=128)
                wi = nc.gpsimd.dma_start(out=dst[cp // 2], in_=ax2[:])
                wis = [wi]
                for dc in range(4):
                    wt = nc.sync.dma_start(
                        out=wt_view[:, dc, cp * 64:(cp + 2) * 64],
                        in_=wsrc[cp * 64:(cp + 2) * 64, 0,
                                 dc * 128:(dc + 1) * 128],
                        transpose=True)
                    add_dep_helper(wt.ins, wi.ins,
                                   reason="win_tok transpose reads axd pair")
                axd_writers.append(wis)
                tis = []
                for dc in range(4):
                    ti = nc.sync.dma_start(
                        out=axt[:, dc, cp * CH:(cp + 2) * CH],
                        in_=axd[cp * CH:(cp + 2) * CH, dc * 128:(dc + 1) * 128],
                        transpose=True)
                    for wi in wis:
                        add_dep_helper(ti.ins, wi.ins,
                                       reason="axt transpose reads axd pair")
                    tis.append(ti)
                axt_trans.append(tis)

            xt_tiles = {0: load_xt_super(0)}
            extra = []      # deferred phase-2 prologue work units
            ax2 = None

            for c in range(NCHUNK + 2):
                if c == 5:
                    load_phase2_weights()
                if c == 7:
                    extra.extend(ln_half_groups(0, ps_proj, "proj"))
                pgroups = []
                if c < NCHUNK:
                    sc = c // 2
                    off = (c % 2) * CH
                    if c % 2 == 0 and sc + 1 < NSC:
                        xt_tiles[sc + 1] = load_xt_super(sc + 1)
                    qxt = xt_tiles[sc]["q"]
                    kxt = xt_tiles[sc]["k"]
                    vxt = xt_tiles[sc]["v"]
                    for jp in range(2):
                        pgroups.append(lambda jp=jp, x=qxt, o=off, c=c:
                                       proj_fm_pair(x, o, "wq8", c, jp))
                    for tp in range(2):
                        pgroups.append(lambda tp=tp, x=vxt, o=off, c=c:
                                       proj_v_pair(x, o, c, tp))
                    for jp in range(2):
                        pgroups.append(lambda jp=jp, x=kxt, o=off, c=c:
                                       proj_fm_pair(x, o, "wk8", c, jp))
                dgroups = []
                dc_ = c - 2
                if dc_ >= 0:
                    if dc_ % 2 == 0:
                        ax2 = ax_pool.tile([128, 8, D], BF16, tag="ax",
                                           name=f"ax_{dc_}")
                    axv = ax2[:, (dc_ % 2) * 4:(dc_ % 2) * 4 + 4, :]
                    dgroups = dsa_group_list(dc_, axv)
                # weave: spread D groups evenly through the P stream;
                # pool(c-1) after the 2 Q-projection pairs; extra units fill
                # remaining slots late in phase 1
                npg, ndg = len(pgroups), len(dgroups)
                if npg:
                    # per-pgroup D-group quota (6 pgroups hosting up to 8)
                    quota = (1, 2, 1, 2, 1, 1)
                    di = 0
                    for i in range(npg):
                        pgroups[i]()
                        if i == 1:
                            halo_copies(c)
                            if c >= 1:
                                pool_chunk(c - 1)
                        for _ in range(quota[i]):
                            if di < ndg:
                                dgroups[di]()
                                di += 1
                                if extra:
                                    extra.pop(0)()
                    while di < ndg:
                        dgroups[di]()
                        di += 1
                else:
                    if 1 <= c <= NCHUNK:
                        pool_chunk(c - 1)
                    for g in dgroups:
                        g()
                        if extra:
                            extra.pop(0)()
                if dc_ >= 0 and dc_ % 2 == 1:
                    store_ax_pair(dc_ - 1, ax2)
            while extra:
                extra.pop(0)()

        # ================= phase 2 =================
        with ExitStack() as ctx:
            p2 = ctx.enter_context(tc.tile_pool(name="p2", bufs=1))
            ps2 = ctx.enter_context(tc.tile_pool(name="ps2", bufs=2, space="PSUM"))
            ps2b = ctx.enter_context(tc.tile_pool(name="ps2b", bufs=2, space="PSUM"))
            sb2 = ctx.enter_context(tc.tile_pool(name="sb2", bufs=2))

            bo_sb = singles.tile([1, D], BF16)
            nc.scalar.dma_start(out=bo_sb[:], in_=t["bo_r"][:, :])

            # pv gathered window-major with a SWDGE cast to fp8 for the
            # DoubleRow pout matmuls.
            pv = p2.tile([128, 4, WIN, D], FP8, tag="pv")
            srcv = axd.ap().rearrange("(cc p w) d -> cc p w d", p=128, w=PW)
            for cc in range(4):
                gi = nc.gpsimd.dma_start(out=pv[:, cc, :, :], in_=srcv[cc, :, 1:PW, :])
                for wi in axd_writers[cc]:
                    add_dep_helper(gi.ins, wi.ins, reason="pv gather")
                # ordering shim: run the gathers after the last attn_x pair's
                # feature-major transposes so the transition chain
                # (store -> axt -> LN -> pq/pk -> PSA) isn't queued behind
                # them; pout doesn't need pv until well after.
                for ti in axt_trans[-1]:
                    add_dep_helper(gi.ins, ti.ins, reason="dma order")

            # ---- LN + GELU + pq/pk for the second window half ----
            for u in ln_half_groups(1, ps2, "ps2"):
                u()

            # ---- PSA softmax per head; pout per head-pair right after ----
            # Scores run fp8 DoubleRow over [32, 2] feature tiles; exp output
            # stays UNNORMALIZED in fp8 (values ~1.0, ideal e4m3 range). pout
            # contracts raw exp against fp8 pv, and the 1/den normalization is
            # applied afterwards on the psum via a per-pair recip tile whose
            # partition rows are already head-matched (h0 on 0:64, h1 on
            # 64:128) thanks to the DoubleRow den matmul's 64-row output.
            zt = p2.tile([128, 4, QLEN], BF16, tag="zt")
            ones8_2 = p2.tile([128, 2, 128], FP8, tag="ones8")
            nc.vector.memset(ones8_2[:], 1.0)

            def psa_scores(h):
                b32 = 32 * (h % 4)
                a = 2 * (h // 4)
                es = sb2.tile([128, 4, WN], FP8, tag="psa_exp", bufs=8,
                              name=f"es_{h}")
                for cp in range(2):
                    ps = ps2b.tile([128, 2, WN], F32, tag="pair",
                                   name=f"st_{h}_{cp}")
                    for ch in range(2):
                        cc = cp * 2 + ch
                        nc.tensor.matmul(
                            ps[:, ch, :],
                            pkT[b32:b32 + 32, a:a + 2, cc * 128:(cc + 1) * 128],
                            pqT[b32:b32 + 32, a:a + 2, :], start=True, stop=True,
                            perf_mode=DR, skip_group_check=True,
                            tile_position=(b32, 0))
                    nc.scalar.activation(es[:, 2 * cp:2 * cp + 2, :], ps[:],
                                         AF.Exp, scale=SCALE)
                return es

            def psa_norm(j, es0, es1):
                """den + recip for head pair j (heads 2j, 2j+1). DoubleRow
                outputs must sit at column position 0, so each head gets a
                full 128-partition replicated den psum; the recips then read
                partition-aligned halves into one pair tile (h0 rows on 0:64,
                h1 on 64:128) for the single pout normalization mul."""
                recipd = sb2.tile([128, WN], F32, tag="psa_recip", bufs=2,
                                  name=f"r_{j}")
                for half, es in ((0, es0), (1, es1)):
                    ps_den = ps2b.tile([128, WN], F32, tag="psa_den", bufs=2,
                                       name=f"d_{j}_{half}")
                    for cp in range(2):
                        nc.tensor.matmul(
                            ps_den[:], ones8_2[:], es[:, 2 * cp:2 * cp + 2, :],
                            start=(cp == 0), stop=(cp == 1),
                            perf_mode=DR, skip_group_check=True)
                    nc.vector.reciprocal(recipd[half * 64:(half + 1) * 64, :],
                                         ps_den[half * 64:(half + 1) * 64, :])
                return recipd

            def pout_pair(j, wh, es0, es1, recipd):
                """pout for head-pair j over query-window half wh."""
                w0 = wh * (WN // 2)
                for i in range(WIN):
                    po = ps2.tile([128, WN // 2], F32, tag="ps2",
                                  name=f"po_{j}_{i}_{wh}")
                    # DoubleRow requires output column position 0, so only
                    # the half-0 head runs DR; half-1 (psum base 64) uses
                    # plain fp8 matmuls.
                    h0 = 2 * j
                    for cp in range(2):
                        nc.tensor.matmul(
                            po[0:64, :],
                            pv[:, 2 * cp:2 * cp + 2, i, h0 * 64:(h0 + 1) * 64],
                            es0[:, 2 * cp:2 * cp + 2, w0:w0 + WN // 2],
                            start=(cp == 0), stop=(cp == 1),
                            perf_mode=DR, skip_group_check=True)
                    h1 = 2 * j + 1
                    for cc in range(4):
                        nc.tensor.matmul(
                            po[64:128, :],
                            pv[:, cc, i, h1 * 64:(h1 + 1) * 64],
                            es1[:, cc, w0:w0 + WN // 2],
                            start=(cc == 0), stop=(cc == 3),
                            skip_group_check=True)
                    pn = sb2.tile([128, WN // 2], BF16, tag="pn", bufs=4,
                                  name=f"pn_{j}_{i}_{wh}")
                    nc.vector.tensor_mul(pn[:], po[:], recipd[:, w0:w0 + WN // 2])
                    # SBUF-only bf16 add: run it on GPSIMD to keep DVE free
                    # for the psum-reading normalization muls
                    nc.gpsimd.tensor_add(
                        zt[:, j, :].rearrange("p (w i) -> p w i", i=WIN)
                        [:, w0:w0 + WN // 2, i],
                        pn[:],
                        axt[:, j, :].rearrange("p (w s) -> p w s", s=PW)
                        [:, w0:w0 + WN // 2, 1 + i])

            outv = out.ap().rearrange("(g tt p) d -> g p tt d", tt=2, p=128)

            def final_group(g, split_store=False):
                o_sb = sb2.tile([128, 2, D], BF16, tag="osb", bufs=4,
                                name=f"osb_{g}")
                for q in range(2):
                    tt = g * 2 + q
                    ps = ps2.tile([128, D], F32, tag="ps2", name=f"fin_{tt}")
                    for dk in range(4):
                        nc.tensor.matmul(ps[:], zt[:, dk, tt * 128:(tt + 1) * 128],
                                         W["wo"][:, dk, :], start=(dk == 0),
                                         stop=False, skip_group_check=True)
                    # bo via rank-1 matmul; psum evacuation on ACT (idle in
                    # the fin tail) instead of a DVE add
                    nc.tensor.matmul(ps[:], ones_row[:], bo_sb[:], start=False,
                                     stop=True, skip_group_check=True)
                    nc.scalar.copy(o_sb[:, q, :], ps[:])
                    if split_store:
                        nc.sync.dma_start(out=outv[g][:, q, :],
                                          in_=o_sb[:, q, :])
                if not split_store:
                    nc.sync.dma_start(out=outv[g], in_=o_sb[:])

            # pipeline: scores(h+1) | norm(j) once its pair of heads is
            # scored | pout(j) right after; the last pair is split by
            # query-window half so the first finals overlap its second half
            es_store = {0: psa_scores(0)}
            recs = {}
            done_pairs = 0
            for h in range(1, H):
                es_store[h] = psa_scores(h)
                if h % 2 == 1:
                    j = h // 2
                    recs[j] = psa_norm(j, es_store[2 * j], es_store[2 * j + 1])
                if h % 2 == 0 and done_pairs in recs:
                    j = done_pairs
                    pout_pair(j, 0, es_store[2 * j], es_store[2 * j + 1], recs[j])
                    pout_pair(j, 1, es_store[2 * j], es_store[2 * j + 1], recs[j])
                    es_store.pop(2 * j), es_store.pop(2 * j + 1), recs.pop(j)
                    done_pairs += 1
            while done_pairs < 4:
                j = done_pairs
                pout_pair(j, 0, es_store[2 * j], es_store[2 * j + 1], recs[j])
                if j == 3:
                    for g in range(7):
                        final_group(g)
                pout_pair(j, 1, es_store[2 * j], es_store[2 * j + 1], recs[j])
                done_pairs += 1
            for g in range(7, QLEN // 256):
                final_group(g, split_store=(g >= QLEN // 256 - 2))


_NC_CACHE = None


def _get_program():
    global _NC_CACHE
    if _NC_CACHE is None:
        _NC_CACHE = build_program()
    return _NC_CACHE


def _fp8_paired(Wm, perm=None, pair="dc"):
    """Host prep for fp8 DoubleRow lhsT: optional column permutation, x64
    scale, then row pairing. pair="dc": rows (2g+s)*128+p -> [g*128+p, s]
    (matches the wtn dc-group layout); pair="consec": rows 2f+s -> [f, s]
    (matches the uint16-pair input transposes)."""
    w = np.asarray(Wm, np.float32)
    if perm is not None:
        w = w[:, perm]
    w = (w * W8SCALE).astype(ml_dtypes.float8_e4m3fn)
    if pair == "consec":
        return np.ascontiguousarray(w.reshape(256, 2, D))
    # rows: r = g*256 + s*128 + p  ->  out[g*128+p, s, :]
    return np.ascontiguousarray(
        w.reshape(2, 2, 128, D).transpose(0, 2, 1, 3).reshape(256, 2, D))


def _host_consts(Wk, bk, Wv, bv, Wq, bq, ln_g, ln_b, Wpq, bpq, Wpk, bpk, Wo, bo):
    bf = ml_dtypes.bfloat16
    col = lambda b: np.asarray(b, np.float32).reshape(4, 128).T.copy()
    perm = np.asarray(PSA_PERM)
    consts = {
        "wq8": _fp8_paired(Wq, pair="consec"),
        "wk8": _fp8_paired(Wk, pair="consec"),
        "wv": np.asarray(Wv, np.float32).astype(bf),
        "wpq8": _fp8_paired(Wpq, perm),
        "wpk8": _fp8_paired(Wpk, perm),
        "wo": np.asarray(Wo, np.float32).astype(bf),
        "bq64_c": col(np.asarray(bq, np.float32) * W8SCALE), "bk_c": col(bk),
        "bpq_c": col(np.asarray(bpq, np.float32)[perm]),
        "bpk_c": col(np.asarray(bpk, np.float32)[perm]),
        "ln_g_c": col(ln_g), "ln_b_c": col(ln_b),
        "bv_r": np.asarray(bv, np.float32).reshape(1, D).astype(bf),
        "bo_r": np.asarray(bo, np.float32).reshape(1, D).astype(bf),
        "bv_f": np.tile(np.asarray(bv, np.float32).reshape(1, D), (128, 1)).astype(bf),
        "bo_f": np.tile(np.asarray(bo, np.float32).reshape(1, D), (128, 1)).astype(bf),
    }
    m = np.zeros((128, 128), np.float32)
    for g in range(16):
        m[g * PW:(g + 1) * PW, g * PW:(g + 1) * PW] = 1.0
    consts["bmask"] = m.astype(bf)
    return consts


def kernel(k, v, q, query_len, Wk, bk, Wv, bv, Wq, bq, ln_g, ln_b,
           Wpq, bpq, Wpk, bpk, Wo, bo):
    nc = _get_program()
    consts = _host_consts(Wk, bk, Wv, bv, Wq, bq, ln_g, ln_b,
                          Wpq, bpq, Wpk, bpk, Wo, bo)
    k = np.asarray(k, np.float32)
    v = np.asarray(v, np.float32)
    q = np.asarray(q, np.float32)
    in_maps = []
    for b in range(B):
        m = {"q": np.ascontiguousarray(q[b]), "k": np.ascontiguousarray(k[b]),
             "v": np.ascontiguousarray(v[b])}
        m.update(consts)
        in_maps.append(m)
    res = run_bass_kernel_spmd(nc, in_maps, core_ids=list(range(B)))
    return np.stack([np.asarray(res.results[b]["out"], np.float32)
                     for b in range(B)], axis=0)


if __name__ == "__main__":
    nc = build_program()
    print("program built ok")



# revision 83
# speedup vs baseline: 1.0063x; 1.0011x over previous
"""Trainium2 Bass kernel for DeformableMultiHeadedAttention.

Data-parallel over batch B=8 across 8 NeuronCores (one batch element per
core, identical programs, no collectives). Heavy matmuls run fp8-e4m3
DoubleRow (0.5 PE cycles/row) wherever a numpy precision study showed the
final rel-err stays ~0.006 (tolerance 2e-2); V projection and the final
Z@Wo stay bf16 (fp8 there blows the budget).

Per-core pipeline (f32 psum accumulate everywhere):
  1. q,k f32 -> SWDGE cast-DMA -> DRAM fp8; v -> bf16. q/k DMA-transpose as
     uint16 feature-PAIRS, which lands directly in DoubleRow's [K,2,M]
     operand layout; v transposes bf16 feature-major.
  2. Projections on PE: Q'/K' via fp8 DoubleRow (weights host-scaled x64,
     paired rows [2f, 2f+1]); 1/64 descales in the psum-evacuation copies
     (k via per-j ACT Identity+bias) or folds into the DSA exp scale (qraw
     keeps 64x, one paired DVE add with a [128,2,1]-broadcast bias). V'
     token-major bf16 with bias via a K=1 rank-1 matmul.
  3. Q pooling (AvgPool k=5, zero pad) as 3 shifted DVE adds; 1/5 folded
     into the exp scale.
  4. DSA (windows of 8): per 128-token tile: bank-segregated 64-row score
     matmuls, exp on ACT, block-diag mask mul alternating DVE/GPSIMD,
     attn@V with a ones-column denominator, 1/den scale on DVE. Output
     token-major -> DRAM (axd, bf16).
  5. Re-layouts from axd: wt_view (window-summary transposes), axt
     (feature-major attn_x), pv (window-major payload, SWDGE cast to fp8).
  6. win_tok LayerNorm + exact GELU -> wtn fp8; pq/pk projections fp8
     DoubleRow with host column-permuted weights so each head sits on 32
     partitions x 2 dc-slots; PSA scores fp8 DoubleRow per head.
  7. PSA softmax normalization deferred: exp stays UNNORMALIZED in fp8
     (values ~1.0, ideal e4m3 range); pout contracts raw exp against fp8 pv
     (DoubleRow half-0 head / plain fp8 half-1 head - DoubleRow outputs must
     sit at psum column position 0); per-pair replicated den (DoubleRow
     ones matmuls) -> recip -> one DVE mul on the pout psum; Z = pn + attn_x
     via GPSIMD adds (SBUF-only bf16).
  8. Final out = Z @ Wo bf16 with bo via rank-1 matmul, ACT psum copies,
     bf16 DRAM stores (host upcasts to f32).
"""

import sys
from contextlib import ExitStack

for _p in ("/opt/trn_rl_repo/concourse", "/opt/trn_rl_repo"):
    if _p not in sys.path:
        sys.path.insert(0, _p)

import numpy as np
import ml_dtypes

import concourse.bass as bass
import concourse.mybir as mybir
import concourse.tile as tile
from concourse import bacc
from concourse.tile import add_dep_helper
from concourse.bass_utils import run_bass_kernel_spmd

BF16 = mybir.dt.bfloat16
F32 = mybir.dt.float32
FP8 = mybir.dt.float8e4
DR = mybir.MatmulPerfMode.DoubleRow
AF = mybir.ActivationFunctionType
ALU = mybir.AluOpType

B, M, D = 8, 4096, 512
H, HD = 8, 64
WIN = 7
PW = WIN + 1
QNB = 5
QLEN = 3584
WN = M // PW
SCALE = D ** -0.5
EPS = 1e-5
NCHUNK = 8
CH = 512
SCH = 1024           # super-chunk (transpose batch) size
NSC = M // SCH
CAST_RANGES = [(0, 1024), (1024, 2048), (2048, 4096)]
SC2CAST = {0: (0, 0), 1: (1, 0), 2: (2, 0), 3: (2, 1024)}  # sc -> (group, row0)
PERM = [(h % 2) * 4 + h // 2 for h in range(H)]  # head -> DSA psum slot
W8SCALE = 64.0  # host pre-scale on fp8 weights; 1/64 folded into psum copies

# PSA pq/pk column permutation: dc-group j, partition p -> original dout.
# Head h occupies 32 partitions at 32*(h%4) across the dc pair 2*(h//4),
# with features 0:32 in the even dc and 32:64 in the odd dc, so psa score
# matmuls can run fp8 DoubleRow over [32, 2] feature tiles.
PSA_PERM = [((j // 2) * 4 + p // 32) * 64 + (j % 2) * 32 + (p % 32)
            for j in range(4) for p in range(128)]


def build_program():
    nc = bacc.Bacc("TRN2", target_bir_lowering=False, debug=False, num_devices=8)

    t = {}
    t["q_in"] = nc.dram_tensor("q", [M, D], F32, kind="ExternalInput")
    t["k_in"] = nc.dram_tensor("k", [M, D], F32, kind="ExternalInput")
    t["v_in"] = nc.dram_tensor("v", [M, D], F32, kind="ExternalInput")
    for nm in ("wv", "wo"):
        t[nm] = nc.dram_tensor(nm, [D, D], BF16, kind="ExternalInput")
    for nm in ("wq8", "wk8", "wpq8", "wpk8"):
        t[nm] = nc.dram_tensor(nm, [256, 2, D], FP8, kind="ExternalInput")
    for nm in ("bq64_c", "bk_c", "bpq_c", "bpk_c", "ln_g_c", "ln_b_c"):
        t[nm] = nc.dram_tensor(nm, [128, 4], F32, kind="ExternalInput")
    t["bv_r"] = nc.dram_tensor("bv_r", [1, D], BF16, kind="ExternalInput")
    t["bo_r"] = nc.dram_tensor("bo_r", [1, D], BF16, kind="ExternalInput")
    t["bv_f"] = nc.dram_tensor("bv_f", [128, D], BF16, kind="ExternalInput")
    t["bo_f"] = nc.dram_tensor("bo_f", [128, D], BF16, kind="ExternalInput")
    t["bmask"] = nc.dram_tensor("bmask", [128, 128], BF16, kind="ExternalInput")
    t["out"] = nc.dram_tensor("out", [QLEN, D], BF16, kind="ExternalOutput")
    t["axd"] = nc.dram_tensor("axd_s", [M, D], BF16, kind="Internal")
    t["zd"] = nc.dram_tensor("zd_s", [QLEN, D], BF16, kind="Internal")
    # per-group cast targets: separate tensors so the tile framework's
    # tensor-granular dependency tracking doesn't serialize casts behind
    # earlier chunks' transpose reads (false WAR). First two groups are
    # small so compute can start early.
    # q/k cast straight to fp8 (transposed later as uint16 feature-pairs,
    # which lands in exactly the DoubleRow [K,2,M] operand layout); v stays
    # bf16 for precision.
    for nm in ("q", "k", "v"):
        dt_ = BF16 if nm == "v" else FP8
        for g, (lo, hi) in enumerate(CAST_RANGES):
            t[f"{nm}b{g}"] = nc.dram_tensor(f"{nm}b{g}_s", [hi - lo, D], dt_,
                                            kind="Internal")

    with tile.TileContext(nc) as tc:
        _build(nc, tc, t)
    nc.compile()
    return nc


def _build(nc, tc, t):
    axd, out = t["axd"], t["out"]

    with ExitStack() as octx:
        singles = octx.enter_context(tc.tile_pool(name="singles", bufs=1))

        # f32->bf16 cast DMAs first; few large batches keep the SWDGE
        # completion-semaphore lanes from being recycled between casts.
        cast_insts = {}
        srcs_d = {"q": t["q_in"], "k": t["k_in"], "v": t["v_in"]}
        for g, (lo, hi) in enumerate(CAST_RANGES):
            for nm in ("q", "v", "k"):
                ci = nc.gpsimd.dma_start(
                    out=t[f"{nm}b{g}"][:, :],
                    in_=srcs_d[nm][lo:hi, :])
                cast_insts[(nm, g)] = ci

        W = {}
        # wq loads immediately (first projection needs it); the other loads
        # are gated behind the first v cast so the q/v transposes win the
        # startup DMA race.
        gate0 = cast_insts[("v", 0)]
        for nm in ("wq8", "wk8"):
            W[nm] = singles.tile([128, 2, 2, D], FP8, tag=nm, name=f"w_{nm}")
            wi = nc.scalar.dma_start(out=W[nm][:],
                                     in_=t[nm].ap().rearrange(
                                         "(g p) s d -> p g s d", p=128))
            if nm != "wq8":
                add_dep_helper(wi.ins, gate0.ins, reason="dma order")
        W["wv"] = singles.tile([128, 4, D], BF16, tag="wv", name="w_wv")
        wi = nc.scalar.dma_start(out=W["wv"][:],
                                 in_=t["wv"].ap().rearrange("(c p) d -> p c d",
                                                            p=128))
        add_dep_helper(wi.ins, gate0.ins, reason="dma order")
        bias_cols = {}
        for nm in ("bq64_c", "bk_c"):
            bias_cols[nm] = singles.tile([128, 4], F32, tag=nm, name=f"bc_{nm}")
            nc.scalar.dma_start(out=bias_cols[nm][:], in_=t[nm][:, :])
        bv_sb = singles.tile([1, D], BF16)
        nc.scalar.dma_start(out=bv_sb[:], in_=t["bv_r"][:, :])
        mask_sb = singles.tile([128, 128], BF16)
        wi = nc.scalar.dma_start(out=mask_sb[:], in_=t["bmask"][:, :])
        add_dep_helper(wi.ins, cast_insts[("k", 0)].ins, reason="dma order")
        ones_row = singles.tile([1, 128], BF16)
        nc.vector.memset(ones_row[:], 1.0)
        ones_col = singles.tile([128, 1], BF16)
        nc.vector.memset(ones_col[:], 1.0)
        ones_full = singles.tile([128, 128], BF16)
        nc.vector.memset(ones_full[:], 1.0)
        eps_sb = singles.tile([128, 1], F32)
        nc.vector.memset(eps_sb[:], EPS)

        axd_writers = []
        axt_trans = []
        p2a = octx.enter_context(tc.tile_pool(name="p2a", bufs=1))
        axt = p2a.tile([128, 4, M], BF16, tag="axt")

        def load_phase2_weights():
            # ordering shim: keep these dep-free loads from being hoisted by
            # the scheduler ahead of the startup-critical input transposes.
            # Gated on the first attn_x pair store (~1/3 through phase 1),
            # which fires well before Act SEQ reaches these instructions, so
            # no head-of-line blocking on the Act sequencer.
            W["wo"] = singles.tile([128, 4, D], BF16, tag="wo", name="w_wo")
            nc.scalar.dma_start(out=W["wo"][:],
                                in_=t["wo"].ap().rearrange(
                                    "(c p) d -> p c d", p=128))
            for nm in ("wpq8", "wpk8"):
                W[nm] = singles.tile([128, 2, 2, D], FP8, tag=nm, name=f"w_{nm}")
                nc.scalar.dma_start(out=W[nm][:],
                                    in_=t[nm].ap().rearrange(
                                        "(g p) s d -> p g s d", p=128))
            for nm in ("bpq_c", "bpk_c", "ln_g_c", "ln_b_c"):
                bias_cols[nm] = singles.tile([128, 4], F32, tag=nm, name=f"bc_{nm}")
                nc.scalar.dma_start(out=bias_cols[nm][:], in_=t[nm][:, :])

        # ---- win_tok LN + GELU + pq/pk projections, by window quarters.
        # Quarters 0-2 run inside phase 1 as their attn_x pairs land; only
        # quarter 3 remains for the phase transition.
        lnp = octx.enter_context(tc.tile_pool(name="lnp", bufs=1))
        wtn = lnp.tile([128, 4, WN], FP8, tag="wtn")
        pqT = lnp.tile([128, 4, WN], FP8, tag="pqT")
        pkT = lnp.tile([128, 4, WN], FP8, tag="pkT")
        # dedicated feature-major copy of the window-summary tokens: cheap
        # strided-row transposes that unblock LN without the full axt pair
        wt_view = lnp.tile([128, 4, WN], BF16, tag="wtT")
        RN = WN // 2

        def ln_half_groups(r, psum_pool, ps_tag):
            st = {}

            def u_sq():
                wsq = lnp.tile([128, 4, RN], BF16, tag="wsq", name=f"wsq_{r}")
                src = wt_view[:, :, r * RN:(r + 1) * RN]
                if r == 1:
                    # transition half: DVE is idle here and this avoids an
                    # ACT Square-table reload on the critical chain
                    nc.vector.tensor_mul(wsq[:], src, src)
                else:
                    nc.scalar.activation(wsq[:], src, AF.Square)
                st["wsq"] = wsq

            def u_moments():
                ps = psum_pool.tile([128, 2, RN], F32, tag=ps_tag,
                                    name=f"ln_ps_{r}")
                for j in range(4):
                    nc.tensor.matmul(ps[:, 0, :], ones_full[:],
                                     wt_view[:, j, r * RN:(r + 1) * RN],
                                     start=(j == 0), stop=(j == 3),
                                     skip_group_check=True)
                    nc.tensor.matmul(ps[:, 1, :], ones_full[:], st["wsq"][:, j, :],
                                     start=(j == 0), stop=(j == 3),
                                     skip_group_check=True)
                mu = lnp.tile([128, RN], F32, tag="mu_sb", name=f"mu_{r}")
                nc.scalar.mul(mu[:], ps[:, 0, :], 1.0 / D)
                ex2 = lnp.tile([128, RN], F32, tag="ex2_sb", name=f"ex2_{r}")
                nc.scalar.mul(ex2[:], ps[:, 1, :], 1.0 / D)
                st["mu"], st["ex2"] = mu, ex2

            def u_stats():
                mu, ex2 = st["mu"], st["ex2"]
                var = lnp.tile([128, RN], F32, tag="var_sb", name=f"var_{r}")
                nc.vector.tensor_mul(var[:], mu[:], mu[:])
                nc.vector.tensor_sub(var[:], ex2[:], var[:])
                sd = lnp.tile([128, RN], F32, tag="sd", name=f"sd_{r}")
                nc.scalar.activation(sd[:], var[:], AF.Sqrt, bias=eps_sb[:])
                rstd = lnp.tile([128, RN], F32, tag="rstd", name=f"rstd_{r}")
                nc.vector.reciprocal(rstd[:], sd[:])
                st["rstd"] = rstd

            def u_ln(j):
                tmp = lnp.tile([128, RN], F32, tag="lntmp", bufs=2,
                               name=f"lnt_{r}_{j}")
                nc.vector.tensor_sub(tmp[:], wt_view[:, j, r * RN:(r + 1) * RN],
                                     st["mu"][:])
                nc.vector.tensor_mul(tmp[:], tmp[:], st["rstd"][:])
                nc.scalar.activation(wtn[:, j, r * RN:(r + 1) * RN], tmp[:],
                                     AF.Gelu,
                                     bias=bias_cols["ln_b_c"][:, j:j + 1],
                                     scale=bias_cols["ln_g_c"][:, j:j + 1])

            def u_pp(j):
                ps = psum_pool.tile([128, 2, RN], F32, tag=ps_tag,
                                    name=f"pp_{r}_{j}")
                for g in range(2):
                    nc.tensor.matmul(ps[:, 0, :],
                                     W["wpq8"][:, g, :, j * 128:(j + 1) * 128],
                                     wtn[:, 2 * g:2 * g + 2, r * RN:(r + 1) * RN],
                                     start=(g == 0), stop=(g == 1),
                                     perf_mode=DR, skip_group_check=True)
                    nc.tensor.matmul(ps[:, 1, :],
                                     W["wpk8"][:, g, :, j * 128:(j + 1) * 128],
                                     wtn[:, 2 * g:2 * g + 2, r * RN:(r + 1) * RN],
                                     start=(g == 0), stop=(g == 1),
                                     perf_mode=DR, skip_group_check=True)
                nc.scalar.activation(pqT[:, j, r * RN:(r + 1) * RN],
                                     ps[:, 0, :], AF.Identity,
                                     bias=bias_cols["bpq_c"][:, j:j + 1],
                                     scale=1.0 / W8SCALE)
                nc.scalar.activation(pkT[:, j, r * RN:(r + 1) * RN],
                                     ps[:, 1, :], AF.Identity,
                                     bias=bias_cols["bpk_c"][:, j:j + 1],
                                     scale=1.0 / W8SCALE)

            g = [u_sq, u_moments, u_stats]
            g += [lambda j=j: u_ln(j) for j in range(4)]
            g += [lambda j=j: u_pp(j) for j in range(4)]
            return g

        # ================= phase 1 =================
        with ExitStack() as ctx:
            p1 = ctx.enter_context(tc.tile_pool(name="p1", bufs=1))
            kT = p1.tile([128, 4, 3, CH], BF16, tag="kT")        # ring of 3 chunks
            qpT = p1.tile([128, 4, 3, CH], BF16, tag="qpT")      # ring of 3 chunks
            vtm = p1.tile([128, 12, 8, 65], BF16, tag="vtm")     # ring of 12 tiles, 65-col/head
            nc.vector.memset(vtm[:, :, :, 64:65], 1.0)           # ones col for denominators
            # projected-q ring of 3 chunk slots with 2-col halos on each side:
            # slot layout [0:2]=left halo, [2:CH+2]=chunk body, [CH+2:CH+4]=right halo
            qraw = p1.tile([128, 4, 3, CH + 4], BF16, tag="qraw")
            nc.vector.memset(qraw[:, :, 0, 0:2], 0.0)            # chunk 0 left edge

            xtp = ctx.enter_context(tc.tile_pool(name="xtp", bufs=3))
            ps_proj = ctx.enter_context(tc.tile_pool(name="ps_proj", bufs=2, space="PSUM"))
            ps_st = ctx.enter_context(tc.tile_pool(name="ps_st", bufs=1, space="PSUM"))
            ps_out = ctx.enter_context(tc.tile_pool(name="ps_out", bufs=1, space="PSUM"))
            dsa_sb = ctx.enter_context(tc.tile_pool(name="dsa_sb", bufs=3))
            pool_tmp = ctx.enter_context(tc.tile_pool(name="pool_tmp", bufs=2))
            ax_pool = ctx.enter_context(tc.tile_pool(name="ax_sb", bufs=2))

            def load_xt_super(sc):
                g, row0 = SC2CAST[sc]
                tiles = {}
                # v: bf16, 4 feature groups of 128. q/k: fp8 transposed as
                # uint16 feature-PAIRS (2 groups of 128 pairs) -> partition p
                # of group gg holds features 2*(gg*128+p), 2*(gg*128+p)+1
                # interleaved, exactly the DoubleRow [K, 2, M] layout.
                xt = xtp.tile([128, 4, SCH], BF16, tag="xt_v",
                              name=f"xt_v_{sc}")
                for dc in range(4):
                    ti = nc.sync.dma_start(
                        out=xt[:, dc, :],
                        in_=t[f"vb{g}"][row0:row0 + SCH,
                                        dc * 128:(dc + 1) * 128],
                        transpose=True)
                    add_dep_helper(ti.ins, cast_insts[("v", g)].ins,
                                   reason="transpose reads cast output")
                tiles["v"] = xt
                for nm in ("q", "k"):
                    xt = xtp.tile([128, 2, SCH], mybir.dt.uint16,
                                  tag=f"xt_{nm}", name=f"xt_{nm}_{sc}")
                    src16 = t[f"{nm}b{g}"].ap().bitcast(mybir.dt.uint16)
                    for gg in range(2):
                        ti = nc.sync.dma_start(
                            out=xt[:, gg, :],
                            in_=src16[row0:row0 + SCH,
                                      gg * 128:(gg + 1) * 128],
                            transpose=True)
                        add_dep_helper(ti.ins, cast_insts[(nm, g)].ins,
                                       reason="transpose reads cast output")
                    tiles[nm] = xt
                return tiles

            def proj_fm_pair(xt, off, wname, c, jp):
                """Projections for j-group pair (2jp, 2jp+1) into one 2-bank
                psum. q: one paired DVE add (bias [128,2,1] broadcast) writes
                qraw at 64x scale (the 1/64 is folded into the DSA exp scale,
                host pre-scales bq by 64). k: two per-j biased ACT copies
                (ACT bias APs are per-partition scalars only)."""
                ps = ps_proj.tile([128, 2, CH], F32, tag="proj",
                                  name=f"ps_{wname}_{jp}")
                for jj in range(2):
                    j = 2 * jp + jj
                    for g in range(2):
                        rhs = (xt[:, g, off:off + CH].bitcast(FP8)
                               .rearrange("p (n s) -> p s n", s=2))
                        nc.tensor.matmul(ps[:, jj, :],
                                         W[wname][:, g, :, j * 128:(j + 1) * 128],
                                         rhs, start=(g == 0), stop=(g == 1),
                                         perf_mode=DR, skip_group_check=True)
                if wname == "wq8":
                    nc.vector.tensor_add(
                        qraw[:, 2 * jp:2 * jp + 2, c % 3, 2:2 + CH], ps[:],
                        bias_cols["bq64_c"][:, 2 * jp:2 * jp + 2]
                        .unsqueeze(2).to_broadcast((128, 2, CH)))
                else:
                    for jj in range(2):
                        j = 2 * jp + jj
                        nc.scalar.activation(kT[:, j, c % 3, :], ps[:, jj, :],
                                             AF.Identity,
                                             bias=bias_cols["bk_c"][:, j:j + 1],
                                             scale=1.0 / W8SCALE)

            def proj_v_pair(xt, off, c, tp):
                ps = ps_proj.tile([128, 2, D], F32, tag="proj", name=f"ps_v_{tp}")
                for tt_ in range(2):
                    tt = 2 * tp + tt_
                    for dk in range(4):
                        nc.tensor.matmul(ps[:, tt_, :],
                                         xt[:, dk, off + tt * 128:off + (tt + 1) * 128],
                                         W["wv"][:, dk, :], start=(dk == 0), stop=False,
                                         skip_group_check=True)
                    nc.tensor.matmul(ps[:, tt_, :], ones_row[:], bv_sb[:], start=False,
                                     stop=True, skip_group_check=True)
                s = (c * 4 + 2 * tp) % 12
                nc.scalar.copy(vtm[:, s:s + 2, :, 0:64],
                               ps[:].rearrange("p t (h d) -> p t h d", h=H))

            def halo_copies(c):
                """After chunk c's q-projections land in slot c%3, export its
                edges into the neighbouring slots' halo columns."""
                if c > 0:
                    nc.scalar.copy(qraw[:, :, (c - 1) % 3, CH + 2:CH + 4],
                                   qraw[:, :, c % 3, 2:4])
                if c + 1 < NCHUNK:
                    nc.scalar.copy(qraw[:, :, (c + 1) % 3, 0:2],
                                   qraw[:, :, c % 3, CH:CH + 2])
                else:
                    nc.vector.memset(qraw[:, :, c % 3, CH + 2:CH + 4], 0.0)

            def pool_chunk(c):
                s = c % 3
                ta = pool_tmp.tile([128, 4, CH + 2], BF16, tag="ta")
                nc.vector.tensor_add(ta[:], qraw[:, :, s, 0:CH + 2],
                                     qraw[:, :, s, 1:CH + 3])
                tb = pool_tmp.tile([128, 4, CH], BF16, tag="tb")
                nc.vector.tensor_add(tb[:], ta[:, :, 0:CH], ta[:, :, 2:CH + 2])
                nc.vector.tensor_add(qpT[:, :, c % 3, :], tb[:],
                                     qraw[:, :, s, 4:CH + 4])

            def dsa_scores(c, lt):
                """MM1 (+ rank-17 additive mask) + exp for tile lt of chunk c."""
                st = ps_st.tile([128, 8, 128], F32, tag="st", name=f"st_{c}_{lt}")
                for h in range(H):
                    hp = PERM[h]
                    base = (h % 2) * 64
                    lhsT = kT[base:base + 64, h // 2, c % 3, lt * 128:(lt + 1) * 128]
                    rhs = qpT[base:base + 64, h // 2, c % 3, lt * 128:(lt + 1) * 128]
                    nc.tensor.matmul(st[:, hp, :], lhsT, rhs, start=True, stop=True,
                                     skip_group_check=True)
                expS = dsa_sb.tile([128, 8, 128], BF16, tag="expS",
                                   name=f"expS_{c}_{lt}")
                # qpT carries a 64x scale (folded out here); alternate the
                # mask mul between DVE and GPSIMD to balance engine load
                nc.scalar.activation(expS[:], st[:], AF.Exp,
                                     scale=SCALE / QNB / W8SCALE)
                eng = nc.vector if lt % 2 == 0 else nc.gpsimd
                eng.tensor_mul(expS[:], expS[:],
                               mask_sb[:].unsqueeze(1).to_broadcast((128, 8, 128)))
                return expS

            def dsa_out(c, lt, masked, ax_out):
                """attn@V with ones-col denominators, then normalize."""
                outp = ps_out.tile([128, 2, 512], F32, tag="outp",
                                   name=f"outp_{c}_{lt}")
                for h in range(H):
                    hp = PERM[h]
                    nc.tensor.matmul(outp[:, h // 4, (h % 4) * 65:(h % 4) * 65 + 65],
                                     masked[:, hp, :],
                                     vtm[:, (c * 4 + lt) % 12, h, :],
                                     start=True, stop=True, skip_group_check=True)
                recip = dsa_sb.tile([128, 2, 4], F32, tag="recip",
                                    name=f"recip_{c}_{lt}")
                den_view = bass.AP(outp.tensor, outp[:].offset + 64,
                                   [outp[:].ap[0], [512, 2], [65, 4]])
                nc.vector.reciprocal(recip[:], den_view)
                # V' already contains +bv (rank-1 matmul in proj_v); attention
                # weights sum to 1 after the 1/den scale, so bias is exact.
                av_view = bass.AP(outp.tensor, outp[:].offset,
                                  [outp[:].ap[0], [512, 2], [65, 4], [1, 64]])
                nc.vector.tensor_mul(
                    ax_out.rearrange("p (a b d) -> p a b d", a=2, b=4),
                    av_view,
                    recip[:].unsqueeze(3).to_broadcast((128, 2, 4, 64)))

            def dsa_group_list(c, ax):
                masked = {}
                g = []
                g.append(lambda: masked.__setitem__(0, dsa_scores(c, 0)))
                g.append(lambda: masked.__setitem__(1, dsa_scores(c, 1)))
                g.append(lambda: dsa_out(c, 0, masked.pop(0), ax[:, 0, :]))
                g.append(lambda: masked.__setitem__(2, dsa_scores(c, 2)))
                g.append(lambda: dsa_out(c, 1, masked.pop(1), ax[:, 1, :]))
                g.append(lambda: masked.__setitem__(3, dsa_scores(c, 3)))
                g.append(lambda: dsa_out(c, 2, masked.pop(2), ax[:, 2, :]))
                g.append(lambda: dsa_out(c, 3, masked.pop(3), ax[:, 3, :]))
                return g

            def store_ax_pair(cp, ax2):
                """Store DSA output for chunks (cp, cp+1), then transpose the
                pair back feature-major (overlaps phase 1). The last pair is
                stored per chunk so the final win_tok transposes - which gate
                the LN chain at the phase transition - wait on a half-size
                store."""
                wsrc = axd.ap().rearrange("(w s) d -> w s d", s=PW)
                dst = axd.ap().rearrange("(g lt p) d -> g p lt d", lt=8, p=128)
                wi = nc.gpsimd.dma_start(out=dst[cp // 2], in_=ax2[:])
                wis = [wi]
                for dc in range(4):
                    wt = nc.sync.dma_start(
                        out=wt_view[:, dc, cp * 64:(cp + 2) * 64],
                        in_=wsrc[cp * 64:(cp + 2) * 64, 0,
                                 dc * 128:(dc + 1) * 128],
                        transpose=True)
                    add_dep_helper(wt.ins, wi.ins,
                                   reason="win_tok transpose reads axd pair")
                axd_writers.append(wis)
                tis = []
                for dc in range(4):
                    ti = nc.sync.dma_start(
                        out=axt[:, dc, cp * CH:(cp + 2) * CH],
                        in_=axd[cp * CH:(cp + 2) * CH, dc * 128:(dc + 1) * 128],
                        transpose=True)
                    for wi in wis:
                        add_dep_helper(ti.ins, wi.ins,
                                       reason="axt transpose reads axd pair")
                    tis.append(ti)
                axt_trans.append(tis)

            xt_tiles = {0: load_xt_super(0)}
            extra = []      # deferred phase-2 prologue work units
            ax2 = None

            for c in range(NCHUNK + 2):
                if c == 5:
                    load_phase2_weights()
                if c == 7:
                    extra.extend(ln_half_groups(0, ps_proj, "proj"))
                pgroups = []
                if c < NCHUNK:
                    sc = c // 2
                    off = (c % 2) * CH
                    if c % 2 == 0 and sc + 1 < NSC:
                        xt_tiles[sc + 1] = load_xt_super(sc + 1)
                    qxt = xt_tiles[sc]["q"]
                    kxt = xt_tiles[sc]["k"]
                    vxt = xt_tiles[sc]["v"]
                    for jp in range(2):
                        pgroups.append(lambda jp=jp, x=qxt, o=off, c=c:
                                       proj_fm_pair(x, o, "wq8", c, jp))
                    for tp in range(2):
                        pgroups.append(lambda tp=tp, x=vxt, o=off, c=c:
                                       proj_v_pair(x, o, c, tp))
                    for jp in range(2):
                        pgroups.append(lambda jp=jp, x=kxt, o=off, c=c:
                                       proj_fm_pair(x, o, "wk8", c, jp))
                dgroups = []
                dc_ = c - 2
                if dc_ >= 0:
                    if dc_ % 2 == 0:
                        ax2 = ax_pool.tile([128, 8, D], BF16, tag="ax",
                                           name=f"ax_{dc_}")
                    axv = ax2[:, (dc_ % 2) * 4:(dc_ % 2) * 4 + 4, :]
                    dgroups = dsa_group_list(dc_, axv)
                # weave: spread D groups evenly through the P stream;
                # pool(c-1) after the 2 Q-projection pairs; extra units fill
                # remaining slots late in phase 1
                npg, ndg = len(pgroups), len(dgroups)
                if npg:
                    # per-pgroup D-group quota (6 pgroups hosting up to 8)
                    quota = (1, 2, 1, 2, 1, 1)
                    di = 0
                    for i in range(npg):
                        pgroups[i]()
                        if i == 2:
                            halo_copies(c)
                            if c >= 1:
                                pool_chunk(c - 1)
                        for _ in range(quota[i]):
                            if di < ndg:
                                dgroups[di]()
                                di += 1
                                if extra:
                                    extra.pop(0)()
                    while di < ndg:
                        dgroups[di]()
                        di += 1
                else:
                    if 1 <= c <= NCHUNK:
                        pool_chunk(c - 1)
                    for g in dgroups:
                        g()
                        if extra:
                            extra.pop(0)()
                if dc_ >= 0 and dc_ % 2 == 1:
                    store_ax_pair(dc_ - 1, ax2)
            while extra:
                extra.pop(0)()

        # ================= phase 2 =================
        with ExitStack() as ctx:
            p2 = ctx.enter_context(tc.tile_pool(name="p2", bufs=1))
            ps2 = ctx.enter_context(tc.tile_pool(name="ps2", bufs=2, space="PSUM"))
            ps2b = ctx.enter_context(tc.tile_pool(name="ps2b", bufs=2, space="PSUM"))
            sb2 = ctx.enter_context(tc.tile_pool(name="sb2", bufs=2))

            bo_sb = singles.tile([1, D], BF16)
            nc.scalar.dma_start(out=bo_sb[:], in_=t["bo_r"][:, :])

            # pv gathered window-major with a SWDGE cast to fp8 for the
            # DoubleRow pout matmuls.
            pv = p2.tile([128, 4, WIN, D], FP8, tag="pv")
            srcv = axd.ap().rearrange("(cc p w) d -> cc p w d", p=128, w=PW)
            for cc in range(4):
                gi = nc.gpsimd.dma_start(out=pv[:, cc, :, :], in_=srcv[cc, :, 1:PW, :])
                for wi in axd_writers[cc]:
                    add_dep_helper(gi.ins, wi.ins, reason="pv gather")
                # ordering shim: run the gathers after the last attn_x pair's
                # feature-major transposes so the transition chain
                # (store -> axt -> LN -> pq/pk -> PSA) isn't queued behind
                # them; pout doesn't need pv until well after.
                for ti in axt_trans[-1]:
                    add_dep_helper(gi.ins, ti.ins, reason="dma order")

            # ---- LN + GELU + pq/pk for the second window half ----
            for u in ln_half_groups(1, ps2, "ps2"):
                u()

            # ---- PSA softmax per head; pout per head-pair right after ----
            # Scores run fp8 DoubleRow over [32, 2] feature tiles; exp output
            # stays UNNORMALIZED in fp8 (values ~1.0, ideal e4m3 range). pout
            # contracts raw exp against fp8 pv, and the 1/den normalization is
            # applied afterwards on the psum via a per-pair recip tile whose
            # partition rows are already head-matched (h0 on 0:64, h1 on
            # 64:128) thanks to the DoubleRow den matmul's 64-row output.
            zt = p2.tile([128, 4, QLEN], BF16, tag="zt")
            ones8_2 = p2.tile([128, 2, 128], FP8, tag="ones8")
            nc.vector.memset(ones8_2[:], 1.0)

            def psa_scores(h):
                b32 = 32 * (h % 4)
                a = 2 * (h // 4)
                es = sb2.tile([128, 4, WN], FP8, tag="psa_exp", bufs=8,
                              name=f"es_{h}")
                for cp in range(2):
                    ps = ps2b.tile([128, 2, WN], F32, tag="pair",
                                   name=f"st_{h}_{cp}")
                    for ch in range(2):
                        cc = cp * 2 + ch
                        nc.tensor.matmul(
                            ps[:, ch, :],
                            pkT[b32:b32 + 32, a:a + 2, cc * 128:(cc + 1) * 128],
                            pqT[b32:b32 + 32, a:a + 2, :], start=True, stop=True,
                            perf_mode=DR, skip_group_check=True,
                            tile_position=(b32, 0))
                    nc.scalar.activation(es[:, 2 * cp:2 * cp + 2, :], ps[:],
                                         AF.Exp, scale=SCALE)
                return es

            def psa_norm(j, es0, es1):
                """den + recip for head pair j (heads 2j, 2j+1). DoubleRow
                outputs must sit at column position 0, so each head gets a
                full 128-partition replicated den psum; the recips then read
                partition-aligned halves into one pair tile (h0 rows on 0:64,
                h1 on 64:128) for the single pout normalization mul."""
                recipd = sb2.tile([128, WN], F32, tag="psa_recip", bufs=2,
                                  name=f"r_{j}")
                for half, es in ((0, es0), (1, es1)):
                    ps_den = ps2b.tile([128, WN], F32, tag="psa_den", bufs=2,
                                       name=f"d_{j}_{half}")
                    for cp in range(2):
                        nc.tensor.matmul(
                            ps_den[:], ones8_2[:], es[:, 2 * cp:2 * cp + 2, :],
                            start=(cp == 0), stop=(cp == 1),
                            perf_mode=DR, skip_group_check=True)
                    nc.vector.reciprocal(recipd[half * 64:(half + 1) * 64, :],
                                         ps_den[half * 64:(half + 1) * 64, :])
                return recipd

            def pout_pair(j, wh, es0, es1, recipd):
                """pout for head-pair j over query-window half wh."""
                w0 = wh * (WN // 2)
                for i in range(WIN):
                    po = ps2.tile([128, WN // 2], F32, tag="ps2",
                                  name=f"po_{j}_{i}_{wh}")
                    # DoubleRow requires output column position 0, so only
                    # the half-0 head runs DR; half-1 (psum base 64) uses
                    # plain fp8 matmuls.
                    h0 = 2 * j
                    for cp in range(2):
                        nc.tensor.matmul(
                            po[0:64, :],
                            pv[:, 2 * cp:2 * cp + 2, i, h0 * 64:(h0 + 1) * 64],
                            es0[:, 2 * cp:2 * cp + 2, w0:w0 + WN // 2],
                            start=(cp == 0), stop=(cp == 1),
                            perf_mode=DR, skip_group_check=True)
                    h1 = 2 * j + 1
                    for cc in range(4):
                        nc.tensor.matmul(
                            po[64:128, :],
                            pv[:, cc, i, h1 * 64:(h1 + 1) * 64],
                            es1[:, cc, w0:w0 + WN // 2],
                            start=(cc == 0), stop=(cc == 3),
                            skip_group_check=True)
                    pn = sb2.tile([128, WN // 2], BF16, tag="pn", bufs=4,
                                  name=f"pn_{j}_{i}_{wh}")
                    nc.vector.tensor_mul(pn[:], po[:], recipd[:, w0:w0 + WN // 2])
                    # SBUF-only bf16 add: run it on GPSIMD to keep DVE free
                    # for the psum-reading normalization muls
                    nc.gpsimd.tensor_add(
                        zt[:, j, :].rearrange("p (w i) -> p w i", i=WIN)
                        [:, w0:w0 + WN // 2, i],
                        pn[:],
                        axt[:, j, :].rearrange("p (w s) -> p w s", s=PW)
                        [:, w0:w0 + WN // 2, 1 + i])

            outv = out.ap().rearrange("(g tt p) d -> g p tt d", tt=2, p=128)

            def final_group(g, split_store=False):
                o_sb = sb2.tile([128, 2, D], BF16, tag="osb", bufs=4,
                                name=f"osb_{g}")
                for q in range(2):
                    tt = g * 2 + q
                    ps = ps2.tile([128, D], F32, tag="ps2", name=f"fin_{tt}")
                    for dk in range(4):
                        nc.tensor.matmul(ps[:], zt[:, dk, tt * 128:(tt + 1) * 128],
                                         W["wo"][:, dk, :], start=(dk == 0),
                                         stop=False, skip_group_check=True)
                    # bo via rank-1 matmul; psum evacuation on ACT (idle in
                    # the fin tail) instead of a DVE add
                    nc.tensor.matmul(ps[:], ones_row[:], bo_sb[:], start=False,
                                     stop=True, skip_group_check=True)
                    nc.scalar.copy(o_sb[:, q, :], ps[:])
                    if split_store:
                        nc.sync.dma_start(out=outv[g][:, q, :],
                                          in_=o_sb[:, q, :])
                if not split_store:
                    nc.sync.dma_start(out=outv[g], in_=o_sb[:])

            # pipeline: scores(h+1) | norm(j) once its pair of heads is
            # scored | pout(j) right after; the last pair is split by
            # query-window half so the first finals overlap its second half
            es_store = {0: psa_scores(0)}
            recs = {}
            done_pairs = 0
            for h in range(1, H):
                es_store[h] = psa_scores(h)
                if h % 2 == 1:
                    j = h // 2
                    recs[j] = psa_norm(j, es_store[2 * j], es_store[2 * j + 1])
                if h % 2 == 0 and done_pairs in recs:
                    j = done_pairs
                    pout_pair(j, 0, es_store[2 * j], es_store[2 * j + 1], recs[j])
                    pout_pair(j, 1, es_store[2 * j], es_store[2 * j + 1], recs[j])
                    es_store.pop(2 * j), es_store.pop(2 * j + 1), recs.pop(j)
                    done_pairs += 1
            while done_pairs < 4:
                j = done_pairs
                pout_pair(j, 0, es_store[2 * j], es_store[2 * j + 1], recs[j])
                if j == 3:
                    for g in range(7):
                        final_group(g)
                pout_pair(j, 1, es_store[2 * j], es_store[2 * j + 1], recs[j])
                done_pairs += 1
            for g in range(7, QLEN // 256):
                final_group(g, split_store=(g >= QLEN // 256 - 2))


_NC_CACHE = None


def _get_program():
    global _NC_CACHE
    if _NC_CACHE is None:
        _NC_CACHE = build_program()
    return _NC_CACHE


def _fp8_paired(Wm, perm=None, pair="dc"):
    """Host prep for fp8 DoubleRow lhsT: optional column permutation, x64
    scale, then row pairing. pair="dc": rows (2g+s)*128+p -> [g*128+p, s]
    (matches the wtn dc-group layout); pair="consec": rows 2f+s -> [f, s]
    (matches the uint16-pair input transposes)."""
    w = np.asarray(Wm, np.float32)
    if perm is not None:
        w = w[:, perm]
    w = (w * W8SCALE).astype(ml_dtypes.float8_e4m3fn)
    if pair == "consec":
        return np.ascontiguousarray(w.reshape(256, 2, D))
    # rows: r = g*256 + s*128 + p  ->  out[g*128+p, s, :]
    return np.ascontiguousarray(
        w.reshape(2, 2, 128, D).transpose(0, 2, 1, 3).reshape(256, 2, D))


def _host_consts(Wk, bk, Wv, bv, Wq, bq, ln_g, ln_b, Wpq, bpq, Wpk, bpk, Wo, bo):
    bf = ml_dtypes.bfloat16
    col = lambda b: np.asarray(b, np.float32).reshape(4, 128).T.copy()
    perm = np.asarray(PSA_PERM)
    consts = {
        "wq8": _fp8_paired(Wq, pair="consec"),
        "wk8": _fp8_paired(Wk, pair="consec"),
        "wv": np.asarray(Wv, np.float32).astype(bf),
        "wpq8": _fp8_paired(Wpq, perm),
        "wpk8": _fp8_paired(Wpk, perm),
        "wo": np.asarray(Wo, np.float32).astype(bf),
        "bq64_c": col(np.asarray(bq, np.float32) * W8SCALE), "bk_c": col(bk),
        "bpq_c": col(np.asarray(bpq, np.float32)[perm]),
        "bpk_c": col(np.asarray(bpk, np.float32)[perm]),
        "ln_g_c": col(ln_g), "ln_b_c": col(ln_b),
        "bv_r": np.asarray(bv, np.float32).reshape(1, D).astype(bf),
        "bo_r": np.asarray(bo, np.float32).reshape(1, D).astype(bf),
        "bv_f": np.tile(np.asarray(bv, np.float32).reshape(1, D), (128, 1)).astype(bf),
        "bo_f": np.tile(np.asarray(bo, np.float32).reshape(1, D), (128, 1)).astype(bf),
    }
    m = np.zeros((128, 128), np.float32)
    for g in range(16):
        m[g * PW:(g + 1) * PW, g * PW:(g + 1) * PW] = 1.0
    consts["bmask"] = m.astype(bf)
    return consts


def kernel(k, v, q, query_len, Wk, bk, Wv, bv, Wq, bq, ln_g, ln_b,
           Wpq, bpq, Wpk, bpk, Wo, bo):
    nc = _get_program()
    consts = _host_consts(Wk, bk, Wv, bv, Wq, bq, ln_g, ln_b,
                          Wpq, bpq, Wpk, bpk, Wo, bo)
    k = np.asarray(k, np.float32)
    v = np.asarray(v, np.float32)
    q = np.asarray(q, np.float32)
    in_maps = []
    for b in range(B):
        m = {"q": np.ascontiguousarray(q[b]), "k": np.ascontiguousarray(k[b]),
             "v": np.ascontiguousarray(v[b])}
        m.update(consts)
        in_maps.append(m)
    res = run_bass_kernel_spmd(nc, in_maps, core_ids=list(range(B)))
    return np.stack([np.asarray(res.results[b]["out"], np.float32)
                     for b in range(B)], axis=0)


if __name__ == "__main__":
    nc = build_program()
    print("program built ok")



# revision 87
# speedup vs baseline: 1.0123x; 1.0059x over previous
"""Trainium2 Bass kernel for DeformableMultiHeadedAttention.

Data-parallel over batch B=8 across 8 NeuronCores (one batch element per
core, identical programs, no collectives). Heavy matmuls run fp8-e4m3
DoubleRow (0.5 PE cycles/row) wherever a numpy precision study showed the
final rel-err stays ~0.006 (tolerance 2e-2); V projection and the final
Z@Wo stay bf16 (fp8 there blows the budget).

Per-core pipeline (f32 psum accumulate everywhere):
  1. q,k f32 -> SWDGE cast-DMA -> DRAM fp8; v -> bf16. q/k DMA-transpose as
     uint16 feature-PAIRS, which lands directly in DoubleRow's [K,2,M]
     operand layout; v transposes bf16 feature-major.
  2. Projections on PE: Q'/K' via fp8 DoubleRow (weights host-scaled x64,
     paired rows [2f, 2f+1]); 1/64 descales in the psum-evacuation copies
     (k via per-j ACT Identity+bias) or folds into the DSA exp scale (qraw
     keeps 64x, one paired DVE add with a [128,2,1]-broadcast bias). V'
     token-major bf16 with bias via a K=1 rank-1 matmul.
  3. Q pooling (AvgPool k=5, zero pad) as 3 shifted DVE adds; 1/5 folded
     into the exp scale.
  4. DSA (windows of 8): per 128-token tile: bank-segregated 64-row score
     matmuls, exp on ACT, block-diag mask mul alternating DVE/GPSIMD,
     attn@V with a ones-column denominator, 1/den scale on DVE. Output
     token-major -> DRAM (axd, bf16).
  5. Re-layouts from axd: wt_view (window-summary transposes), axt
     (feature-major attn_x), pv (window-major payload, SWDGE cast to fp8).
  6. win_tok LayerNorm + exact GELU -> wtn fp8; pq/pk projections fp8
     DoubleRow with host column-permuted weights so each head sits on 32
     partitions x 2 dc-slots; PSA scores fp8 DoubleRow per head.
  7. PSA softmax normalization deferred: exp stays UNNORMALIZED in fp8
     (values ~1.0, ideal e4m3 range); pout contracts raw exp against fp8 pv
     (DoubleRow half-0 head / plain fp8 half-1 head - DoubleRow outputs must
     sit at psum column position 0); per-pair replicated den (DoubleRow
     ones matmuls) -> recip -> one DVE mul on the pout psum; Z = pn + attn_x
     via GPSIMD adds (SBUF-only bf16).
  8. Final out = Z @ Wo bf16 with bo via rank-1 matmul, ACT psum copies,
     bf16 DRAM stores (host upcasts to f32).
"""

import sys
from contextlib import ExitStack

for _p in ("/opt/trn_rl_repo/concourse", "/opt/trn_rl_repo"):
    if _p not in sys.path:
        sys.path.insert(0, _p)

import numpy as np
import ml_dtypes

import concourse.bass as bass
import concourse.mybir as mybir
import concourse.tile as tile
from concourse import bacc
from concourse.tile import add_dep_helper
from concourse.bass_utils import run_bass_kernel_spmd

BF16 = mybir.dt.bfloat16
F32 = mybir.dt.float32
FP8 = mybir.dt.float8e4
DR = mybir.MatmulPerfMode.DoubleRow
AF = mybir.ActivationFunctionType
ALU = mybir.AluOpType

B, M, D = 8, 4096, 512
H, HD = 8, 64
WIN = 7
PW = WIN + 1
QNB = 5
QLEN = 3584
WN = M // PW
SCALE = D ** -0.5
EPS = 1e-5
NCHUNK = 8
CH = 512
SCH = 1024           # super-chunk (transpose batch) size
NSC = M // SCH
CAST_RANGES = [(0, 1024), (1024, 2048), (2048, 4096)]
SC2CAST = {0: (0, 0), 1: (1, 0), 2: (2, 0), 3: (2, 1024)}  # sc -> (group, row0)
PERM = [(h % 2) * 4 + h // 2 for h in range(H)]  # head -> DSA psum slot
W8SCALE = 64.0  # host pre-scale on fp8 weights; 1/64 folded into psum copies

# PSA pq/pk column permutation: dc-group j, partition p -> original dout.
# Head h occupies 32 partitions at 32*(h%4) across the dc pair 2*(h//4),
# with features 0:32 in the even dc and 32:64 in the odd dc, so psa score
# matmuls can run fp8 DoubleRow over [32, 2] feature tiles.
PSA_PERM = [((j // 2) * 4 + p // 32) * 64 + (j % 2) * 32 + (p % 32)
            for j in range(4) for p in range(128)]


def build_program():
    nc = bacc.Bacc("TRN2", target_bir_lowering=False, debug=False, num_devices=8)

    t = {}
    t["q_in"] = nc.dram_tensor("q", [M, D], F32, kind="ExternalInput")
    t["k_in"] = nc.dram_tensor("k", [M, D], F32, kind="ExternalInput")
    t["v_in"] = nc.dram_tensor("v", [M, D], F32, kind="ExternalInput")
    for nm in ("wv", "wo"):
        t[nm] = nc.dram_tensor(nm, [D, D], BF16, kind="ExternalInput")
    for nm in ("wq8", "wk8", "wpq8", "wpk8"):
        t[nm] = nc.dram_tensor(nm, [256, 2, D], FP8, kind="ExternalInput")
    for nm in ("bq64_c", "bk_c", "bpq_c", "bpk_c", "ln_g_c", "ln_b_c"):
        t[nm] = nc.dram_tensor(nm, [128, 4], F32, kind="ExternalInput")
    t["bv_r"] = nc.dram_tensor("bv_r", [1, D], BF16, kind="ExternalInput")
    t["bo_r"] = nc.dram_tensor("bo_r", [1, D], BF16, kind="ExternalInput")
    t["bv_f"] = nc.dram_tensor("bv_f", [128, D], BF16, kind="ExternalInput")
    t["bo_f"] = nc.dram_tensor("bo_f", [128, D], BF16, kind="ExternalInput")
    t["bmask"] = nc.dram_tensor("bmask", [128, 128], BF16, kind="ExternalInput")
    t["out"] = nc.dram_tensor("out", [QLEN, D], BF16, kind="ExternalOutput")
    t["axd"] = nc.dram_tensor("axd_s", [M, D], BF16, kind="Internal")
    t["zd"] = nc.dram_tensor("zd_s", [QLEN, D], BF16, kind="Internal")
    # per-group cast targets: separate tensors so the tile framework's
    # tensor-granular dependency tracking doesn't serialize casts behind
    # earlier chunks' transpose reads (false WAR). First two groups are
    # small so compute can start early.
    # q/k cast straight to fp8 (transposed later as uint16 feature-pairs,
    # which lands in exactly the DoubleRow [K,2,M] operand layout); v stays
    # bf16 for precision.
    for nm in ("q", "k", "v"):
        dt_ = BF16 if nm == "v" else FP8
        for g, (lo, hi) in enumerate(CAST_RANGES):
            t[f"{nm}b{g}"] = nc.dram_tensor(f"{nm}b{g}_s", [hi - lo, D], dt_,
                                            kind="Internal")

    with tile.TileContext(nc) as tc:
        _build(nc, tc, t)
    nc.compile()
    return nc


def _build(nc, tc, t):
    axd, out = t["axd"], t["out"]

    with ExitStack() as octx:
        singles = octx.enter_context(tc.tile_pool(name="singles", bufs=1))

        # f32->bf16 cast DMAs first; few large batches keep the SWDGE
        # completion-semaphore lanes from being recycled between casts.
        cast_insts = {}
        srcs_d = {"q": t["q_in"], "k": t["k_in"], "v": t["v_in"]}
        for g, (lo, hi) in enumerate(CAST_RANGES):
            for nm in ("q", "v", "k"):
                ci = nc.gpsimd.dma_start(
                    out=t[f"{nm}b{g}"][:, :],
                    in_=srcs_d[nm][lo:hi, :])
                cast_insts[(nm, g)] = ci

        W = {}
        # wq loads immediately (first projection needs it); the other loads
        # are gated behind the first v cast so the q/v transposes win the
        # startup DMA race.
        gate0 = cast_insts[("v", 0)]
        for nm in ("wq8", "wk8"):
            W[nm] = singles.tile([128, 2, 2, D], FP8, tag=nm, name=f"w_{nm}")
            wi = nc.scalar.dma_start(out=W[nm][:],
                                     in_=t[nm].ap().rearrange(
                                         "(g p) s d -> p g s d", p=128))
            if nm != "wq8":
                add_dep_helper(wi.ins, gate0.ins, reason="dma order")
        W["wv"] = singles.tile([128, 4, D], BF16, tag="wv", name="w_wv")
        wi = nc.scalar.dma_start(out=W["wv"][:],
                                 in_=t["wv"].ap().rearrange("(c p) d -> p c d",
                                                            p=128))
        add_dep_helper(wi.ins, gate0.ins, reason="dma order")
        bias_cols = {}
        for nm in ("bq64_c", "bk_c"):
            bias_cols[nm] = singles.tile([128, 4], F32, tag=nm, name=f"bc_{nm}")
            nc.scalar.dma_start(out=bias_cols[nm][:], in_=t[nm][:, :])
        bv_sb = singles.tile([1, D], BF16)
        nc.scalar.dma_start(out=bv_sb[:], in_=t["bv_r"][:, :])
        mask_sb = singles.tile([128, 128], BF16)
        wi = nc.scalar.dma_start(out=mask_sb[:], in_=t["bmask"][:, :])
        add_dep_helper(wi.ins, cast_insts[("k", 0)].ins, reason="dma order")
        ones_row = singles.tile([1, 128], BF16)
        nc.vector.memset(ones_row[:], 1.0)
        ones_col = singles.tile([128, 1], BF16)
        nc.vector.memset(ones_col[:], 1.0)
        ones_full = singles.tile([128, 128], BF16)
        nc.vector.memset(ones_full[:], 1.0)
        eps_sb = singles.tile([128, 1], F32)
        nc.vector.memset(eps_sb[:], EPS)

        axd_writers = []
        axt_trans = []
        p2a = octx.enter_context(tc.tile_pool(name="p2a", bufs=1))
        axt = p2a.tile([128, 4, M], BF16, tag="axt")

        def load_phase2_weights():
            # ordering shim: keep these dep-free loads from being hoisted by
            # the scheduler ahead of the startup-critical input transposes.
            # Gated on the first attn_x pair store (~1/3 through phase 1),
            # which fires well before Act SEQ reaches these instructions, so
            # no head-of-line blocking on the Act sequencer.
            W["wo"] = singles.tile([128, 4, D], BF16, tag="wo", name="w_wo")
            nc.scalar.dma_start(out=W["wo"][:],
                                in_=t["wo"].ap().rearrange(
                                    "(c p) d -> p c d", p=128))
            for nm in ("wpq8", "wpk8"):
                W[nm] = singles.tile([128, 2, 2, D], FP8, tag=nm, name=f"w_{nm}")
                nc.scalar.dma_start(out=W[nm][:],
                                    in_=t[nm].ap().rearrange(
                                        "(g p) s d -> p g s d", p=128))
            for nm in ("bpq_c", "bpk_c", "ln_g_c", "ln_b_c"):
                bias_cols[nm] = singles.tile([128, 4], F32, tag=nm, name=f"bc_{nm}")
                nc.scalar.dma_start(out=bias_cols[nm][:], in_=t[nm][:, :])

        # ---- win_tok LN + GELU + pq/pk projections, by window quarters.
        # Quarters 0-2 run inside phase 1 as their attn_x pairs land; only
        # quarter 3 remains for the phase transition.
        lnp = octx.enter_context(tc.tile_pool(name="lnp", bufs=1))
        wtn = lnp.tile([128, 4, WN], FP8, tag="wtn")
        pqT = lnp.tile([128, 4, WN], FP8, tag="pqT")
        pkT = lnp.tile([128, 4, WN], FP8, tag="pkT")
        # dedicated feature-major copy of the window-summary tokens: cheap
        # strided-row transposes that unblock LN without the full axt pair
        wt_view = lnp.tile([128, 4, WN], BF16, tag="wtT")
        RN = WN // 2

        def ln_half_groups(r, psum_pool, ps_tag):
            st = {}

            def u_sq():
                wsq = lnp.tile([128, 4, RN], BF16, tag="wsq", name=f"wsq_{r}")
                src = wt_view[:, :, r * RN:(r + 1) * RN]
                if r == 1:
                    # transition half: DVE is idle here and this avoids an
                    # ACT Square-table reload on the critical chain
                    nc.vector.tensor_mul(wsq[:], src, src)
                else:
                    nc.scalar.activation(wsq[:], src, AF.Square)
                st["wsq"] = wsq

            def u_moments():
                ps = psum_pool.tile([128, 2, RN], F32, tag=ps_tag,
                                    name=f"ln_ps_{r}")
                for j in range(4):
                    nc.tensor.matmul(ps[:, 0, :], ones_full[:],
                                     wt_view[:, j, r * RN:(r + 1) * RN],
                                     start=(j == 0), stop=(j == 3),
                                     skip_group_check=True)
                    nc.tensor.matmul(ps[:, 1, :], ones_full[:], st["wsq"][:, j, :],
                                     start=(j == 0), stop=(j == 3),
                                     skip_group_check=True)
                mu = lnp.tile([128, RN], F32, tag="mu_sb", name=f"mu_{r}")
                nc.scalar.mul(mu[:], ps[:, 0, :], 1.0 / D)
                ex2 = lnp.tile([128, RN], F32, tag="ex2_sb", name=f"ex2_{r}")
                nc.scalar.mul(ex2[:], ps[:, 1, :], 1.0 / D)
                st["mu"], st["ex2"] = mu, ex2

            def u_stats():
                mu, ex2 = st["mu"], st["ex2"]
                var = lnp.tile([128, RN], F32, tag="var_sb", name=f"var_{r}")
                nc.vector.tensor_mul(var[:], mu[:], mu[:])
                nc.vector.tensor_sub(var[:], ex2[:], var[:])
                sd = lnp.tile([128, RN], F32, tag="sd", name=f"sd_{r}")
                nc.scalar.activation(sd[:], var[:], AF.Sqrt, bias=eps_sb[:])
                rstd = lnp.tile([128, RN], F32, tag="rstd", name=f"rstd_{r}")
                nc.vector.reciprocal(rstd[:], sd[:])
                st["rstd"] = rstd

            def u_ln(j):
                tmp = lnp.tile([128, RN], F32, tag="lntmp", bufs=2,
                               name=f"lnt_{r}_{j}")
                nc.vector.tensor_sub(tmp[:], wt_view[:, j, r * RN:(r + 1) * RN],
                                     st["mu"][:])
                nc.vector.tensor_mul(tmp[:], tmp[:], st["rstd"][:])
                nc.scalar.activation(wtn[:, j, r * RN:(r + 1) * RN], tmp[:],
                                     AF.Gelu,
                                     bias=bias_cols["ln_b_c"][:, j:j + 1],
                                     scale=bias_cols["ln_g_c"][:, j:j + 1])

            def u_pp(j):
                ps = psum_pool.tile([128, 2, RN], F32, tag=ps_tag,
                                    name=f"pp_{r}_{j}")
                for g in range(2):
                    nc.tensor.matmul(ps[:, 0, :],
                                     W["wpq8"][:, g, :, j * 128:(j + 1) * 128],
                                     wtn[:, 2 * g:2 * g + 2, r * RN:(r + 1) * RN],
                                     start=(g == 0), stop=(g == 1),
                                     perf_mode=DR, skip_group_check=True)
                    nc.tensor.matmul(ps[:, 1, :],
                                     W["wpk8"][:, g, :, j * 128:(j + 1) * 128],
                                     wtn[:, 2 * g:2 * g + 2, r * RN:(r + 1) * RN],
                                     start=(g == 0), stop=(g == 1),
                                     perf_mode=DR, skip_group_check=True)
                nc.scalar.activation(pqT[:, j, r * RN:(r + 1) * RN],
                                     ps[:, 0, :], AF.Identity,
                                     bias=bias_cols["bpq_c"][:, j:j + 1],
                                     scale=1.0 / W8SCALE)
                nc.scalar.activation(pkT[:, j, r * RN:(r + 1) * RN],
                                     ps[:, 1, :], AF.Identity,
                                     bias=bias_cols["bpk_c"][:, j:j + 1],
                                     scale=1.0 / W8SCALE)

            g = [u_sq, u_moments, u_stats]
            g += [lambda j=j: u_ln(j) for j in range(4)]
            g += [lambda j=j: u_pp(j) for j in range(4)]
            return g

        # ================= phase 1 =================
        with ExitStack() as ctx:
            p1 = ctx.enter_context(tc.tile_pool(name="p1", bufs=1))
            kT = p1.tile([128, 4, 3, CH], BF16, tag="kT")        # ring of 3 chunks
            qpT = p1.tile([128, 4, 3, CH], BF16, tag="qpT")      # ring of 3 chunks
            vtm = p1.tile([128, 12, 8, 65], BF16, tag="vtm")     # ring of 12 tiles, 65-col/head
            nc.vector.memset(vtm[:, :, :, 64:65], 1.0)           # ones col for denominators
            # projected-q ring of 3 chunk slots with 2-col halos on each side:
            # slot layout [0:2]=left halo, [2:CH+2]=chunk body, [CH+2:CH+4]=right halo
            qraw = p1.tile([128, 4, 3, CH + 4], BF16, tag="qraw")
            nc.vector.memset(qraw[:, :, 0, 0:2], 0.0)            # chunk 0 left edge

            xtp = ctx.enter_context(tc.tile_pool(name="xtp", bufs=3))
            ps_proj = ctx.enter_context(tc.tile_pool(name="ps_proj", bufs=2, space="PSUM"))
            ps_st = ctx.enter_context(tc.tile_pool(name="ps_st", bufs=1, space="PSUM"))
            ps_out = ctx.enter_context(tc.tile_pool(name="ps_out", bufs=1, space="PSUM"))
            dsa_sb = ctx.enter_context(tc.tile_pool(name="dsa_sb", bufs=3))
            pool_tmp = ctx.enter_context(tc.tile_pool(name="pool_tmp", bufs=2))
            ax_pool = ctx.enter_context(tc.tile_pool(name="ax_sb", bufs=2))

            def load_xt_super(sc):
                g, row0 = SC2CAST[sc]
                tiles = {}
                # v: bf16, 4 feature groups of 128. q/k: fp8 transposed as
                # uint16 feature-PAIRS (2 groups of 128 pairs) -> partition p
                # of group gg holds features 2*(gg*128+p), 2*(gg*128+p)+1
                # interleaved, exactly the DoubleRow [K, 2, M] layout.
                xt = xtp.tile([128, 4, SCH], BF16, tag="xt_v",
                              name=f"xt_v_{sc}")
                for dc in range(4):
                    ti = nc.sync.dma_start(
                        out=xt[:, dc, :],
                        in_=t[f"vb{g}"][row0:row0 + SCH,
                                        dc * 128:(dc + 1) * 128],
                        transpose=True)
                    add_dep_helper(ti.ins, cast_insts[("v", g)].ins,
                                   reason="transpose reads cast output")
                tiles["v"] = xt
                for nm in ("q", "k"):
                    xt = xtp.tile([128, 2, SCH], mybir.dt.uint16,
                                  tag=f"xt_{nm}", name=f"xt_{nm}_{sc}")
                    src16 = t[f"{nm}b{g}"].ap().bitcast(mybir.dt.uint16)
                    for gg in range(2):
                        ti = nc.sync.dma_start(
                            out=xt[:, gg, :],
                            in_=src16[row0:row0 + SCH,
                                      gg * 128:(gg + 1) * 128],
                            transpose=True)
                        add_dep_helper(ti.ins, cast_insts[(nm, g)].ins,
                                       reason="transpose reads cast output")
                    tiles[nm] = xt
                return tiles

            def proj_fm_pair(xt, off, wname, c, jp):
                """Projections for j-group pair (2jp, 2jp+1) into one 2-bank
                psum. q: one paired DVE add (bias [128,2,1] broadcast) writes
                qraw at 64x scale (the 1/64 is folded into the DSA exp scale,
                host pre-scales bq by 64). k: two per-j biased ACT copies
                (ACT bias APs are per-partition scalars only)."""
                ps = ps_proj.tile([128, 2, CH], F32, tag="proj",
                                  name=f"ps_{wname}_{jp}")
                for jj in range(2):
                    j = 2 * jp + jj
                    for g in range(2):
                        rhs = (xt[:, g, off:off + CH].bitcast(FP8)
                               .rearrange("p (n s) -> p s n", s=2))
                        nc.tensor.matmul(ps[:, jj, :],
                                         W[wname][:, g, :, j * 128:(j + 1) * 128],
                                         rhs, start=(g == 0), stop=(g == 1),
                                         perf_mode=DR, skip_group_check=True)
                if wname == "wq8":
                    nc.vector.tensor_add(
                        qraw[:, 2 * jp:2 * jp + 2, c % 3, 2:2 + CH], ps[:],
                        bias_cols["bq64_c"][:, 2 * jp:2 * jp + 2]
                        .unsqueeze(2).to_broadcast((128, 2, CH)))
                else:
                    for jj in range(2):
                        j = 2 * jp + jj
                        nc.scalar.activation(kT[:, j, c % 3, :], ps[:, jj, :],
                                             AF.Identity,
                                             bias=bias_cols["bk_c"][:, j:j + 1],
                                             scale=1.0 / W8SCALE)

            def proj_v_pair(xt, off, c, tp):
                ps = ps_proj.tile([128, 2, D], F32, tag="proj", name=f"ps_v_{tp}")
                for tt_ in range(2):
                    tt = 2 * tp + tt_
                    for dk in range(4):
                        nc.tensor.matmul(ps[:, tt_, :],
                                         xt[:, dk, off + tt * 128:off + (tt + 1) * 128],
                                         W["wv"][:, dk, :], start=(dk == 0), stop=False,
                                         skip_group_check=True)
                    nc.tensor.matmul(ps[:, tt_, :], ones_row[:], bv_sb[:], start=False,
                                     stop=True, skip_group_check=True)
                s = (c * 4 + 2 * tp) % 12
                nc.scalar.copy(vtm[:, s:s + 2, :, 0:64],
                               ps[:].rearrange("p t (h d) -> p t h d", h=H))

            def halo_copies(c):
                """After chunk c's q-projections land in slot c%3, export its
                edges into the neighbouring slots' halo columns."""
                if c > 0:
                    nc.scalar.copy(qraw[:, :, (c - 1) % 3, CH + 2:CH + 4],
                                   qraw[:, :, c % 3, 2:4])
                if c + 1 < NCHUNK:
                    nc.scalar.copy(qraw[:, :, (c + 1) % 3, 0:2],
                                   qraw[:, :, c % 3, CH:CH + 2])
                else:
                    nc.vector.memset(qraw[:, :, c % 3, CH + 2:CH + 4], 0.0)

            def pool_chunk(c):
                s = c % 3
                ta = pool_tmp.tile([128, 4, CH + 2], BF16, tag="ta")
                nc.vector.tensor_add(ta[:], qraw[:, :, s, 0:CH + 2],
                                     qraw[:, :, s, 1:CH + 3])
                tb = pool_tmp.tile([128, 4, CH], BF16, tag="tb")
                nc.vector.tensor_add(tb[:], ta[:, :, 0:CH], ta[:, :, 2:CH + 2])
                nc.vector.tensor_add(qpT[:, :, c % 3, :], tb[:],
                                     qraw[:, :, s, 4:CH + 4])

            def dsa_scores(c, lt):
                """MM1 (+ rank-17 additive mask) + exp for tile lt of chunk c."""
                st = ps_st.tile([128, 8, 128], F32, tag="st", name=f"st_{c}_{lt}")
                for h in range(H):
                    hp = PERM[h]
                    base = (h % 2) * 64
                    lhsT = kT[base:base + 64, h // 2, c % 3, lt * 128:(lt + 1) * 128]
                    rhs = qpT[base:base + 64, h // 2, c % 3, lt * 128:(lt + 1) * 128]
                    nc.tensor.matmul(st[:, hp, :], lhsT, rhs, start=True, stop=True,
                                     skip_group_check=True)
                expS = dsa_sb.tile([128, 8, 128], BF16, tag="expS",
                                   name=f"expS_{c}_{lt}")
                # qpT carries a 64x scale (folded out here); alternate the
                # mask mul between DVE and GPSIMD to balance engine load
                nc.scalar.activation(expS[:], st[:], AF.Exp,
                                     scale=SCALE / QNB / W8SCALE)
                eng = nc.vector if lt % 2 == 0 else nc.gpsimd
                eng.tensor_mul(expS[:], expS[:],
                               mask_sb[:].unsqueeze(1).to_broadcast((128, 8, 128)))
                return expS

            def dsa_out(c, lt, masked, ax_out):
                """attn@V with ones-col denominators, then normalize."""
                outp = ps_out.tile([128, 2, 512], F32, tag="outp",
                                   name=f"outp_{c}_{lt}")
                for h in range(H):
                    hp = PERM[h]
                    nc.tensor.matmul(outp[:, h // 4, (h % 4) * 65:(h % 4) * 65 + 65],
                                     masked[:, hp, :],
                                     vtm[:, (c * 4 + lt) % 12, h, :],
                                     start=True, stop=True, skip_group_check=True)
                recip = dsa_sb.tile([128, 2, 4], F32, tag="recip",
                                    name=f"recip_{c}_{lt}")
                den_view = bass.AP(outp.tensor, outp[:].offset + 64,
                                   [outp[:].ap[0], [512, 2], [65, 4]])
                nc.vector.reciprocal(recip[:], den_view)
                # V' already contains +bv (rank-1 matmul in proj_v); attention
                # weights sum to 1 after the 1/den scale, so bias is exact.
                av_view = bass.AP(outp.tensor, outp[:].offset,
                                  [outp[:].ap[0], [512, 2], [65, 4], [1, 64]])
                nc.vector.tensor_mul(
                    ax_out.rearrange("p (a b d) -> p a b d", a=2, b=4),
                    av_view,
                    recip[:].unsqueeze(3).to_broadcast((128, 2, 4, 64)))

            def dsa_group_list(c, ax):
                masked = {}
                g = []
                g.append(lambda: masked.__setitem__(0, dsa_scores(c, 0)))
                g.append(lambda: masked.__setitem__(1, dsa_scores(c, 1)))
                g.append(lambda: dsa_out(c, 0, masked.pop(0), ax[:, 0, :]))
                g.append(lambda: masked.__setitem__(2, dsa_scores(c, 2)))
                g.append(lambda: dsa_out(c, 1, masked.pop(1), ax[:, 1, :]))
                g.append(lambda: masked.__setitem__(3, dsa_scores(c, 3)))
                g.append(lambda: dsa_out(c, 2, masked.pop(2), ax[:, 2, :]))
                g.append(lambda: dsa_out(c, 3, masked.pop(3), ax[:, 3, :]))
                return g

            def store_ax_pair(cp, ax2):
                """Store DSA output for chunks (cp, cp+1), then transpose the
                pair back feature-major (overlaps phase 1). The last pair is
                stored per chunk so the final win_tok transposes - which gate
                the LN chain at the phase transition - wait on a half-size
                store."""
                wsrc = axd.ap().rearrange("(w s) d -> w s d", s=PW)
                dst = axd.ap().rearrange("(g lt p) d -> g p lt d", lt=8, p=128)
                wi = nc.gpsimd.dma_start(out=dst[cp // 2], in_=ax2[:])
                wis = [wi]
                for dc in range(4):
                    wt = nc.sync.dma_start(
                        out=wt_view[:, dc, cp * 64:(cp + 2) * 64],
                        in_=wsrc[cp * 64:(cp + 2) * 64, 0,
                                 dc * 128:(dc + 1) * 128],
                        transpose=True)
                    add_dep_helper(wt.ins, wi.ins,
                                   reason="win_tok transpose reads axd pair")
                axd_writers.append(wis)
                tis = []
                for dc in range(4):
                    ti = nc.sync.dma_start(
                        out=axt[:, dc, cp * CH:(cp + 2) * CH],
                        in_=axd[cp * CH:(cp + 2) * CH, dc * 128:(dc + 1) * 128],
                        transpose=True)
                    for wi in wis:
                        add_dep_helper(ti.ins, wi.ins,
                                       reason="axt transpose reads axd pair")
                    tis.append(ti)
                axt_trans.append(tis)

            xt_tiles = {0: load_xt_super(0)}
            extra = []      # deferred phase-2 prologue work units
            ax2 = None

            for c in range(NCHUNK + 2):
                if c == 5:
                    load_phase2_weights()
                if c == 7:
                    extra.extend(ln_half_groups(0, ps_proj, "proj"))
                pgroups = []
                if c < NCHUNK:
                    sc = c // 2
                    off = (c % 2) * CH
                    if c % 2 == 0 and sc + 1 < NSC:
                        xt_tiles[sc + 1] = load_xt_super(sc + 1)
                    qxt = xt_tiles[sc]["q"]
                    kxt = xt_tiles[sc]["k"]
                    vxt = xt_tiles[sc]["v"]
                    for jp in range(2):
                        pgroups.append(lambda jp=jp, x=qxt, o=off, c=c:
                                       proj_fm_pair(x, o, "wq8", c, jp))
                    for tp in range(2):
                        pgroups.append(lambda tp=tp, x=vxt, o=off, c=c:
                                       proj_v_pair(x, o, c, tp))
                    for jp in range(2):
                        pgroups.append(lambda jp=jp, x=kxt, o=off, c=c:
                                       proj_fm_pair(x, o, "wk8", c, jp))
                dgroups = []
                dc_ = c - 2
                if dc_ >= 0:
                    if dc_ % 2 == 0:
                        ax2 = ax_pool.tile([128, 8, D], BF16, tag="ax",
                                           name=f"ax_{dc_}")
                    axv = ax2[:, (dc_ % 2) * 4:(dc_ % 2) * 4 + 4, :]
                    dgroups = dsa_group_list(dc_, axv)
                # weave: spread D groups evenly through the P stream;
                # pool(c-1) after the 2 Q-projection pairs; extra units fill
                # remaining slots late in phase 1
                npg, ndg = len(pgroups), len(dgroups)
                if npg:
                    # per-pgroup D-group quota (6 pgroups hosting up to 8)
                    quota = (1, 2, 1, 2, 1, 1)
                    di = 0
                    for i in range(npg):
                        pgroups[i]()
                        if i == 2:
                            halo_copies(c)
                            if c >= 1:
                                pool_chunk(c - 1)
                        for _ in range(quota[i]):
                            if di < ndg:
                                dgroups[di]()
                                di += 1
                                if extra:
                                    extra.pop(0)()
                    while di < ndg:
                        dgroups[di]()
                        di += 1
                else:
                    if 1 <= c <= NCHUNK:
                        pool_chunk(c - 1)
                    for g in dgroups:
                        g()
                        if extra:
                            extra.pop(0)()
                if dc_ >= 0 and dc_ % 2 == 1:
                    store_ax_pair(dc_ - 1, ax2)
            while extra:
                extra.pop(0)()

        # ================= phase 2 =================
        with ExitStack() as ctx:
            p2 = ctx.enter_context(tc.tile_pool(name="p2", bufs=1))
            ps2 = ctx.enter_context(tc.tile_pool(name="ps2", bufs=3, space="PSUM"))
            ps2b = ctx.enter_context(tc.tile_pool(name="ps2b", bufs=2, space="PSUM"))
            sb2 = ctx.enter_context(tc.tile_pool(name="sb2", bufs=2))

            bo_sb = singles.tile([1, D], BF16)
            nc.scalar.dma_start(out=bo_sb[:], in_=t["bo_r"][:, :])

            # pv gathered window-major with a SWDGE cast to fp8 for the
            # DoubleRow pout matmuls.
            pv = p2.tile([128, 4, WIN, D], FP8, tag="pv")
            srcv = axd.ap().rearrange("(cc p w) d -> cc p w d", p=128, w=PW)
            for cc in range(4):
                gi = nc.gpsimd.dma_start(out=pv[:, cc, :, :], in_=srcv[cc, :, 1:PW, :])
                for wi in axd_writers[cc]:
                    add_dep_helper(gi.ins, wi.ins, reason="pv gather")
                # ordering shim: run the gathers after the last attn_x pair's
                # window-summary transposes so the transition chain
                # (store -> wt_view -> LN -> pq/pk -> PSA) isn't queued
                # behind them; pout doesn't need pv until well after.
                for ti in axt_trans[-1]:
                    add_dep_helper(gi.ins, ti.ins, reason="dma order")


            # ---- LN + GELU + pq/pk for the second window half ----
            for u in ln_half_groups(1, ps2, "ps2"):
                u()

            # ---- PSA softmax per head; pout per head-pair right after ----
            # Scores run fp8 DoubleRow over [32, 2] feature tiles; exp output
            # stays UNNORMALIZED in fp8 (values ~1.0, ideal e4m3 range). pout
            # contracts raw exp against fp8 pv, and the 1/den normalization is
            # applied afterwards on the psum via a per-pair recip tile whose
            # partition rows are already head-matched (h0 on 0:64, h1 on
            # 64:128) thanks to the DoubleRow den matmul's 64-row output.
            zt = p2.tile([128, 4, QLEN], BF16, tag="zt")
            ones8_2 = p2.tile([128, 2, 128], FP8, tag="ones8")
            nc.vector.memset(ones8_2[:], 1.0)

            def psa_scores(h):
                b32 = 32 * (h % 4)
                a = 2 * (h // 4)
                es = sb2.tile([128, 4, WN], FP8, tag="psa_exp", bufs=8,
                              name=f"es_{h}")
                for cp in range(2):
                    ps = ps2b.tile([128, 2, WN], F32, tag="pair",
                                   name=f"st_{h}_{cp}")
                    for ch in range(2):
                        cc = cp * 2 + ch
                        nc.tensor.matmul(
                            ps[:, ch, :],
                            pkT[b32:b32 + 32, a:a + 2, cc * 128:(cc + 1) * 128],
                            pqT[b32:b32 + 32, a:a + 2, :], start=True, stop=True,
                            perf_mode=DR, skip_group_check=True,
                            tile_position=(b32, 0))
                    nc.scalar.activation(es[:, 2 * cp:2 * cp + 2, :], ps[:],
                                         AF.Exp, scale=SCALE)
                return es

            def psa_norm(j, es0, es1):
                """den + recip for head pair j (heads 2j, 2j+1). DoubleRow
                outputs must sit at column position 0, so each head gets a
                full 128-partition replicated den psum; the recips then read
                partition-aligned halves into one pair tile (h0 rows on 0:64,
                h1 on 64:128) for the single pout normalization mul."""
                recipd = sb2.tile([128, WN], F32, tag="psa_recip", bufs=2,
                                  name=f"r_{j}")
                for half, es in ((0, es0), (1, es1)):
                    ps_den = ps2b.tile([128, WN], F32, tag="psa_den", bufs=1,
                                       name=f"d_{j}_{half}")
                    for cp in range(2):
                        nc.tensor.matmul(
                            ps_den[:], ones8_2[:], es[:, 2 * cp:2 * cp + 2, :],
                            start=(cp == 0), stop=(cp == 1),
                            perf_mode=DR, skip_group_check=True)
                    nc.vector.reciprocal(recipd[half * 64:(half + 1) * 64, :],
                                         ps_den[half * 64:(half + 1) * 64, :])
                return recipd

            def pout_pair(j, wh, es0, es1, recipd):
                """pout for head-pair j over query-window half wh."""
                w0 = wh * (WN // 2)
                for i in range(WIN):
                    po = ps2.tile([128, WN // 2], F32, tag="ps2",
                                  name=f"po_{j}_{i}_{wh}")
                    # DoubleRow requires output column position 0, so only
                    # the half-0 head runs DR; half-1 (psum base 64) uses
                    # plain fp8 matmuls.
                    h0 = 2 * j
                    for cp in range(2):
                        nc.tensor.matmul(
                            po[0:64, :],
                            pv[:, 2 * cp:2 * cp + 2, i, h0 * 64:(h0 + 1) * 64],
                            es0[:, 2 * cp:2 * cp + 2, w0:w0 + WN // 2],
                            start=(cp == 0), stop=(cp == 1),
                            perf_mode=DR, skip_group_check=True)
                    h1 = 2 * j + 1
                    for cc in range(4):
                        nc.tensor.matmul(
                            po[64:128, :],
                            pv[:, cc, i, h1 * 64:(h1 + 1) * 64],
                            es1[:, cc, w0:w0 + WN // 2],
                            start=(cc == 0), stop=(cc == 3),
                            skip_group_check=True)
                    pn = sb2.tile([128, WN // 2], BF16, tag="pn", bufs=4,
                                  name=f"pn_{j}_{i}_{wh}")
                    nc.vector.tensor_mul(pn[:], po[:], recipd[:, w0:w0 + WN // 2])
                    # SBUF-only bf16 add: run it on GPSIMD to keep DVE free
                    # for the psum-reading normalization muls
                    nc.gpsimd.tensor_add(
                        zt[:, j, :].rearrange("p (w i) -> p w i", i=WIN)
                        [:, w0:w0 + WN // 2, i],
                        pn[:],
                        axt[:, j, :].rearrange("p (w s) -> p w s", s=PW)
                        [:, w0:w0 + WN // 2, 1 + i])

            outv = out.ap().rearrange("(g tt p) d -> g p tt d", tt=2, p=128)

            def final_group(g, split_store=False):
                o_sb = sb2.tile([128, 2, D], BF16, tag="osb", bufs=4,
                                name=f"osb_{g}")
                for q in range(2):
                    tt = g * 2 + q
                    ps = ps2.tile([128, D], F32, tag="ps2", name=f"fin_{tt}")
                    for dk in range(4):
                        nc.tensor.matmul(ps[:], zt[:, dk, tt * 128:(tt + 1) * 128],
                                         W["wo"][:, dk, :], start=(dk == 0),
                                         stop=False, skip_group_check=True)
                    # bo via rank-1 matmul; psum evacuation on ACT (idle in
                    # the fin tail) instead of a DVE add
                    nc.tensor.matmul(ps[:], ones_row[:], bo_sb[:], start=False,
                                     stop=True, skip_group_check=True)
                    nc.scalar.copy(o_sb[:, q, :], ps[:])
                    if split_store:
                        nc.sync.dma_start(out=outv[g][:, q, :],
                                          in_=o_sb[:, q, :])
                if not split_store:
                    nc.sync.dma_start(out=outv[g], in_=o_sb[:])

            # pipeline: scores(h+1) | norm(j) once its pair of heads is
            # scored | pout(j) right after; the last pair is split by
            # query-window half so the first finals overlap its second half
            es_store = {0: psa_scores(0)}
            recs = {}
            done_pairs = 0
            for h in range(1, H):
                es_store[h] = psa_scores(h)
                if h % 2 == 1:
                    j = h // 2
                    recs[j] = psa_norm(j, es_store[2 * j], es_store[2 * j + 1])
                if h % 2 == 0 and done_pairs in recs:
                    j = done_pairs
                    pout_pair(j, 0, es_store[2 * j], es_store[2 * j + 1], recs[j])
                    pout_pair(j, 1, es_store[2 * j], es_store[2 * j + 1], recs[j])
                    es_store.pop(2 * j), es_store.pop(2 * j + 1), recs.pop(j)
                    done_pairs += 1
            while done_pairs < 4:
                j = done_pairs
                pout_pair(j, 0, es_store[2 * j], es_store[2 * j + 1], recs[j])
                if j == 3:
                    for g in range(7):
                        final_group(g)
                pout_pair(j, 1, es_store[2 * j], es_store[2 * j + 1], recs[j])
                done_pairs += 1
            for g in range(7, QLEN // 256):
                final_group(g, split_store=(g >= QLEN // 256 - 2))


_NC_CACHE = None


def _get_program():
    global _NC_CACHE
    if _NC_CACHE is None:
        _NC_CACHE = build_program()
    return _NC_CACHE


def _fp8_paired(Wm, perm=None, pair="dc"):
    """Host prep for fp8 DoubleRow lhsT: optional column permutation, x64
    scale, then row pairing. pair="dc": rows (2g+s)*128+p -> [g*128+p, s]
    (matches the wtn dc-group layout); pair="consec": rows 2f+s -> [f, s]
    (matches the uint16-pair input transposes)."""
    w = np.asarray(Wm, np.float32)
    if perm is not None:
        w = w[:, perm]
    w = (w * W8SCALE).astype(ml_dtypes.float8_e4m3fn)
    if pair == "consec":
        return np.ascontiguousarray(w.reshape(256, 2, D))
    # rows: r = g*256 + s*128 + p  ->  out[g*128+p, s, :]
    return np.ascontiguousarray(
        w.reshape(2, 2, 128, D).transpose(0, 2, 1, 3).reshape(256, 2, D))


def _host_consts(Wk, bk, Wv, bv, Wq, bq, ln_g, ln_b, Wpq, bpq, Wpk, bpk, Wo, bo):
    bf = ml_dtypes.bfloat16
    col = lambda b: np.asarray(b, np.float32).reshape(4, 128).T.copy()
    perm = np.asarray(PSA_PERM)
    consts = {
        "wq8": _fp8_paired(Wq, pair="consec"),
        "wk8": _fp8_paired(Wk, pair="consec"),
        "wv": np.asarray(Wv, np.float32).astype(bf),
        "wpq8": _fp8_paired(Wpq, perm),
        "wpk8": _fp8_paired(Wpk, perm),
        "wo": np.asarray(Wo, np.float32).astype(bf),
        "bq64_c": col(np.asarray(bq, np.float32) * W8SCALE), "bk_c": col(bk),
        "bpq_c": col(np.asarray(bpq, np.float32)[perm]),
        "bpk_c": col(np.asarray(bpk, np.float32)[perm]),
        "ln_g_c": col(ln_g), "ln_b_c": col(ln_b),
        "bv_r": np.asarray(bv, np.float32).reshape(1, D).astype(bf),
        "bo_r": np.asarray(bo, np.float32).reshape(1, D).astype(bf),
        "bv_f": np.tile(np.asarray(bv, np.float32).reshape(1, D), (128, 1)).astype(bf),
        "bo_f": np.tile(np.asarray(bo, np.float32).reshape(1, D), (128, 1)).astype(bf),
    }
    m = np.zeros((128, 128), np.float32)
    for g in range(16):
        m[g * PW:(g + 1) * PW, g * PW:(g + 1) * PW] = 1.0
    consts["bmask"] = m.astype(bf)
    return consts


def kernel(k, v, q, query_len, Wk, bk, Wv, bv, Wq, bq, ln_g, ln_b,
           Wpq, bpq, Wpk, bpk, Wo, bo):
    nc = _get_program()
    consts = _host_consts(Wk, bk, Wv, bv, Wq, bq, ln_g, ln_b,
                          Wpq, bpq, Wpk, bpk, Wo, bo)
    k = np.asarray(k, np.float32)
    v = np.asarray(v, np.float32)
    q = np.asarray(q, np.float32)
    in_maps = []
    for b in range(B):
        m = {"q": np.ascontiguousarray(q[b]), "k": np.ascontiguousarray(k[b]),
             "v": np.ascontiguousarray(v[b])}
        m.update(consts)
        in_maps.append(m)
    res = run_bass_kernel_spmd(nc, in_maps, core_ids=list(range(B)))
    return np.stack([np.asarray(res.results[b]["out"], np.float32)
                     for b in range(B)], axis=0)


if __name__ == "__main__":
    nc = build_program()
    print("program built ok")



# revision 90
# speedup vs baseline: 1.0127x; 1.0004x over previous
"""Trainium2 Bass kernel for DeformableMultiHeadedAttention.

Data-parallel over batch B=8 across 8 NeuronCores (one batch element per
core, identical programs, no collectives). Heavy matmuls run fp8-e4m3
DoubleRow (0.5 PE cycles/row) wherever a numpy precision study showed the
final rel-err stays ~0.006 (tolerance 2e-2); V projection and the final
Z@Wo stay bf16 (fp8 there blows the budget).

Per-core pipeline (f32 psum accumulate everywhere):
  1. q,k f32 -> SWDGE cast-DMA -> DRAM fp8; v -> bf16. q/k DMA-transpose as
     uint16 feature-PAIRS, which lands directly in DoubleRow's [K,2,M]
     operand layout; v transposes bf16 feature-major.
  2. Projections on PE: Q'/K' via fp8 DoubleRow (weights host-scaled x64,
     paired rows [2f, 2f+1]); 1/64 descales in the psum-evacuation copies
     (k via per-j ACT Identity+bias) or folds into the DSA exp scale (qraw
     keeps 64x, one paired DVE add with a [128,2,1]-broadcast bias). V'
     token-major bf16 with bias via a K=1 rank-1 matmul.
  3. Q pooling (AvgPool k=5, zero pad) as 3 shifted DVE adds; 1/5 folded
     into the exp scale.
  4. DSA (windows of 8): per 128-token tile: bank-segregated 64-row score
     matmuls, exp on ACT, block-diag mask mul alternating DVE/GPSIMD,
     attn@V with a ones-column denominator, 1/den scale on DVE. Output
     token-major -> DRAM (axd, bf16).
  5. Re-layouts from axd: wt_view (window-summary transposes), axt
     (feature-major attn_x), pv (window-major payload, SWDGE cast to fp8).
  6. win_tok LayerNorm + exact GELU -> wtn fp8; pq/pk projections fp8
     DoubleRow with host column-permuted weights so each head sits on 32
     partitions x 2 dc-slots; PSA scores fp8 DoubleRow per head.
  7. PSA softmax normalization deferred: exp stays UNNORMALIZED in fp8
     (values ~1.0, ideal e4m3 range); pout contracts raw exp against fp8 pv
     (DoubleRow half-0 head / plain fp8 half-1 head - DoubleRow outputs must
     sit at psum column position 0); per-pair replicated den (DoubleRow
     ones matmuls) -> recip -> one DVE mul on the pout psum; Z = pn + attn_x
     via GPSIMD adds (SBUF-only bf16).
  8. Final out = Z @ Wo bf16 with bo via rank-1 matmul, ACT psum copies,
     bf16 DRAM stores (host upcasts to f32).
"""

import sys
from contextlib import ExitStack

for _p in ("/opt/trn_rl_repo/concourse", "/opt/trn_rl_repo"):
    if _p not in sys.path:
        sys.path.insert(0, _p)

import numpy as np
import ml_dtypes

import concourse.bass as bass
import concourse.mybir as mybir
import concourse.tile as tile
from concourse import bacc
from concourse.tile import add_dep_helper
from concourse.bass_utils import run_bass_kernel_spmd

BF16 = mybir.dt.bfloat16
F32 = mybir.dt.float32
FP8 = mybir.dt.float8e4
DR = mybir.MatmulPerfMode.DoubleRow
AF = mybir.ActivationFunctionType
ALU = mybir.AluOpType

B, M, D = 8, 4096, 512
H, HD = 8, 64
WIN = 7
PW = WIN + 1
QNB = 5
QLEN = 3584
WN = M // PW
SCALE = D ** -0.5
EPS = 1e-5
NCHUNK = 8
CH = 512
SCH = 1024           # super-chunk (transpose batch) size
NSC = M // SCH
CAST_RANGES = [(0, 1024), (1024, 2048), (2048, 4096)]
SC2CAST = {0: (0, 0), 1: (1, 0), 2: (2, 0), 3: (2, 1024)}  # sc -> (group, row0)
PERM = [(h % 2) * 4 + h // 2 for h in range(H)]  # head -> DSA psum slot
W8SCALE = 64.0  # host pre-scale on fp8 weights; 1/64 folded into psum copies

# PSA pq/pk column permutation: dc-group j, partition p -> original dout.
# Head h occupies 32 partitions at 32*(h%4) across the dc pair 2*(h//4),
# with features 0:32 in the even dc and 32:64 in the odd dc, so psa score
# matmuls can run fp8 DoubleRow over [32, 2] feature tiles.
PSA_PERM = [((j // 2) * 4 + p // 32) * 64 + (j % 2) * 32 + (p % 32)
            for j in range(4) for p in range(128)]


def build_program():
    nc = bacc.Bacc("TRN2", target_bir_lowering=False, debug=False, num_devices=8)

    t = {}
    t["q_in"] = nc.dram_tensor("q", [M, D], F32, kind="ExternalInput")
    t["k_in"] = nc.dram_tensor("k", [M, D], F32, kind="ExternalInput")
    t["v_in"] = nc.dram_tensor("v", [M, D], F32, kind="ExternalInput")
    for nm in ("wv", "wo"):
        t[nm] = nc.dram_tensor(nm, [D, D], BF16, kind="ExternalInput")
    for nm in ("wq8", "wk8", "wpq8", "wpk8"):
        t[nm] = nc.dram_tensor(nm, [256, 2, D], FP8, kind="ExternalInput")
    for nm in ("bq64_c", "bk_c", "bpq_c", "bpk_c", "ln_g_c", "ln_b_c"):
        t[nm] = nc.dram_tensor(nm, [128, 4], F32, kind="ExternalInput")
    t["bv_r"] = nc.dram_tensor("bv_r", [1, D], BF16, kind="ExternalInput")
    t["bo_r"] = nc.dram_tensor("bo_r", [1, D], BF16, kind="ExternalInput")
    t["bv_f"] = nc.dram_tensor("bv_f", [128, D], BF16, kind="ExternalInput")
    t["bo_f"] = nc.dram_tensor("bo_f", [128, D], BF16, kind="ExternalInput")
    t["bmask"] = nc.dram_tensor("bmask", [128, 128], BF16, kind="ExternalInput")
    t["out"] = nc.dram_tensor("out", [QLEN, D], BF16, kind="ExternalOutput")
    t["axd"] = nc.dram_tensor("axd_s", [M, D], BF16, kind="Internal")
    t["zd"] = nc.dram_tensor("zd_s", [QLEN, D], BF16, kind="Internal")
    # per-group cast targets: separate tensors so the tile framework's
    # tensor-granular dependency tracking doesn't serialize casts behind
    # earlier chunks' transpose reads (false WAR). First two groups are
    # small so compute can start early.
    # q/k cast straight to fp8 (transposed later as uint16 feature-pairs,
    # which lands in exactly the DoubleRow [K,2,M] operand layout); v stays
    # bf16 for precision.
    for nm in ("q", "k", "v"):
        dt_ = BF16 if nm == "v" else FP8
        for g, (lo, hi) in enumerate(CAST_RANGES):
            t[f"{nm}b{g}"] = nc.dram_tensor(f"{nm}b{g}_s", [hi - lo, D], dt_,
                                            kind="Internal")

    with tile.TileContext(nc) as tc:
        _build(nc, tc, t)
    nc.compile()
    return nc


def _build(nc, tc, t):
    axd, out = t["axd"], t["out"]

    with ExitStack() as octx:
        singles = octx.enter_context(tc.tile_pool(name="singles", bufs=1))

        # f32->bf16 cast DMAs first; few large batches keep the SWDGE
        # completion-semaphore lanes from being recycled between casts.
        cast_insts = {}
        srcs_d = {"q": t["q_in"], "k": t["k_in"], "v": t["v_in"]}
        for g, (lo, hi) in enumerate(CAST_RANGES):
            for nm in ("q", "v", "k"):
                ci = nc.gpsimd.dma_start(
                    out=t[f"{nm}b{g}"][:, :],
                    in_=srcs_d[nm][lo:hi, :])
                cast_insts[(nm, g)] = ci

        W = {}
        # wq loads immediately (first projection needs it); the other loads
        # are gated behind the first v cast so the q/v transposes win the
        # startup DMA race.
        gate0 = cast_insts[("v", 0)]
        for nm in ("wq8", "wk8"):
            W[nm] = singles.tile([128, 2, 2, D], FP8, tag=nm, name=f"w_{nm}")
            wi = nc.scalar.dma_start(out=W[nm][:],
                                     in_=t[nm].ap().rearrange(
                                         "(g p) s d -> p g s d", p=128))
            if nm != "wq8":
                add_dep_helper(wi.ins, gate0.ins, reason="dma order")
        W["wv"] = singles.tile([128, 4, D], BF16, tag="wv", name="w_wv")
        wi = nc.scalar.dma_start(out=W["wv"][:],
                                 in_=t["wv"].ap().rearrange("(c p) d -> p c d",
                                                            p=128))
        add_dep_helper(wi.ins, gate0.ins, reason="dma order")
        bias_cols = {}
        for nm in ("bq64_c", "bk_c"):
            bias_cols[nm] = singles.tile([128, 4], F32, tag=nm, name=f"bc_{nm}")
            nc.scalar.dma_start(out=bias_cols[nm][:], in_=t[nm][:, :])
        bv_sb = singles.tile([1, D], BF16)
        nc.scalar.dma_start(out=bv_sb[:], in_=t["bv_r"][:, :])
        mask_sb = singles.tile([128, 128], BF16)
        wi = nc.scalar.dma_start(out=mask_sb[:], in_=t["bmask"][:, :])
        add_dep_helper(wi.ins, cast_insts[("k", 0)].ins, reason="dma order")
        ones_row = singles.tile([1, 128], BF16)
        nc.vector.memset(ones_row[:], 1.0)
        ones_col = singles.tile([128, 1], BF16)
        nc.vector.memset(ones_col[:], 1.0)
        ones_full = singles.tile([128, 128], BF16)
        nc.vector.memset(ones_full[:], 1.0)
        eps_sb = singles.tile([128, 1], F32)
        nc.vector.memset(eps_sb[:], EPS)

        axd_writers = []
        axt_trans = []
        p2a = octx.enter_context(tc.tile_pool(name="p2a", bufs=1))
        axt = p2a.tile([128, 4, M], BF16, tag="axt")

        def load_phase2_weights():
            # ordering shim: keep these dep-free loads from being hoisted by
            # the scheduler ahead of the startup-critical input transposes.
            # Gated on the first attn_x pair store (~1/3 through phase 1),
            # which fires well before Act SEQ reaches these instructions, so
            # no head-of-line blocking on the Act sequencer.
            W["wo"] = singles.tile([128, 4, D], BF16, tag="wo", name="w_wo")
            nc.scalar.dma_start(out=W["wo"][:],
                                in_=t["wo"].ap().rearrange(
                                    "(c p) d -> p c d", p=128))
            for nm in ("wpq8", "wpk8"):
                W[nm] = singles.tile([128, 2, 2, D], FP8, tag=nm, name=f"w_{nm}")
                nc.scalar.dma_start(out=W[nm][:],
                                    in_=t[nm].ap().rearrange(
                                        "(g p) s d -> p g s d", p=128))
            for nm in ("bpq_c", "bpk_c", "ln_g_c", "ln_b_c"):
                bias_cols[nm] = singles.tile([128, 4], F32, tag=nm, name=f"bc_{nm}")
                nc.scalar.dma_start(out=bias_cols[nm][:], in_=t[nm][:, :])

        # ---- win_tok LN + GELU + pq/pk projections, by window quarters.
        # Quarters 0-2 run inside phase 1 as their attn_x pairs land; only
        # quarter 3 remains for the phase transition.
        lnp = octx.enter_context(tc.tile_pool(name="lnp", bufs=1))
        wtn = lnp.tile([128, 4, WN], FP8, tag="wtn")
        pqT = lnp.tile([128, 4, WN], FP8, tag="pqT")
        pkT = lnp.tile([128, 4, WN], FP8, tag="pkT")
        # dedicated feature-major copy of the window-summary tokens: cheap
        # strided-row transposes that unblock LN without the full axt pair
        wt_view = lnp.tile([128, 4, WN], BF16, tag="wtT")
        RN = WN // 2

        def ln_half_groups(r, psum_pool, ps_tag):
            st = {}

            def u_sq():
                wsq = lnp.tile([128, 4, RN], BF16, tag="wsq", name=f"wsq_{r}")
                src = wt_view[:, :, r * RN:(r + 1) * RN]
                if r == 1:
                    # transition half: DVE is idle here and this avoids an
                    # ACT Square-table reload on the critical chain
                    nc.vector.tensor_mul(wsq[:], src, src)
                else:
                    nc.scalar.activation(wsq[:], src, AF.Square)
                st["wsq"] = wsq

            def u_moments():
                ps = psum_pool.tile([128, 2, RN], F32, tag=ps_tag,
                                    name=f"ln_ps_{r}")
                for j in range(4):
                    nc.tensor.matmul(ps[:, 0, :], ones_full[:],
                                     wt_view[:, j, r * RN:(r + 1) * RN],
                                     start=(j == 0), stop=(j == 3),
                                     skip_group_check=True)
                    nc.tensor.matmul(ps[:, 1, :], ones_full[:], st["wsq"][:, j, :],
                                     start=(j == 0), stop=(j == 3),
                                     skip_group_check=True)
                mu = lnp.tile([128, RN], F32, tag="mu_sb", name=f"mu_{r}")
                nc.scalar.mul(mu[:], ps[:, 0, :], 1.0 / D)
                ex2 = lnp.tile([128, RN], F32, tag="ex2_sb", name=f"ex2_{r}")
                nc.scalar.mul(ex2[:], ps[:, 1, :], 1.0 / D)
                st["mu"], st["ex2"] = mu, ex2

            def u_stats():
                mu, ex2 = st["mu"], st["ex2"]
                var = lnp.tile([128, RN], F32, tag="var_sb", name=f"var_{r}")
                nc.vector.tensor_mul(var[:], mu[:], mu[:])
                nc.vector.tensor_sub(var[:], ex2[:], var[:])
                sd = lnp.tile([128, RN], F32, tag="sd", name=f"sd_{r}")
                nc.scalar.activation(sd[:], var[:], AF.Sqrt, bias=eps_sb[:])
                rstd = lnp.tile([128, RN], F32, tag="rstd", name=f"rstd_{r}")
                nc.vector.reciprocal(rstd[:], sd[:])
                st["rstd"] = rstd

            def u_ln(j):
                tmp = lnp.tile([128, RN], F32, tag="lntmp", bufs=2,
                               name=f"lnt_{r}_{j}")
                nc.vector.tensor_sub(tmp[:], wt_view[:, j, r * RN:(r + 1) * RN],
                                     st["mu"][:])
                nc.vector.tensor_mul(tmp[:], tmp[:], st["rstd"][:])
                nc.scalar.activation(wtn[:, j, r * RN:(r + 1) * RN], tmp[:],
                                     AF.Gelu,
                                     bias=bias_cols["ln_b_c"][:, j:j + 1],
                                     scale=bias_cols["ln_g_c"][:, j:j + 1])

            def u_pp(j):
                ps = psum_pool.tile([128, 2, RN], F32, tag=ps_tag,
                                    name=f"pp_{r}_{j}")
                for g in range(2):
                    nc.tensor.matmul(ps[:, 0, :],
                                     W["wpq8"][:, g, :, j * 128:(j + 1) * 128],
                                     wtn[:, 2 * g:2 * g + 2, r * RN:(r + 1) * RN],
                                     start=(g == 0), stop=(g == 1),
                                     perf_mode=DR, skip_group_check=True)
                    nc.tensor.matmul(ps[:, 1, :],
                                     W["wpk8"][:, g, :, j * 128:(j + 1) * 128],
                                     wtn[:, 2 * g:2 * g + 2, r * RN:(r + 1) * RN],
                                     start=(g == 0), stop=(g == 1),
                                     perf_mode=DR, skip_group_check=True)
                nc.scalar.activation(pqT[:, j, r * RN:(r + 1) * RN],
                                     ps[:, 0, :], AF.Identity,
                                     bias=bias_cols["bpq_c"][:, j:j + 1],
                                     scale=1.0 / W8SCALE)
                nc.scalar.activation(pkT[:, j, r * RN:(r + 1) * RN],
                                     ps[:, 1, :], AF.Identity,
                                     bias=bias_cols["bpk_c"][:, j:j + 1],
                                     scale=1.0 / W8SCALE)

            g = [u_sq, u_moments, u_stats]
            g += [lambda j=j: u_ln(j) for j in range(4)]
            g += [lambda j=j: u_pp(j) for j in range(4)]
            return g

        # ================= phase 1 =================
        with ExitStack() as ctx:
            p1 = ctx.enter_context(tc.tile_pool(name="p1", bufs=1))
            kT = p1.tile([128, 4, 3, CH], BF16, tag="kT")        # ring of 3 chunks
            qpT = p1.tile([128, 4, 3, CH], BF16, tag="qpT")      # ring of 3 chunks
            vtm = p1.tile([128, 12, 8, 65], BF16, tag="vtm")     # ring of 12 tiles, 65-col/head
            nc.vector.memset(vtm[:, :, :, 64:65], 1.0)           # ones col for denominators
            # projected-q ring of 3 chunk slots with 2-col halos on each side:
            # slot layout [0:2]=left halo, [2:CH+2]=chunk body, [CH+2:CH+4]=right halo
            qraw = p1.tile([128, 4, 3, CH + 4], BF16, tag="qraw")
            nc.vector.memset(qraw[:, :, 0, 0:2], 0.0)            # chunk 0 left edge

            xtp = ctx.enter_context(tc.tile_pool(name="xtp", bufs=3))
            ps_proj = ctx.enter_context(tc.tile_pool(name="ps_proj", bufs=2, space="PSUM"))
            ps_st = ctx.enter_context(tc.tile_pool(name="ps_st", bufs=1, space="PSUM"))
            ps_out = ctx.enter_context(tc.tile_pool(name="ps_out", bufs=1, space="PSUM"))
            dsa_sb = ctx.enter_context(tc.tile_pool(name="dsa_sb", bufs=3))
            pool_tmp = ctx.enter_context(tc.tile_pool(name="pool_tmp", bufs=2))
            ax_pool = ctx.enter_context(tc.tile_pool(name="ax_sb", bufs=2))

            def load_xt_super(sc):
                g, row0 = SC2CAST[sc]
                tiles = {}
                # v: bf16, 4 feature groups of 128. q/k: fp8 transposed as
                # uint16 feature-PAIRS (2 groups of 128 pairs) -> partition p
                # of group gg holds features 2*(gg*128+p), 2*(gg*128+p)+1
                # interleaved, exactly the DoubleRow [K, 2, M] layout.
                xt = xtp.tile([128, 4, SCH], BF16, tag="xt_v",
                              name=f"xt_v_{sc}")
                for dc in range(4):
                    ti = nc.sync.dma_start(
                        out=xt[:, dc, :],
                        in_=t[f"vb{g}"][row0:row0 + SCH,
                                        dc * 128:(dc + 1) * 128],
                        transpose=True)
                    add_dep_helper(ti.ins, cast_insts[("v", g)].ins,
                                   reason="transpose reads cast output")
                tiles["v"] = xt
                for nm in ("q", "k"):
                    xt = xtp.tile([128, 2, SCH], mybir.dt.uint16,
                                  tag=f"xt_{nm}", name=f"xt_{nm}_{sc}")
                    src16 = t[f"{nm}b{g}"].ap().bitcast(mybir.dt.uint16)
                    for gg in range(2):
                        ti = nc.sync.dma_start(
                            out=xt[:, gg, :],
                            in_=src16[row0:row0 + SCH,
                                      gg * 128:(gg + 1) * 128],
                            transpose=True)
                        add_dep_helper(ti.ins, cast_insts[(nm, g)].ins,
                                       reason="transpose reads cast output")
                    tiles[nm] = xt
                return tiles

            def proj_fm_pair(xt, off, wname, c, jp):
                """Projections for j-group pair (2jp, 2jp+1) into one 2-bank
                psum. q: one paired DVE add (bias [128,2,1] broadcast) writes
                qraw at 64x scale (the 1/64 is folded into the DSA exp scale,
                host pre-scales bq by 64). k: two per-j biased ACT copies
                (ACT bias APs are per-partition scalars only)."""
                ps = ps_proj.tile([128, 2, CH], F32, tag="proj",
                                  name=f"ps_{wname}_{jp}")
                for jj in range(2):
                    j = 2 * jp + jj
                    for g in range(2):
                        rhs = (xt[:, g, off:off + CH].bitcast(FP8)
                               .rearrange("p (n s) -> p s n", s=2))
                        nc.tensor.matmul(ps[:, jj, :],
                                         W[wname][:, g, :, j * 128:(j + 1) * 128],
                                         rhs, start=(g == 0), stop=(g == 1),
                                         perf_mode=DR, skip_group_check=True)
                if wname == "wq8":
                    nc.vector.tensor_add(
                        qraw[:, 2 * jp:2 * jp + 2, c % 3, 2:2 + CH], ps[:],
                        bias_cols["bq64_c"][:, 2 * jp:2 * jp + 2]
                        .unsqueeze(2).to_broadcast((128, 2, CH)))
                else:
                    for jj in range(2):
                        j = 2 * jp + jj
                        nc.scalar.activation(kT[:, j, c % 3, :], ps[:, jj, :],
                                             AF.Identity,
                                             bias=bias_cols["bk_c"][:, j:j + 1],
                                             scale=1.0 / W8SCALE)

            def proj_v_pair(xt, off, c, tp):
                ps = ps_proj.tile([128, 2, D], F32, tag="proj", name=f"ps_v_{tp}")
                for tt_ in range(2):
                    tt = 2 * tp + tt_
                    for dk in range(4):
                        nc.tensor.matmul(ps[:, tt_, :],
                                         xt[:, dk, off + tt * 128:off + (tt + 1) * 128],
                                         W["wv"][:, dk, :], start=(dk == 0), stop=False,
                                         skip_group_check=True)
                    nc.tensor.matmul(ps[:, tt_, :], ones_row[:], bv_sb[:], start=False,
                                     stop=True, skip_group_check=True)
                s = (c * 4 + 2 * tp) % 12
                nc.scalar.copy(vtm[:, s:s + 2, :, 0:64],
                               ps[:].rearrange("p t (h d) -> p t h d", h=H))

            def halo_copies(c):
                """After chunk c's q-projections land in slot c%3, export its
                edges into the neighbouring slots' halo columns."""
                if c > 0:
                    nc.scalar.copy(qraw[:, :, (c - 1) % 3, CH + 2:CH + 4],
                                   qraw[:, :, c % 3, 2:4])
                if c + 1 < NCHUNK:
                    nc.scalar.copy(qraw[:, :, (c + 1) % 3, 0:2],
                                   qraw[:, :, c % 3, CH:CH + 2])
                else:
                    nc.vector.memset(qraw[:, :, c % 3, CH + 2:CH + 4], 0.0)

            def pool_chunk(c):
                s = c % 3
                ta = pool_tmp.tile([128, 4, CH + 2], BF16, tag="ta")
                nc.vector.tensor_add(ta[:], qraw[:, :, s, 0:CH + 2],
                                     qraw[:, :, s, 1:CH + 3])
                tb = pool_tmp.tile([128, 4, CH], BF16, tag="tb")
                nc.vector.tensor_add(tb[:], ta[:, :, 0:CH], ta[:, :, 2:CH + 2])
                nc.vector.tensor_add(qpT[:, :, c % 3, :], tb[:],
                                     qraw[:, :, s, 4:CH + 4])

            def dsa_scores(c, lt):
                """MM1 (+ rank-17 additive mask) + exp for tile lt of chunk c."""
                st = ps_st.tile([128, 8, 128], F32, tag="st", name=f"st_{c}_{lt}")
                for h in range(H):
                    hp = PERM[h]
                    base = (h % 2) * 64
                    lhsT = kT[base:base + 64, h // 2, c % 3, lt * 128:(lt + 1) * 128]
                    rhs = qpT[base:base + 64, h // 2, c % 3, lt * 128:(lt + 1) * 128]
                    nc.tensor.matmul(st[:, hp, :], lhsT, rhs, start=True, stop=True,
                                     skip_group_check=True)
                expS = dsa_sb.tile([128, 8, 128], BF16, tag="expS",
                                   name=f"expS_{c}_{lt}")
                # qpT carries a 64x scale (folded out here); alternate the
                # mask mul between DVE and GPSIMD to balance engine load
                nc.scalar.activation(expS[:], st[:], AF.Exp,
                                     scale=SCALE / QNB / W8SCALE)
                eng = nc.vector if lt % 2 == 0 else nc.gpsimd
                eng.tensor_mul(expS[:], expS[:],
                               mask_sb[:].unsqueeze(1).to_broadcast((128, 8, 128)))
                return expS

            def dsa_out(c, lt, masked, ax_out):
                """attn@V with ones-col denominators, then normalize."""
                outp = ps_out.tile([128, 2, 512], F32, tag="outp",
                                   name=f"outp_{c}_{lt}")
                for h in range(H):
                    hp = PERM[h]
                    nc.tensor.matmul(outp[:, h // 4, (h % 4) * 65:(h % 4) * 65 + 65],
                                     masked[:, hp, :],
                                     vtm[:, (c * 4 + lt) % 12, h, :],
                                     start=True, stop=True, skip_group_check=True)
                recip = dsa_sb.tile([128, 2, 4], F32, tag="recip",
                                    name=f"recip_{c}_{lt}")
                den_view = bass.AP(outp.tensor, outp[:].offset + 64,
                                   [outp[:].ap[0], [512, 2], [65, 4]])
                nc.vector.reciprocal(recip[:], den_view)
                # V' already contains +bv (rank-1 matmul in proj_v); attention
                # weights sum to 1 after the 1/den scale, so bias is exact.
                av_view = bass.AP(outp.tensor, outp[:].offset,
                                  [outp[:].ap[0], [512, 2], [65, 4], [1, 64]])
                nc.vector.tensor_mul(
                    ax_out.rearrange("p (a b d) -> p a b d", a=2, b=4),
                    av_view,
                    recip[:].unsqueeze(3).to_broadcast((128, 2, 4, 64)))

            def dsa_group_list(c, ax):
                masked = {}
                g = []
                g.append(lambda: masked.__setitem__(0, dsa_scores(c, 0)))
                g.append(lambda: masked.__setitem__(1, dsa_scores(c, 1)))
                g.append(lambda: dsa_out(c, 0, masked.pop(0), ax[:, 0, :]))
                g.append(lambda: masked.__setitem__(2, dsa_scores(c, 2)))
                g.append(lambda: dsa_out(c, 1, masked.pop(1), ax[:, 1, :]))
                g.append(lambda: masked.__setitem__(3, dsa_scores(c, 3)))
                g.append(lambda: dsa_out(c, 2, masked.pop(2), ax[:, 2, :]))
                g.append(lambda: dsa_out(c, 3, masked.pop(3), ax[:, 3, :]))
                return g

            def store_ax_pair(cp, ax2):
                """Store DSA output for chunks (cp, cp+1), then transpose the
                pair back feature-major (overlaps phase 1). The last pair is
                stored per chunk so the final win_tok transposes - which gate
                the LN chain at the phase transition - wait on a half-size
                store."""
                wsrc = axd.ap().rearrange("(w s) d -> w s d", s=PW)
                dst = axd.ap().rearrange("(g lt p) d -> g p lt d", lt=8, p=128)
                if cp == NCHUNK - 2:
                    # last pair: store per chunk so the final win_tok
                    # transposes - which gate the LN r=1 chain at the phase
                    # transition - wait on half-size stores
                    wis = []
                    for half in range(2):
                        wi = nc.gpsimd.dma_start(
                            out=dst[cp // 2][:, 4 * half:4 * half + 4, :],
                            in_=ax2[:, 4 * half:4 * half + 4, :])
                        wis.append(wi)
                    for dc in range(4):
                        for half in range(2):
                            c64 = (cp + half) * 64
                            wt = nc.sync.dma_start(
                                out=wt_view[:, dc, c64:c64 + 64],
                                in_=wsrc[c64:c64 + 64, 0,
                                         dc * 128:(dc + 1) * 128],
                                transpose=True)
                            add_dep_helper(wt.ins, wis[half].ins,
                                           reason="win_tok transpose")
                else:
                    wi = nc.gpsimd.dma_start(out=dst[cp // 2], in_=ax2[:])
                    wis = [wi]
                    for dc in range(4):
                        wt = nc.sync.dma_start(
                            out=wt_view[:, dc, cp * 64:(cp + 2) * 64],
                            in_=wsrc[cp * 64:(cp + 2) * 64, 0,
                                     dc * 128:(dc + 1) * 128],
                            transpose=True)
                        add_dep_helper(wt.ins, wi.ins,
                                       reason="win_tok transpose reads axd pair")
                axd_writers.append(wis)
                tis = []
                for dc in range(4):
                    ti = nc.sync.dma_start(
                        out=axt[:, dc, cp * CH:(cp + 2) * CH],
                        in_=axd[cp * CH:(cp + 2) * CH, dc * 128:(dc + 1) * 128],
                        transpose=True)
                    for wi in wis:
                        add_dep_helper(ti.ins, wi.ins,
                                       reason="axt transpose reads axd pair")
                    tis.append(ti)
                axt_trans.append(tis)

            xt_tiles = {0: load_xt_super(0)}
            extra = []      # deferred phase-2 prologue work units
            ax2 = None

            for c in range(NCHUNK + 2):
                if c == 5:
                    load_phase2_weights()
                if c == 7:
                    extra.extend(ln_half_groups(0, ps_proj, "proj"))
                pgroups = []
                if c < NCHUNK:
                    sc = c // 2
                    off = (c % 2) * CH
                    if c % 2 == 0 and sc + 1 < NSC:
                        xt_tiles[sc + 1] = load_xt_super(sc + 1)
                    qxt = xt_tiles[sc]["q"]
                    kxt = xt_tiles[sc]["k"]
                    vxt = xt_tiles[sc]["v"]
                    for jp in range(2):
                        pgroups.append(lambda jp=jp, x=qxt, o=off, c=c:
                                       proj_fm_pair(x, o, "wq8", c, jp))
                    for tp in range(2):
                        pgroups.append(lambda tp=tp, x=vxt, o=off, c=c:
                                       proj_v_pair(x, o, c, tp))
                    for jp in range(2):
                        pgroups.append(lambda jp=jp, x=kxt, o=off, c=c:
                                       proj_fm_pair(x, o, "wk8", c, jp))
                dgroups = []
                dc_ = c - 2
                if dc_ >= 0:
                    if dc_ % 2 == 0:
                        ax2 = ax_pool.tile([128, 8, D], BF16, tag="ax",
                                           name=f"ax_{dc_}")
                    axv = ax2[:, (dc_ % 2) * 4:(dc_ % 2) * 4 + 4, :]
                    dgroups = dsa_group_list(dc_, axv)
                # weave: spread D groups evenly through the P stream;
                # pool(c-1) after the 2 Q-projection pairs; extra units fill
                # remaining slots late in phase 1
                npg, ndg = len(pgroups), len(dgroups)
                if npg:
                    # per-pgroup D-group quota (6 pgroups hosting up to 8)
                    quota = (1, 2, 1, 2, 1, 1)
                    di = 0
                    for i in range(npg):
                        pgroups[i]()
                        if i == 2:
                            halo_copies(c)
                            if c >= 1:
                                pool_chunk(c - 1)
                        for _ in range(quota[i]):
                            if di < ndg:
                                dgroups[di]()
                                di += 1
                                if extra:
                                    extra.pop(0)()
                    while di < ndg:
                        dgroups[di]()
                        di += 1
                else:
                    if 1 <= c <= NCHUNK:
                        pool_chunk(c - 1)
                    for g in dgroups:
                        g()
                        if extra:
                            extra.pop(0)()
                if dc_ >= 0 and dc_ % 2 == 1:
                    store_ax_pair(dc_ - 1, ax2)
            while extra:
                extra.pop(0)()

        # ================= phase 2 =================
        with ExitStack() as ctx:
            p2 = ctx.enter_context(tc.tile_pool(name="p2", bufs=1))
            ps2 = ctx.enter_context(tc.tile_pool(name="ps2", bufs=3, space="PSUM"))
            ps2b = ctx.enter_context(tc.tile_pool(name="ps2b", bufs=2, space="PSUM"))
            sb2 = ctx.enter_context(tc.tile_pool(name="sb2", bufs=2))

            bo_sb = singles.tile([1, D], BF16)
            nc.scalar.dma_start(out=bo_sb[:], in_=t["bo_r"][:, :])

            # pv gathered window-major with a SWDGE cast to fp8 for the
            # DoubleRow pout matmuls.
            pv = p2.tile([128, 4, WIN, D], FP8, tag="pv")
            srcv = axd.ap().rearrange("(cc p w) d -> cc p w d", p=128, w=PW)
            for cc in range(4):
                gi = nc.gpsimd.dma_start(out=pv[:, cc, :, :], in_=srcv[cc, :, 1:PW, :])
                for wi in axd_writers[cc]:
                    add_dep_helper(gi.ins, wi.ins, reason="pv gather")
                # ordering shim: run the gathers after the last attn_x pair's
                # window-summary transposes so the transition chain
                # (store -> wt_view -> LN -> pq/pk -> PSA) isn't queued
                # behind them; pout doesn't need pv until well after.
                for ti in axt_trans[-1]:
                    add_dep_helper(gi.ins, ti.ins, reason="dma order")


            # ---- LN + GELU + pq/pk for the second window half ----
            for u in ln_half_groups(1, ps2, "ps2"):
                u()

            # ---- PSA softmax per head; pout per head-pair right after ----
            # Scores run fp8 DoubleRow over [32, 2] feature tiles; exp output
            # stays UNNORMALIZED in fp8 (values ~1.0, ideal e4m3 range). pout
            # contracts raw exp against fp8 pv, and the 1/den normalization is
            # applied afterwards on the psum via a per-pair recip tile whose
            # partition rows are already head-matched (h0 on 0:64, h1 on
            # 64:128) thanks to the DoubleRow den matmul's 64-row output.
            zt = p2.tile([128, 4, QLEN], BF16, tag="zt")
            ones8_2 = p2.tile([128, 2, 128], FP8, tag="ones8")
            nc.vector.memset(ones8_2[:], 1.0)

            def psa_scores(h):
                b32 = 32 * (h % 4)
                a = 2 * (h // 4)
                es = sb2.tile([128, 4, WN], FP8, tag="psa_exp", bufs=8,
                              name=f"es_{h}")
                for cp in range(2):
                    ps = ps2b.tile([128, 2, WN], F32, tag="pair",
                                   name=f"st_{h}_{cp}")
                    for ch in range(2):
                        cc = cp * 2 + ch
                        nc.tensor.matmul(
                            ps[:, ch, :],
                            pkT[b32:b32 + 32, a:a + 2, cc * 128:(cc + 1) * 128],
                            pqT[b32:b32 + 32, a:a + 2, :], start=True, stop=True,
                            perf_mode=DR, skip_group_check=True,
                            tile_position=(b32, 0))
                    nc.scalar.activation(es[:, 2 * cp:2 * cp + 2, :], ps[:],
                                         AF.Exp, scale=SCALE)
                return es

            def psa_norm(j, es0, es1):
                """den + recip for head pair j (heads 2j, 2j+1). DoubleRow
                outputs must sit at column position 0, so each head gets a
                full 128-partition replicated den psum; the recips then read
                partition-aligned halves into one pair tile (h0 rows on 0:64,
                h1 on 64:128) for the single pout normalization mul."""
                recipd = sb2.tile([128, WN], F32, tag="psa_recip", bufs=2,
                                  name=f"r_{j}")
                for half, es in ((0, es0), (1, es1)):
                    ps_den = ps2b.tile([128, WN], F32, tag="psa_den", bufs=1,
                                       name=f"d_{j}_{half}")
                    for cp in range(2):
                        nc.tensor.matmul(
                            ps_den[:], ones8_2[:], es[:, 2 * cp:2 * cp + 2, :],
                            start=(cp == 0), stop=(cp == 1),
                            perf_mode=DR, skip_group_check=True)
                    nc.vector.reciprocal(recipd[half * 64:(half + 1) * 64, :],
                                         ps_den[half * 64:(half + 1) * 64, :])
                return recipd

            def pout_pair(j, wh, es0, es1, recipd):
                """pout for head-pair j over query-window half wh."""
                w0 = wh * (WN // 2)
                for i in range(WIN):
                    po = ps2.tile([128, WN // 2], F32, tag="ps2",
                                  name=f"po_{j}_{i}_{wh}")
                    # DoubleRow requires output column position 0, so only
                    # the half-0 head runs DR; half-1 (psum base 64) uses
                    # plain fp8 matmuls.
                    h0 = 2 * j
                    for cp in range(2):
                        nc.tensor.matmul(
                            po[0:64, :],
                            pv[:, 2 * cp:2 * cp + 2, i, h0 * 64:(h0 + 1) * 64],
                            es0[:, 2 * cp:2 * cp + 2, w0:w0 + WN // 2],
                            start=(cp == 0), stop=(cp == 1),
                            perf_mode=DR, skip_group_check=True)
                    h1 = 2 * j + 1
                    for cc in range(4):
                        nc.tensor.matmul(
                            po[64:128, :],
                            pv[:, cc, i, h1 * 64:(h1 + 1) * 64],
                            es1[:, cc, w0:w0 + WN // 2],
                            start=(cc == 0), stop=(cc == 3),
                            skip_group_check=True)
                    pn = sb2.tile([128, WN // 2], BF16, tag="pn", bufs=4,
                                  name=f"pn_{j}_{i}_{wh}")
                    nc.vector.tensor_mul(pn[:], po[:], recipd[:, w0:w0 + WN // 2])
                    # SBUF-only bf16 add: run it on GPSIMD to keep DVE free
                    # for the psum-reading normalization muls
                    nc.gpsimd.tensor_add(
                        zt[:, j, :].rearrange("p (w i) -> p w i", i=WIN)
                        [:, w0:w0 + WN // 2, i],
                        pn[:],
                        axt[:, j, :].rearrange("p (w s) -> p w s", s=PW)
                        [:, w0:w0 + WN // 2, 1 + i])

            outv = out.ap().rearrange("(g tt p) d -> g p tt d", tt=2, p=128)

            def final_group(g, split_store=False):
                o_sb = sb2.tile([128, 2, D], BF16, tag="osb", bufs=4,
                                name=f"osb_{g}")
                for q in range(2):
                    tt = g * 2 + q
                    ps = ps2.tile([128, D], F32, tag="ps2", name=f"fin_{tt}")
                    for dk in range(4):
                        nc.tensor.matmul(ps[:], zt[:, dk, tt * 128:(tt + 1) * 128],
                                         W["wo"][:, dk, :], start=(dk == 0),
                                         stop=False, skip_group_check=True)
                    # bo via rank-1 matmul; psum evacuation on ACT (idle in
                    # the fin tail) instead of a DVE add
                    nc.tensor.matmul(ps[:], ones_row[:], bo_sb[:], start=False,
                                     stop=True, skip_group_check=True)
                    nc.scalar.copy(o_sb[:, q, :], ps[:])
                    if split_store:
                        nc.sync.dma_start(out=outv[g][:, q, :],
                                          in_=o_sb[:, q, :])
                if not split_store:
                    nc.sync.dma_start(out=outv[g], in_=o_sb[:])

            # pipeline: scores(h+1) | norm(j) once its pair of heads is
            # scored | pout(j) right after; the last pair is split by
            # query-window half so the first finals overlap its second half
            es_store = {0: psa_scores(0)}
            recs = {}
            done_pairs = 0
            for h in range(1, H):
                es_store[h] = psa_scores(h)
                if h % 2 == 1:
                    j = h // 2
                    recs[j] = psa_norm(j, es_store[2 * j], es_store[2 * j + 1])
                if h % 2 == 0 and done_pairs in recs:
                    j = done_pairs
                    pout_pair(j, 0, es_store[2 * j], es_store[2 * j + 1], recs[j])
                    pout_pair(j, 1, es_store[2 * j], es_store[2 * j + 1], recs[j])
                    es_store.pop(2 * j), es_store.pop(2 * j + 1), recs.pop(j)
                    done_pairs += 1
            while done_pairs < 4:
                j = done_pairs
                pout_pair(j, 0, es_store[2 * j], es_store[2 * j + 1], recs[j])
                if j == 3:
                    for g in range(7):
                        final_group(g)
                pout_pair(j, 1, es_store[2 * j], es_store[2 * j + 1], recs[j])
                done_pairs += 1
            for g in range(7, QLEN // 256):
                final_group(g, split_store=(g >= QLEN // 256 - 2))


_NC_CACHE = None


def _get_program():
    global _NC_CACHE
    if _NC_CACHE is None:
        _NC_CACHE = build_program()
    return _NC_CACHE


def _fp8_paired(Wm, perm=None, pair="dc"):
    """Host prep for fp8 DoubleRow lhsT: optional column permutation, x64
    scale, then row pairing. pair="dc": rows (2g+s)*128+p -> [g*128+p, s]
    (matches the wtn dc-group layout); pair="consec": rows 2f+s -> [f, s]
    (matches the uint16-pair input transposes)."""
    w = np.asarray(Wm, np.float32)
    if perm is not None:
        w = w[:, perm]
    w = (w * W8SCALE).astype(ml_dtypes.float8_e4m3fn)
    if pair == "consec":
        return np.ascontiguousarray(w.reshape(256, 2, D))
    # rows: r = g*256 + s*128 + p  ->  out[g*128+p, s, :]
    return np.ascontiguousarray(
        w.reshape(2, 2, 128, D).transpose(0, 2, 1, 3).reshape(256, 2, D))


def _host_consts(Wk, bk, Wv, bv, Wq, bq, ln_g, ln_b, Wpq, bpq, Wpk, bpk, Wo, bo):
    bf = ml_dtypes.bfloat16
    col = lambda b: np.asarray(b, np.float32).reshape(4, 128).T.copy()
    perm = np.asarray(PSA_PERM)
    consts = {
        "wq8": _fp8_paired(Wq, pair="consec"),
        "wk8": _fp8_paired(Wk, pair="consec"),
        "wv": np.asarray(Wv, np.float32).astype(bf),
        "wpq8": _fp8_paired(Wpq, perm),
        "wpk8": _fp8_paired(Wpk, perm),
        "wo": np.asarray(Wo, np.float32).astype(bf),
        "bq64_c": col(np.asarray(bq, np.float32) * W8SCALE), "bk_c": col(bk),
        "bpq_c": col(np.asarray(bpq, np.float32)[perm]),
        "bpk_c": col(np.asarray(bpk, np.float32)[perm]),
        "ln_g_c": col(ln_g), "ln_b_c": col(ln_b),
        "bv_r": np.asarray(bv, np.float32).reshape(1, D).astype(bf),
        "bo_r": np.asarray(bo, np.float32).reshape(1, D).astype(bf),
        "bv_f": np.tile(np.asarray(bv, np.float32).reshape(1, D), (128, 1)).astype(bf),
        "bo_f": np.tile(np.asarray(bo, np.float32).reshape(1, D), (128, 1)).astype(bf),
    }
    m = np.zeros((128, 128), np.float32)
    for g in range(16):
        m[g * PW:(g + 1) * PW, g * PW:(g + 1) * PW] = 1.0
    consts["bmask"] = m.astype(bf)
    return consts


def kernel(k, v, q, query_len, Wk, bk, Wv, bv, Wq, bq, ln_g, ln_b,
           Wpq, bpq, Wpk, bpk, Wo, bo):
    nc = _get_program()
    consts = _host_consts(Wk, bk, Wv, bv, Wq, bq, ln_g, ln_b,
                          Wpq, bpq, Wpk, bpk, Wo, bo)
    k = np.asarray(k, np.float32)
    v = np.asarray(v, np.float32)
    q = np.asarray(q, np.float32)
    in_maps = []
    for b in range(B):
        m = {"q": np.ascontiguousarray(q[b]), "k": np.ascontiguousarray(k[b]),
             "v": np.ascontiguousarray(v[b])}
        m.update(consts)
        in_maps.append(m)
    res = run_bass_kernel_spmd(nc, in_maps, core_ids=list(range(B)))
    return np.stack([np.asarray(res.results[b]["out"], np.float32)
                     for b in range(B)], axis=0)


if __name__ == "__main__":
    nc = build_program()
    print("program built ok")



# revision 94
# speedup vs baseline: 1.0158x; 1.0031x over previous
"""Trainium2 Bass kernel for DeformableMultiHeadedAttention.

Data-parallel over batch B=8 across 8 NeuronCores (one batch element per
core, identical programs, no collectives). Heavy matmuls run fp8-e4m3
DoubleRow (0.5 PE cycles/row) wherever a numpy precision study showed the
final rel-err stays ~0.006 (tolerance 2e-2); V projection and the final
Z@Wo stay bf16 (fp8 there blows the budget).

Per-core pipeline (f32 psum accumulate everywhere):
  1. q,k f32 -> SWDGE cast-DMA -> DRAM fp8; v -> bf16. q/k DMA-transpose as
     uint16 feature-PAIRS, which lands directly in DoubleRow's [K,2,M]
     operand layout; v transposes bf16 feature-major.
  2. Projections on PE: Q'/K' via fp8 DoubleRow (weights host-scaled x64,
     paired rows [2f, 2f+1]); 1/64 descales in the psum-evacuation copies
     (k via per-j ACT Identity+bias) or folds into the DSA exp scale (qraw
     keeps 64x, one paired DVE add with a [128,2,1]-broadcast bias). V'
     token-major bf16 with bias via a K=1 rank-1 matmul.
  3. Q pooling (AvgPool k=5, zero pad) as 3 shifted DVE adds; 1/5 folded
     into the exp scale.
  4. DSA (windows of 8): per 128-token tile: bank-segregated 64-row score
     matmuls, exp on ACT, block-diag mask mul alternating DVE/GPSIMD,
     attn@V with a ones-column denominator, 1/den scale on DVE. Output
     token-major -> DRAM (axd, bf16).
  5. Re-layouts from axd: wt_view (window-summary transposes), axt
     (feature-major attn_x), pv (window-major payload, SWDGE cast to fp8).
  6. win_tok LayerNorm + exact GELU -> wtn fp8; pq/pk projections fp8
     DoubleRow with host column-permuted weights so each head sits on 32
     partitions x 2 dc-slots; PSA scores fp8 DoubleRow per head.
  7. PSA softmax normalization deferred: exp stays UNNORMALIZED in fp8
     (values ~1.0, ideal e4m3 range); pout contracts raw exp against fp8 pv
     (DoubleRow half-0 head / plain fp8 half-1 head - DoubleRow outputs must
     sit at psum column position 0); per-pair replicated den (DoubleRow
     ones matmuls) -> recip -> one DVE mul on the pout psum; Z = pn + attn_x
     via GPSIMD adds (SBUF-only bf16).
  8. Final out = Z @ Wo bf16 with bo via rank-1 matmul, ACT psum copies,
     bf16 DRAM stores (host upcasts to f32).
"""

import sys
from contextlib import ExitStack

for _p in ("/opt/trn_rl_repo/concourse", "/opt/trn_rl_repo"):
    if _p not in sys.path:
        sys.path.insert(0, _p)

import numpy as np
import ml_dtypes

import concourse.bass as bass
import concourse.mybir as mybir
import concourse.tile as tile
from concourse import bacc
from concourse.tile import add_dep_helper
from concourse.bass_utils import run_bass_kernel_spmd

BF16 = mybir.dt.bfloat16
F32 = mybir.dt.float32
FP8 = mybir.dt.float8e4
DR = mybir.MatmulPerfMode.DoubleRow
AF = mybir.ActivationFunctionType
ALU = mybir.AluOpType

B, M, D = 8, 4096, 512
H, HD = 8, 64
WIN = 7
PW = WIN + 1
QNB = 5
QLEN = 3584
WN = M // PW
SCALE = D ** -0.5
EPS = 1e-5
NCHUNK = 8
CH = 512
SCH = 1024           # super-chunk (transpose batch) size
NSC = M // SCH
CAST_RANGES = [(0, 1024), (1024, 2048), (2048, 4096)]
SC2CAST = {0: (0, 0), 1: (1, 0), 2: (2, 0), 3: (2, 1024)}  # sc -> (group, row0)
PERM = [(h % 2) * 4 + h // 2 for h in range(H)]  # head -> DSA psum slot
W8SCALE = 64.0  # host pre-scale on fp8 weights; 1/64 folded into psum copies

# PSA pq/pk column permutation: dc-group j, partition p -> original dout.
# Head h occupies 32 partitions at 32*(h%4) across the dc pair 2*(h//4),
# with features 0:32 in the even dc and 32:64 in the odd dc, so psa score
# matmuls can run fp8 DoubleRow over [32, 2] feature tiles.
PSA_PERM = [((j // 2) * 4 + p // 32) * 64 + (j % 2) * 32 + (p % 32)
            for j in range(4) for p in range(128)]


def build_program():
    nc = bacc.Bacc("TRN2", target_bir_lowering=False, debug=False, num_devices=8)

    t = {}
    t["q_in"] = nc.dram_tensor("q", [M, D], F32, kind="ExternalInput")
    t["k_in"] = nc.dram_tensor("k", [M, D], F32, kind="ExternalInput")
    t["v_in"] = nc.dram_tensor("v", [M, D], F32, kind="ExternalInput")
    for nm in ("wv", "wo"):
        t[nm] = nc.dram_tensor(nm, [D, D], BF16, kind="ExternalInput")
    for nm in ("wq8", "wk8", "wpq8", "wpk8"):
        t[nm] = nc.dram_tensor(nm, [256, 2, D], FP8, kind="ExternalInput")
    for nm in ("bq64_c", "bk_c", "bpq_c", "bpk_c", "ln_g_c", "ln_b_c"):
        t[nm] = nc.dram_tensor(nm, [128, 4], F32, kind="ExternalInput")
    t["bv_r"] = nc.dram_tensor("bv_r", [1, D], BF16, kind="ExternalInput")
    t["bo_r"] = nc.dram_tensor("bo_r", [1, D], BF16, kind="ExternalInput")
    t["bv_f"] = nc.dram_tensor("bv_f", [128, D], BF16, kind="ExternalInput")
    t["bo_f"] = nc.dram_tensor("bo_f", [128, D], BF16, kind="ExternalInput")
    t["bmask"] = nc.dram_tensor("bmask", [128, 128], BF16, kind="ExternalInput")
    t["out"] = nc.dram_tensor("out", [QLEN, D], BF16, kind="ExternalOutput")
    t["axd"] = nc.dram_tensor("axd_s", [M, D], BF16, kind="Internal")
    t["zd"] = nc.dram_tensor("zd_s", [QLEN, D], BF16, kind="Internal")
    # per-group cast targets: separate tensors so the tile framework's
    # tensor-granular dependency tracking doesn't serialize casts behind
    # earlier chunks' transpose reads (false WAR). First two groups are
    # small so compute can start early.
    # q/k cast straight to fp8 (transposed later as uint16 feature-pairs,
    # which lands in exactly the DoubleRow [K,2,M] operand layout); v stays
    # bf16 for precision.
    for nm in ("q", "k", "v"):
        dt_ = BF16 if nm == "v" else FP8
        for g, (lo, hi) in enumerate(CAST_RANGES):
            t[f"{nm}b{g}"] = nc.dram_tensor(f"{nm}b{g}_s", [hi - lo, D], dt_,
                                            kind="Internal")

    with tile.TileContext(nc) as tc:
        _build(nc, tc, t)
    nc.compile()
    return nc


def _build(nc, tc, t):
    axd, out = t["axd"], t["out"]

    with ExitStack() as octx:
        singles = octx.enter_context(tc.tile_pool(name="singles", bufs=1))

        # f32->bf16 cast DMAs first; few large batches keep the SWDGE
        # completion-semaphore lanes from being recycled between casts.
        cast_insts = {}
        srcs_d = {"q": t["q_in"], "k": t["k_in"], "v": t["v_in"]}
        for g, (lo, hi) in enumerate(CAST_RANGES):
            for nm in ("q", "v", "k"):
                ci = nc.gpsimd.dma_start(
                    out=t[f"{nm}b{g}"][:, :],
                    in_=srcs_d[nm][lo:hi, :])
                cast_insts[(nm, g)] = ci

        W = {}
        # wq loads immediately (first projection needs it); the other loads
        # are gated behind the first v cast so the q/v transposes win the
        # startup DMA race.
        gate0 = cast_insts[("v", 0)]
        for nm in ("wq8", "wk8"):
            W[nm] = singles.tile([128, 2, 2, D], FP8, tag=nm, name=f"w_{nm}")
            wi = nc.scalar.dma_start(out=W[nm][:],
                                     in_=t[nm].ap().rearrange(
                                         "(g p) s d -> p g s d", p=128))
            if nm != "wq8":
                add_dep_helper(wi.ins, gate0.ins, reason="dma order")
        W["wv"] = singles.tile([128, 4, D], BF16, tag="wv", name="w_wv")
        wi = nc.scalar.dma_start(out=W["wv"][:],
                                 in_=t["wv"].ap().rearrange("(c p) d -> p c d",
                                                            p=128))
        add_dep_helper(wi.ins, gate0.ins, reason="dma order")
        bias_cols = {}
        for nm in ("bq64_c", "bk_c"):
            bias_cols[nm] = singles.tile([128, 4], F32, tag=nm, name=f"bc_{nm}")
            nc.scalar.dma_start(out=bias_cols[nm][:], in_=t[nm][:, :])
        bv_sb = singles.tile([1, D], BF16)
        nc.scalar.dma_start(out=bv_sb[:], in_=t["bv_r"][:, :])
        mask_sb = singles.tile([128, 128], BF16)
        wi = nc.scalar.dma_start(out=mask_sb[:], in_=t["bmask"][:, :])
        add_dep_helper(wi.ins, cast_insts[("k", 0)].ins, reason="dma order")
        ones_row = singles.tile([1, 128], BF16)
        nc.vector.memset(ones_row[:], 1.0)
        ones_col = singles.tile([128, 1], BF16)
        nc.vector.memset(ones_col[:], 1.0)
        ones_full = singles.tile([128, 128], BF16)
        nc.vector.memset(ones_full[:], 1.0)
        eps_sb = singles.tile([128, 1], F32)
        nc.vector.memset(eps_sb[:], EPS)

        axd_writers = []
        axt_trans = []
        p2a = octx.enter_context(tc.tile_pool(name="p2a", bufs=1))
        axt = p2a.tile([128, 4, M], BF16, tag="axt")

        def load_phase2_weights():
            # ordering shim: keep these dep-free loads from being hoisted by
            # the scheduler ahead of the startup-critical input transposes.
            # Gated on the first attn_x pair store (~1/3 through phase 1),
            # which fires well before Act SEQ reaches these instructions, so
            # no head-of-line blocking on the Act sequencer.
            W["wo"] = singles.tile([128, 4, D], BF16, tag="wo", name="w_wo")
            nc.scalar.dma_start(out=W["wo"][:],
                                in_=t["wo"].ap().rearrange(
                                    "(c p) d -> p c d", p=128))
            for nm in ("wpq8", "wpk8"):
                W[nm] = singles.tile([128, 2, 2, D], FP8, tag=nm, name=f"w_{nm}")
                nc.scalar.dma_start(out=W[nm][:],
                                    in_=t[nm].ap().rearrange(
                                        "(g p) s d -> p g s d", p=128))
            for nm in ("bpq_c", "bpk_c", "ln_g_c", "ln_b_c"):
                bias_cols[nm] = singles.tile([128, 4], F32, tag=nm, name=f"bc_{nm}")
                nc.scalar.dma_start(out=bias_cols[nm][:], in_=t[nm][:, :])

        # ---- win_tok LN + GELU + pq/pk projections, by window quarters.
        # Quarters 0-2 run inside phase 1 as their attn_x pairs land; only
        # quarter 3 remains for the phase transition.
        lnp = octx.enter_context(tc.tile_pool(name="lnp", bufs=1))
        wtn = lnp.tile([128, 4, WN], FP8, tag="wtn")
        pqT = lnp.tile([128, 4, WN], FP8, tag="pqT")
        pkT = lnp.tile([128, 4, WN], FP8, tag="pkT")
        # dedicated feature-major copy of the window-summary tokens: cheap
        # strided-row transposes that unblock LN without the full axt pair
        wt_view = lnp.tile([128, 4, WN], BF16, tag="wtT")
        RN = WN // 2

        def ln_half_groups(r, psum_pool, ps_tag):
            st = {}

            def u_sq():
                wsq = lnp.tile([128, 4, RN], BF16, tag="wsq", name=f"wsq_{r}")
                src = wt_view[:, :, r * RN:(r + 1) * RN]
                if r == 1:
                    # transition half: DVE is idle here and this avoids an
                    # ACT Square-table reload on the critical chain
                    nc.vector.tensor_mul(wsq[:], src, src)
                else:
                    nc.scalar.activation(wsq[:], src, AF.Square)
                st["wsq"] = wsq

            def u_moments():
                ps = psum_pool.tile([128, 2, RN], F32, tag=ps_tag,
                                    name=f"ln_ps_{r}")
                for j in range(4):
                    nc.tensor.matmul(ps[:, 0, :], ones_full[:],
                                     wt_view[:, j, r * RN:(r + 1) * RN],
                                     start=(j == 0), stop=(j == 3),
                                     skip_group_check=True)
                    nc.tensor.matmul(ps[:, 1, :], ones_full[:], st["wsq"][:, j, :],
                                     start=(j == 0), stop=(j == 3),
                                     skip_group_check=True)
                mu = lnp.tile([128, RN], F32, tag="mu_sb", name=f"mu_{r}")
                nc.scalar.mul(mu[:], ps[:, 0, :], 1.0 / D)
                ex2 = lnp.tile([128, RN], F32, tag="ex2_sb", name=f"ex2_{r}")
                nc.scalar.mul(ex2[:], ps[:, 1, :], 1.0 / D)
                st["mu"], st["ex2"] = mu, ex2

            def u_stats():
                mu, ex2 = st["mu"], st["ex2"]
                var = lnp.tile([128, RN], F32, tag="var_sb", name=f"var_{r}")
                nc.vector.tensor_mul(var[:], mu[:], mu[:])
                nc.vector.tensor_sub(var[:], ex2[:], var[:])
                sd = lnp.tile([128, RN], F32, tag="sd", name=f"sd_{r}")
                nc.scalar.activation(sd[:], var[:], AF.Sqrt, bias=eps_sb[:])
                rstd = lnp.tile([128, RN], F32, tag="rstd", name=f"rstd_{r}")
                nc.vector.reciprocal(rstd[:], sd[:])
                st["rstd"] = rstd

            def u_ln(j):
                tmp = lnp.tile([128, RN], F32, tag="lntmp", bufs=2,
                               name=f"lnt_{r}_{j}")
                nc.vector.tensor_sub(tmp[:], wt_view[:, j, r * RN:(r + 1) * RN],
                                     st["mu"][:])
                nc.vector.tensor_mul(tmp[:], tmp[:], st["rstd"][:])
                nc.scalar.activation(wtn[:, j, r * RN:(r + 1) * RN], tmp[:],
                                     AF.Gelu,
                                     bias=bias_cols["ln_b_c"][:, j:j + 1],
                                     scale=bias_cols["ln_g_c"][:, j:j + 1])

            def u_pp(j):
                ps = psum_pool.tile([128, 2, RN], F32, tag=ps_tag,
                                    name=f"pp_{r}_{j}")
                for g in range(2):
                    nc.tensor.matmul(ps[:, 0, :],
                                     W["wpq8"][:, g, :, j * 128:(j + 1) * 128],
                                     wtn[:, 2 * g:2 * g + 2, r * RN:(r + 1) * RN],
                                     start=(g == 0), stop=(g == 1),
                                     perf_mode=DR, skip_group_check=True)
                    nc.tensor.matmul(ps[:, 1, :],
                                     W["wpk8"][:, g, :, j * 128:(j + 1) * 128],
                                     wtn[:, 2 * g:2 * g + 2, r * RN:(r + 1) * RN],
                                     start=(g == 0), stop=(g == 1),
                                     perf_mode=DR, skip_group_check=True)
                nc.scalar.activation(pqT[:, j, r * RN:(r + 1) * RN],
                                     ps[:, 0, :], AF.Identity,
                                     bias=bias_cols["bpq_c"][:, j:j + 1],
                                     scale=1.0 / W8SCALE)
                nc.scalar.activation(pkT[:, j, r * RN:(r + 1) * RN],
                                     ps[:, 1, :], AF.Identity,
                                     bias=bias_cols["bpk_c"][:, j:j + 1],
                                     scale=1.0 / W8SCALE)

            g = [u_sq, u_moments, u_stats]
            g += [lambda j=j: u_ln(j) for j in range(4)]
            g += [lambda j=j: u_pp(j) for j in range(4)]
            return g

        # ================= phase 1 =================
        with ExitStack() as ctx:
            p1 = ctx.enter_context(tc.tile_pool(name="p1", bufs=1))
            kT = p1.tile([128, 4, 3, CH], BF16, tag="kT")        # ring of 3 chunks
            qpT = p1.tile([128, 4, 3, CH], BF16, tag="qpT")      # ring of 3 chunks
            vtm = p1.tile([128, 12, 8, 65], BF16, tag="vtm")     # ring of 12 tiles, 65-col/head
            nc.vector.memset(vtm[:, :, :, 64:65], 1.0)           # ones col for denominators
            # projected-q ring of 3 chunk slots with 2-col halos on each side:
            # slot layout [0:2]=left halo, [2:CH+2]=chunk body, [CH+2:CH+4]=right halo
            qraw = p1.tile([128, 4, 3, CH + 4], BF16, tag="qraw")
            nc.vector.memset(qraw[:, :, 0, 0:2], 0.0)            # chunk 0 left edge

            xtp = ctx.enter_context(tc.tile_pool(name="xtp", bufs=3))
            ps_proj = ctx.enter_context(tc.tile_pool(name="ps_proj", bufs=2, space="PSUM"))
            ps_st = ctx.enter_context(tc.tile_pool(name="ps_st", bufs=1, space="PSUM"))
            ps_out = ctx.enter_context(tc.tile_pool(name="ps_out", bufs=1, space="PSUM"))
            dsa_sb = ctx.enter_context(tc.tile_pool(name="dsa_sb", bufs=3))
            pool_tmp = ctx.enter_context(tc.tile_pool(name="pool_tmp", bufs=2))
            ax_pool = ctx.enter_context(tc.tile_pool(name="ax_sb", bufs=2))

            def load_xt_super(sc):
                g, row0 = SC2CAST[sc]
                tiles = {}
                # v: bf16, 4 feature groups of 128. q/k: fp8 transposed as
                # uint16 feature-PAIRS (2 groups of 128 pairs) -> partition p
                # of group gg holds features 2*(gg*128+p), 2*(gg*128+p)+1
                # interleaved, exactly the DoubleRow [K, 2, M] layout.
                xt = xtp.tile([128, 4, SCH], BF16, tag="xt_v",
                              name=f"xt_v_{sc}")
                for dc in range(4):
                    ti = nc.sync.dma_start(
                        out=xt[:, dc, :],
                        in_=t[f"vb{g}"][row0:row0 + SCH,
                                        dc * 128:(dc + 1) * 128],
                        transpose=True)
                    add_dep_helper(ti.ins, cast_insts[("v", g)].ins,
                                   reason="transpose reads cast output")
                tiles["v"] = xt
                for nm in ("q", "k"):
                    xt = xtp.tile([128, 2, SCH], mybir.dt.uint16,
                                  tag=f"xt_{nm}", name=f"xt_{nm}_{sc}")
                    src16 = t[f"{nm}b{g}"].ap().bitcast(mybir.dt.uint16)
                    for gg in range(2):
                        ti = nc.sync.dma_start(
                            out=xt[:, gg, :],
                            in_=src16[row0:row0 + SCH,
                                      gg * 128:(gg + 1) * 128],
                            transpose=True)
                        add_dep_helper(ti.ins, cast_insts[(nm, g)].ins,
                                       reason="transpose reads cast output")
                    tiles[nm] = xt
                return tiles

            def proj_fm_pair(xt, off, wname, c, jp):
                """Projections for j-group pair (2jp, 2jp+1) into one 2-bank
                psum. q: one paired DVE add (bias [128,2,1] broadcast) writes
                qraw at 64x scale (the 1/64 is folded into the DSA exp scale,
                host pre-scales bq by 64). k: two per-j biased ACT copies
                (ACT bias APs are per-partition scalars only)."""
                ps = ps_proj.tile([128, 2, CH], F32, tag="proj",
                                  name=f"ps_{wname}_{jp}")
                for jj in range(2):
                    j = 2 * jp + jj
                    for g in range(2):
                        rhs = (xt[:, g, off:off + CH].bitcast(FP8)
                               .rearrange("p (n s) -> p s n", s=2))
                        nc.tensor.matmul(ps[:, jj, :],
                                         W[wname][:, g, :, j * 128:(j + 1) * 128],
                                         rhs, start=(g == 0), stop=(g == 1),
                                         perf_mode=DR, skip_group_check=True)
                if wname == "wq8":
                    nc.vector.tensor_add(
                        qraw[:, 2 * jp:2 * jp + 2, c % 3, 2:2 + CH], ps[:],
                        bias_cols["bq64_c"][:, 2 * jp:2 * jp + 2]
                        .unsqueeze(2).to_broadcast((128, 2, CH)))
                else:
                    for jj in range(2):
                        j = 2 * jp + jj
                        nc.scalar.activation(kT[:, j, c % 3, :], ps[:, jj, :],
                                             AF.Identity,
                                             bias=bias_cols["bk_c"][:, j:j + 1],
                                             scale=1.0 / W8SCALE)

            def proj_v_pair(xt, off, c, tp):
                ps = ps_proj.tile([128, 2, D], F32, tag="proj", name=f"ps_v_{tp}")
                for tt_ in range(2):
                    tt = 2 * tp + tt_
                    for dk in range(4):
                        nc.tensor.matmul(ps[:, tt_, :],
                                         xt[:, dk, off + tt * 128:off + (tt + 1) * 128],
                                         W["wv"][:, dk, :], start=(dk == 0), stop=False,
                                         skip_group_check=True)
                    nc.tensor.matmul(ps[:, tt_, :], ones_row[:], bv_sb[:], start=False,
                                     stop=True, skip_group_check=True)
                s = (c * 4 + 2 * tp) % 12
                nc.scalar.copy(vtm[:, s:s + 2, :, 0:64],
                               ps[:].rearrange("p t (h d) -> p t h d", h=H))

            def halo_copies(c):
                """After chunk c's q-projections land in slot c%3, export its
                edges into the neighbouring slots' halo columns."""
                if c > 0:
                    nc.scalar.copy(qraw[:, :, (c - 1) % 3, CH + 2:CH + 4],
                                   qraw[:, :, c % 3, 2:4])
                if c + 1 < NCHUNK:
                    nc.scalar.copy(qraw[:, :, (c + 1) % 3, 0:2],
                                   qraw[:, :, c % 3, CH:CH + 2])
                else:
                    nc.vector.memset(qraw[:, :, c % 3, CH + 2:CH + 4], 0.0)

            def pool_chunk(c):
                s = c % 3
                ta = pool_tmp.tile([128, 4, CH + 2], BF16, tag="ta")
                nc.vector.tensor_add(ta[:], qraw[:, :, s, 0:CH + 2],
                                     qraw[:, :, s, 1:CH + 3])
                tb = pool_tmp.tile([128, 4, CH], BF16, tag="tb")
                nc.vector.tensor_add(tb[:], ta[:, :, 0:CH], ta[:, :, 2:CH + 2])
                nc.vector.tensor_add(qpT[:, :, c % 3, :], tb[:],
                                     qraw[:, :, s, 4:CH + 4])

            def dsa_scores(c, lt):
                """MM1 (+ rank-17 additive mask) + exp for tile lt of chunk c."""
                st = ps_st.tile([128, 8, 128], F32, tag="st", name=f"st_{c}_{lt}")
                for h in range(H):
                    hp = PERM[h]
                    base = (h % 2) * 64
                    lhsT = kT[base:base + 64, h // 2, c % 3, lt * 128:(lt + 1) * 128]
                    rhs = qpT[base:base + 64, h // 2, c % 3, lt * 128:(lt + 1) * 128]
                    nc.tensor.matmul(st[:, hp, :], lhsT, rhs, start=True, stop=True,
                                     skip_group_check=True)
                expS = dsa_sb.tile([128, 8, 128], BF16, tag="expS",
                                   name=f"expS_{c}_{lt}")
                # qpT carries a 64x scale (folded out here); alternate the
                # mask mul between DVE and GPSIMD to balance engine load
                nc.scalar.activation(expS[:], st[:], AF.Exp,
                                     scale=SCALE / QNB / W8SCALE)
                eng = nc.vector if lt % 2 == 0 else nc.gpsimd
                eng.tensor_mul(expS[:], expS[:],
                               mask_sb[:].unsqueeze(1).to_broadcast((128, 8, 128)))
                return expS

            def dsa_out(c, lt, masked, ax_out):
                """attn@V with ones-col denominators, then normalize."""
                outp = ps_out.tile([128, 2, 512], F32, tag="outp",
                                   name=f"outp_{c}_{lt}")
                for h in range(H):
                    hp = PERM[h]
                    nc.tensor.matmul(outp[:, h // 4, (h % 4) * 65:(h % 4) * 65 + 65],
                                     masked[:, hp, :],
                                     vtm[:, (c * 4 + lt) % 12, h, :],
                                     start=True, stop=True, skip_group_check=True)
                recip = dsa_sb.tile([128, 2, 4], F32, tag="recip",
                                    name=f"recip_{c}_{lt}")
                den_view = bass.AP(outp.tensor, outp[:].offset + 64,
                                   [outp[:].ap[0], [512, 2], [65, 4]])
                nc.vector.reciprocal(recip[:], den_view)
                # V' already contains +bv (rank-1 matmul in proj_v); attention
                # weights sum to 1 after the 1/den scale, so bias is exact.
                av_view = bass.AP(outp.tensor, outp[:].offset,
                                  [outp[:].ap[0], [512, 2], [65, 4], [1, 64]])
                nc.vector.tensor_mul(
                    ax_out.rearrange("p (a b d) -> p a b d", a=2, b=4),
                    av_view,
                    recip[:].unsqueeze(3).to_broadcast((128, 2, 4, 64)))

            def dsa_group_list(c, ax):
                masked = {}
                g = []
                g.append(lambda: masked.__setitem__(0, dsa_scores(c, 0)))
                g.append(lambda: masked.__setitem__(1, dsa_scores(c, 1)))
                g.append(lambda: dsa_out(c, 0, masked.pop(0), ax[:, 0, :]))
                g.append(lambda: masked.__setitem__(2, dsa_scores(c, 2)))
                g.append(lambda: dsa_out(c, 1, masked.pop(1), ax[:, 1, :]))
                g.append(lambda: masked.__setitem__(3, dsa_scores(c, 3)))
                g.append(lambda: dsa_out(c, 2, masked.pop(2), ax[:, 2, :]))
                g.append(lambda: dsa_out(c, 3, masked.pop(3), ax[:, 3, :]))
                return g

            def store_ax_pair(cp, ax2):
                """Store DSA output for chunks (cp, cp+1), then transpose the
                pair back feature-major (overlaps phase 1). The last pair is
                stored per chunk so the final win_tok transposes - which gate
                the LN chain at the phase transition - wait on a half-size
                store."""
                wsrc = axd.ap().rearrange("(w s) d -> w s d", s=PW)
                dst = axd.ap().rearrange("(g lt p) d -> g p lt d", lt=8, p=128)
                if cp == NCHUNK - 2:
                    # last pair: store per chunk so the final win_tok
                    # transposes - which gate the LN r=1 chain at the phase
                    # transition - wait on half-size stores
                    wis = []
                    for half in range(2):
                        wi = nc.gpsimd.dma_start(
                            out=dst[cp // 2][:, 4 * half:4 * half + 4, :],
                            in_=ax2[:, 4 * half:4 * half + 4, :])
                        wis.append(wi)
                    for dc in range(4):
                        for half in range(2):
                            c64 = (cp + half) * 64
                            wt = nc.sync.dma_start(
                                out=wt_view[:, dc, c64:c64 + 64],
                                in_=wsrc[c64:c64 + 64, 0,
                                         dc * 128:(dc + 1) * 128],
                                transpose=True)
                            add_dep_helper(wt.ins, wis[half].ins,
                                           reason="win_tok transpose")
                else:
                    wi = nc.gpsimd.dma_start(out=dst[cp // 2], in_=ax2[:])
                    wis = [wi]
                    for dc in range(4):
                        wt = nc.sync.dma_start(
                            out=wt_view[:, dc, cp * 64:(cp + 2) * 64],
                            in_=wsrc[cp * 64:(cp + 2) * 64, 0,
                                     dc * 128:(dc + 1) * 128],
                            transpose=True)
                        add_dep_helper(wt.ins, wi.ins,
                                       reason="win_tok transpose reads axd pair")
                axd_writers.append(wis)
                tis = []
                for dc in range(4):
                    ti = nc.sync.dma_start(
                        out=axt[:, dc, cp * CH:(cp + 2) * CH],
                        in_=axd[cp * CH:(cp + 2) * CH, dc * 128:(dc + 1) * 128],
                        transpose=True)
                    for wi in wis:
                        add_dep_helper(ti.ins, wi.ins,
                                       reason="axt transpose reads axd pair")
                    tis.append(ti)
                axt_trans.append(tis)

            xt_tiles = {0: load_xt_super(0)}
            extra = []      # deferred phase-2 prologue work units
            ax2 = None

            for c in range(NCHUNK + 2):
                if c == 5:
                    load_phase2_weights()
                if c == 7:
                    extra.extend(ln_half_groups(0, ps_proj, "proj"))
                pgroups = []
                if c < NCHUNK:
                    sc = c // 2
                    off = (c % 2) * CH
                    if c % 2 == 0 and sc + 1 < NSC:
                        xt_tiles[sc + 1] = load_xt_super(sc + 1)
                    qxt = xt_tiles[sc]["q"]
                    kxt = xt_tiles[sc]["k"]
                    vxt = xt_tiles[sc]["v"]
                    for jp in range(2):
                        pgroups.append(lambda jp=jp, x=qxt, o=off, c=c:
                                       proj_fm_pair(x, o, "wq8", c, jp))
                    for tp in range(2):
                        pgroups.append(lambda tp=tp, x=vxt, o=off, c=c:
                                       proj_v_pair(x, o, c, tp))
                    for jp in range(2):
                        pgroups.append(lambda jp=jp, x=kxt, o=off, c=c:
                                       proj_fm_pair(x, o, "wk8", c, jp))
                dgroups = []
                dc_ = c - 2
                if dc_ >= 0:
                    if dc_ % 2 == 0:
                        ax2 = ax_pool.tile([128, 8, D], BF16, tag="ax",
                                           name=f"ax_{dc_}")
                    axv = ax2[:, (dc_ % 2) * 4:(dc_ % 2) * 4 + 4, :]
                    dgroups = dsa_group_list(dc_, axv)
                # weave: spread D groups evenly through the P stream;
                # pool(c-1) after the 2 Q-projection pairs; extra units fill
                # remaining slots late in phase 1
                npg, ndg = len(pgroups), len(dgroups)
                if npg:
                    # per-pgroup D-group quota (6 pgroups hosting up to 8)
                    quota = (1, 2, 1, 2, 1, 1)
                    di = 0
                    for i in range(npg):
                        pgroups[i]()
                        if i == 2:
                            halo_copies(c)
                            if c >= 1:
                                pool_chunk(c - 1)
                        for _ in range(quota[i]):
                            if di < ndg:
                                dgroups[di]()
                                di += 1
                                if extra:
                                    extra.pop(0)()
                    while di < ndg:
                        dgroups[di]()
                        di += 1
                else:
                    if 1 <= c <= NCHUNK:
                        pool_chunk(c - 1)
                    for g in dgroups:
                        g()
                        if extra:
                            extra.pop(0)()
                if dc_ >= 0 and dc_ % 2 == 1:
                    store_ax_pair(dc_ - 1, ax2)
            while extra:
                extra.pop(0)()

        # ================= phase 2 =================
        with ExitStack() as ctx:
            p2 = ctx.enter_context(tc.tile_pool(name="p2", bufs=1))
            ps2 = ctx.enter_context(tc.tile_pool(name="ps2", bufs=3, space="PSUM"))
            ps2b = ctx.enter_context(tc.tile_pool(name="ps2b", bufs=2, space="PSUM"))
            sb2 = ctx.enter_context(tc.tile_pool(name="sb2", bufs=2))

            bo_sb = singles.tile([1, D], BF16)
            nc.scalar.dma_start(out=bo_sb[:], in_=t["bo_r"][:, :])

            # pv gathered window-major with a SWDGE cast to fp8 for the
            # DoubleRow pout matmuls.
            pv = p2.tile([128, 4, WIN, D], FP8, tag="pv")
            srcv = axd.ap().rearrange("(cc p w) d -> cc p w d", p=128, w=PW)
            for cc in range(4):
                gi = nc.gpsimd.dma_start(out=pv[:, cc, :, :], in_=srcv[cc, :, 1:PW, :])
                for wi in axd_writers[cc]:
                    add_dep_helper(gi.ins, wi.ins, reason="pv gather")
                # ordering shim: run the gathers after the last attn_x pair's
                # window-summary transposes so the transition chain
                # (store -> wt_view -> LN -> pq/pk -> PSA) isn't queued
                # behind them; pout doesn't need pv until well after.
                for ti in axt_trans[-1]:
                    add_dep_helper(gi.ins, ti.ins, reason="dma order")


            # ---- LN + GELU + pq/pk for the second window half ----
            for u in ln_half_groups(1, ps2, "ps2"):
                u()

            # ---- PSA softmax per head; pout per head-pair right after ----
            # Scores run fp8 DoubleRow over [32, 2] feature tiles; exp output
            # stays UNNORMALIZED in fp8 (values ~1.0, ideal e4m3 range). pout
            # contracts raw exp against fp8 pv, and the 1/den normalization is
            # applied afterwards on the psum via a per-pair recip tile whose
            # partition rows are already head-matched (h0 on 0:64, h1 on
            # 64:128) thanks to the DoubleRow den matmul's 64-row output.
            zt = p2.tile([128, 4, QLEN], BF16, tag="zt")
            ones8_2 = p2.tile([128, 2, 128], FP8, tag="ones8")
            nc.vector.memset(ones8_2[:], 1.0)

            def psa_scores(h):
                b32 = 32 * (h % 4)
                a = 2 * (h // 4)
                es = sb2.tile([128, 4, WN], FP8, tag="psa_exp", bufs=8,
                              name=f"es_{h}")
                for cp in range(2):
                    ps = ps2b.tile([128, 2, WN], F32, tag="pair",
                                   name=f"st_{h}_{cp}")
                    for ch in range(2):
                        cc = cp * 2 + ch
                        nc.tensor.matmul(
                            ps[:, ch, :],
                            pkT[b32:b32 + 32, a:a + 2, cc * 128:(cc + 1) * 128],
                            pqT[b32:b32 + 32, a:a + 2, :], start=True, stop=True,
                            perf_mode=DR, skip_group_check=True,
                            tile_position=(b32, 0))
                    nc.scalar.activation(es[:, 2 * cp:2 * cp + 2, :], ps[:],
                                         AF.Exp, scale=SCALE)
                return es

            def psa_norm(j, es0, es1):
                """den + recip for head pair j (heads 2j, 2j+1). DoubleRow
                outputs must sit at column position 0, so each head gets a
                full 128-partition replicated den psum; the recips then read
                partition-aligned halves into one pair tile (h0 rows on 0:64,
                h1 on 64:128) for the single pout normalization mul."""
                recipd = sb2.tile([128, WN], F32, tag="psa_recip", bufs=2,
                                  name=f"r_{j}")
                for half, es in ((0, es0), (1, es1)):
                    ps_den = ps2b.tile([128, WN], F32, tag="psa_den", bufs=1,
                                       name=f"d_{j}_{half}")
                    for cp in range(2):
                        nc.tensor.matmul(
                            ps_den[:], ones8_2[:], es[:, 2 * cp:2 * cp + 2, :],
                            start=(cp == 0), stop=(cp == 1),
                            perf_mode=DR, skip_group_check=True)
                    nc.vector.reciprocal(recipd[half * 64:(half + 1) * 64, :],
                                         ps_den[half * 64:(half + 1) * 64, :])
                return recipd

            def pout_pair(j, wh, es0, es1, recipd):
                """pout for head-pair j over query-window half wh."""
                w0 = wh * (WN // 2)
                for i in range(WIN):
                    po = ps2.tile([128, WN // 2], F32, tag="ps2",
                                  name=f"po_{j}_{i}_{wh}")
                    # DoubleRow requires output column position 0, so only
                    # the half-0 head runs DR; half-1 (psum base 64) uses
                    # plain fp8 matmuls.
                    h0 = 2 * j
                    for cp in range(2):
                        nc.tensor.matmul(
                            po[0:64, :],
                            pv[:, 2 * cp:2 * cp + 2, i, h0 * 64:(h0 + 1) * 64],
                            es0[:, 2 * cp:2 * cp + 2, w0:w0 + WN // 2],
                            start=(cp == 0), stop=(cp == 1),
                            perf_mode=DR, skip_group_check=True)
                    h1 = 2 * j + 1
                    for cc in range(4):
                        nc.tensor.matmul(
                            po[64:128, :],
                            pv[:, cc, i, h1 * 64:(h1 + 1) * 64],
                            es1[:, cc, w0:w0 + WN // 2],
                            start=(cc == 0), stop=(cc == 3),
                            skip_group_check=True)
                    pn = sb2.tile([128, WN // 2], BF16, tag="pn", bufs=4,
                                  name=f"pn_{j}_{i}_{wh}")
                    nc.vector.tensor_mul(pn[:], po[:], recipd[:, w0:w0 + WN // 2])
                    # SBUF-only bf16 add: run it on GPSIMD to keep DVE free
                    # for the psum-reading normalization muls
                    nc.gpsimd.tensor_add(
                        zt[:, j, :].rearrange("p (w i) -> p w i", i=WIN)
                        [:, w0:w0 + WN // 2, i],
                        pn[:],
                        axt[:, j, :].rearrange("p (w s) -> p w s", s=PW)
                        [:, w0:w0 + WN // 2, 1 + i])

            outv = out.ap().rearrange("(g tt p) d -> g p tt d", tt=2, p=128)

            def final_group(g, split_store=False):
                o_sb = sb2.tile([128, 2, D], BF16, tag="osb", bufs=4,
                                name=f"osb_{g}")
                for q in range(2):
                    tt = g * 2 + q
                    ps = ps2.tile([128, D], F32, tag="ps2", name=f"fin_{tt}")
                    for dk in range(4):
                        nc.tensor.matmul(ps[:], zt[:, dk, tt * 128:(tt + 1) * 128],
                                         W["wo"][:, dk, :], start=(dk == 0),
                                         stop=False, skip_group_check=True)
                    # bo via rank-1 matmul; psum evacuation on ACT (idle in
                    # the fin tail) instead of a DVE add
                    nc.tensor.matmul(ps[:], ones_row[:], bo_sb[:], start=False,
                                     stop=True, skip_group_check=True)
                    nc.scalar.copy(o_sb[:, q, :], ps[:])
                    if split_store:
                        nc.sync.dma_start(out=outv[g][:, q, :],
                                          in_=o_sb[:, q, :])
                if not split_store:
                    nc.sync.dma_start(out=outv[g], in_=o_sb[:])

            # pipeline: scores(h+1) | norm(j) once its pair of heads is
            # scored | pout(j) right after; the last pair is split by
            # query-window half so the first finals overlap its second half
            es_store = {0: psa_scores(0)}
            recs = {}
            done_pairs = 0
            for h in range(1, H):
                es_store[h] = psa_scores(h)
                if h % 2 == 1:
                    j = h // 2
                    recs[j] = psa_norm(j, es_store[2 * j], es_store[2 * j + 1])
                if h % 2 == 0 and done_pairs in recs:
                    j = done_pairs
                    pout_pair(j, 0, es_store[2 * j], es_store[2 * j + 1], recs[j])
                    pout_pair(j, 1, es_store[2 * j], es_store[2 * j + 1], recs[j])
                    es_store.pop(2 * j), es_store.pop(2 * j + 1), recs.pop(j)
                    done_pairs += 1
            while done_pairs < 4:
                j = done_pairs
                pout_pair(j, 0, es_store[2 * j], es_store[2 * j + 1], recs[j])
                if j == 3:
                    for g in range(5):
                        final_group(g)
                pout_pair(j, 1, es_store[2 * j], es_store[2 * j + 1], recs[j])
                done_pairs += 1
            for g in range(5, QLEN // 256):
                final_group(g, split_store=(g >= QLEN // 256 - 2))


_NC_CACHE = None


def _get_program():
    global _NC_CACHE
    if _NC_CACHE is None:
        _NC_CACHE = build_program()
    return _NC_CACHE


def _fp8_paired(Wm, perm=None, pair="dc"):
    """Host prep for fp8 DoubleRow lhsT: optional column permutation, x64
    scale, then row pairing. pair="dc": rows (2g+s)*128+p -> [g*128+p, s]
    (matches the wtn dc-group layout); pair="consec": rows 2f+s -> [f, s]
    (matches the uint16-pair input transposes)."""
    w = np.asarray(Wm, np.float32)
    if perm is not None:
        w = w[:, perm]
    w = (w * W8SCALE).astype(ml_dtypes.float8_e4m3fn)
    if pair == "consec":
        return np.ascontiguousarray(w.reshape(256, 2, D))
    # rows: r = g*256 + s*128 + p  ->  out[g*128+p, s, :]
    return np.ascontiguousarray(
        w.reshape(2, 2, 128, D).transpose(0, 2, 1, 3).reshape(256, 2, D))


def _host_consts(Wk, bk, Wv, bv, Wq, bq, ln_g, ln_b, Wpq, bpq, Wpk, bpk, Wo, bo):
    bf = ml_dtypes.bfloat16
    col = lambda b: np.asarray(b, np.float32).reshape(4, 128).T.copy()
    perm = np.asarray(PSA_PERM)
    consts = {
        "wq8": _fp8_paired(Wq, pair="consec"),
        "wk8": _fp8_paired(Wk, pair="consec"),
        "wv": np.asarray(Wv, np.float32).astype(bf),
        "wpq8": _fp8_paired(Wpq, perm),
        "wpk8": _fp8_paired(Wpk, perm),
        "wo": np.asarray(Wo, np.float32).astype(bf),
        "bq64_c": col(np.asarray(bq, np.float32) * W8SCALE), "bk_c": col(bk),
        "bpq_c": col(np.asarray(bpq, np.float32)[perm]),
        "bpk_c": col(np.asarray(bpk, np.float32)[perm]),
        "ln_g_c": col(ln_g), "ln_b_c": col(ln_b),
        "bv_r": np.asarray(bv, np.float32).reshape(1, D).astype(bf),
        "bo_r": np.asarray(bo, np.float32).reshape(1, D).astype(bf),
        "bv_f": np.tile(np.asarray(bv, np.float32).reshape(1, D), (128, 1)).astype(bf),
        "bo_f": np.tile(np.asarray(bo, np.float32).reshape(1, D), (128, 1)).astype(bf),
    }
    m = np.zeros((128, 128), np.float32)
    for g in range(16):
        m[g * PW:(g + 1) * PW, g * PW:(g + 1) * PW] = 1.0
    consts["bmask"] = m.astype(bf)
    return consts


def kernel(k, v, q, query_len, Wk, bk, Wv, bv, Wq, bq, ln_g, ln_b,
           Wpq, bpq, Wpk, bpk, Wo, bo):
    nc = _get_program()
    consts = _host_consts(Wk, bk, Wv, bv, Wq, bq, ln_g, ln_b,
                          Wpq, bpq, Wpk, bpk, Wo, bo)
    k = np.asarray(k, np.float32)
    v = np.asarray(v, np.float32)
    q = np.asarray(q, np.float32)
    in_maps = []
    for b in range(B):
        m = {"q": np.ascontiguousarray(q[b]), "k": np.ascontiguousarray(k[b]),
             "v": np.ascontiguousarray(v[b])}
        m.update(consts)
        in_maps.append(m)
    res = run_bass_kernel_spmd(nc, in_maps, core_ids=list(range(B)))
    return np.stack([np.asarray(res.results[b]["out"], np.float32)
                     for b in range(B)], axis=0)


if __name__ == "__main__":
    nc = build_program()
    print("program built ok")

